# revision 36
# baseline (speedup 1.0000x reference)
"""Trainium2 Bass kernel for the rumor-GCN masked-autoencoder loss.

Strategy (8 NeuronCores, SPMD single NEFF):
  - Nodes partitioned into 8 contiguous ranges (25000 each), then per-core
    RE-ORDERED: unmasked own nodes first [0, UM), masked own compact at
    [UMPAD, UMPAD+MK).  All host-side index maps are relabeled, so the
    permutation is free at runtime and makes (a) mask-aggregation self terms
    a contiguous z2own slice and (b) L1 self-loop terms a contiguous z
    readback -- neither needs dma_gather (~8ns/idx on GpSimd, the dominant
    cost; see /root/problem/microbench.py).
  - z = [x1|x] @ W1 for all 4 GCN heads in one fused [512->512] bf16 matmul
    over the per-core needed set (own + halo, pre-gathered by host).  Row
    scales dinv[src] folded at the copy-out, dinv[dst] at finalize.
  - L1 edge aggregation: flat slot schedule bucketed by (group-of-8-dst-
    blocks, z-window).  Slots sorted by dst block inside each bucket, padded
    only at bucket tails; one dma_gather per bucket; one matmul per
    (K-tile x dst-block-segment) with host-built one-hot S.  Tiles may span
    dst blocks (extra matmul, no extra gather).  Self-loop term z[own]
    added at finalize via direct DMA readback.  global_add_pool is fused
    into the finalize: pool[g] += z2sb^T @ poolS (src-side rewrite).
  - L2 is only needed at masked nodes.  Mask aggregation is node-major
    ([128 masked nodes, 128 feat(on|tgt)] PSUM per block): halo edges
    gathered from the AllGathered z2full with the same flat scheduling;
    self term + b2 bias added at finalize from the contiguous z2own slice.
    Cosine terms reduce along the free dim via accum_out into per-block
    columns; one short wide chain finishes the masked SCE.
  - Each direction's z2 AllGather is issued as soon as that direction's L1
    finishes, overlapping the other direction's aggregation; pooled sums +
    the mask partial go through one small AllReduce.
"""

import sys

import numpy as np

sys.path.insert(0, "/opt/trn_rl_repo")

# ---------------------------------------------------------------- config

WIN = 32768
GB1 = 8       # L1 dst blocks per PSUM group
GB2 = 8       # L2 mask blocks per PSUM group
NF = 2048     # P1 column chunk

N, E, G, M, C = 200000, 400000, 128, 100000, 8
OWN = N // C

_WNAMES = [p + s for p in ("on_td", "on_bu", "tgt_td", "tgt_bu")
           for s in ("_W1", "_b1", "_W2", "_b2")]


def _rep16(idx_flat, nslots):
    """int16 index list -> [128, nslots//16] layout (16-part wrap, 8x rep)."""
    blk = np.zeros((16, nslots // 16), dtype=np.int16)
    k = np.arange(len(idx_flat))
    blk[k % 16, k // 16] = idx_flat
    return np.tile(blk, (8, 1))


def _bcast(vec, parts=128):
    return np.broadcast_to(np.asarray(vec)[None, :], (parts, len(vec))).copy()


def _pad128(n):
    return -(-n // 128) * 128


# ---------------------------------------------------------------- host prep

W1SCALE = 16.0  # lift fp8 W1 out of the subnormal range; undone in dloc


def host_prep(inp):
    import ml_dtypes
    bf16 = ml_dtypes.bfloat16
    f8 = ml_dtypes.float8_e4m3
    x = np.asarray(inp["x"], np.float32)
    token = np.asarray(inp["enc_mask_token"], np.float32).reshape(-1)
    ei = np.asarray(inp["edge_index"])
    src, dst = ei[0].astype(np.int64), ei[1].astype(np.int64)
    batch = np.asarray(inp["batch"]).astype(np.int64)
    mask_nodes = np.asarray(inp["mask_nodes"]).astype(np.int64)
    W = {k: np.asarray(inp[k], np.float32) for k in _WNAMES}

    dinv = [
        (1.0 / np.sqrt(np.bincount(dst, minlength=N) + 1.0)).astype(np.float32),
        (1.0 / np.sqrt(np.bincount(src, minlength=N) + 1.0)).astype(np.float32),
    ]
    is_masked = np.zeros(N, bool)
    is_masked[mask_nodes] = True
    mcnt_global = np.bincount(mask_nodes, minlength=N).astype(np.float32)
    xbf = x.astype(bf16)

    # ---- per-core own-node permutation: unmasked first, masked at tail
    um_nodes, mk_nodes = [], []
    for ci in range(C):
        lo = ci * OWN
        m = is_masked[lo:lo + OWN]
        um_nodes.append(np.where(~m)[0] + lo)
        mk_nodes.append(np.where(m)[0] + lo)
    UM = [len(a) for a in um_nodes]
    MK = [len(a) for a in mk_nodes]
    UMPAD = _pad128(max(UM))
    MKPAD = _pad128(max(MK))
    OWNP = UMPAD + MKPAD
    NB = OWNP // 128
    MB = MKPAD // 128
    NPAD = C * OWNP
    NW2 = -(-NPAD // WIN)

    pos_own = []          # [C] array [OWN] -> p-order position
    for ci in range(C):
        lo = ci * OWN
        p = np.empty(OWN, np.int64)
        p[um_nodes[ci] - lo] = np.arange(UM[ci])
        p[mk_nodes[ci] - lo] = UMPAD + np.arange(MK[ci])
        pos_own.append(p)

    # ---- per-core edge lists (dir 0 = TD: dst-agg; dir 1 = BU: src-agg)
    core_edges = []       # [core][dir] -> (adst_local, asrc_global)
    for ci in range(C):
        lo, hi = ci * OWN, (ci + 1) * OWN
        per = []
        for d in range(2):
            ad, as_ = (dst, src) if d == 0 else (src, dst)
            sel = (ad >= lo) & (ad < hi)
            per.append((ad[sel] - lo, as_[sel]))
        core_edges.append(per)

    # ---- halo sets (union over both dirs), split unmasked/masked
    halo_um, halo_mk = [], []
    for ci in range(C):
        lo, hi = ci * OWN, (ci + 1) * OWN
        srcs = np.unique(np.concatenate(
            [core_edges[ci][0][1], core_edges[ci][1][1]]))
        srcs = srcs[(srcs < lo) | (srcs >= hi)]
        halo_um.append(srcs[~is_masked[srcs]])
        halo_mk.append(srcs[is_masked[srcs]])
    HU = [len(a) for a in halo_um]
    HM = [len(a) for a in halo_mk]
    HUPAD = _pad128(max(HU))
    HMPAD = _pad128(max(HM))
    RT = OWNP + HUPAD + HMPAD
    NW1 = -(-RT // WIN)

    # z-row map per core: global node -> z row (own p-order | halo)
    zrow = []
    for ci in range(C):
        lo = ci * OWN
        zm = np.full(N, -1, np.int64)
        zm[lo + np.arange(OWN)] = pos_own[ci]
        zm[halo_um[ci]] = OWNP + np.arange(HU[ci])
        zm[halo_mk[ci]] = OWNP + HUPAD + np.arange(HM[ci])
        zrow.append(zm)

    # P1 sections: (row0, rowlen, is_masked_section)
    sections = [(0, UMPAD, False), (UMPAD, MKPAD, True),
                (OWNP, HUPAD, False), (OWNP + HUPAD, HMPAD, True)]

    # ---- generic flat scheduler -----------------------------------------
    def build_flat(percore_bwrlv, NBLK, GBX, NWX):
        """percore_bwrlv: per core (blk, win, rel, lane, val) arrays.
        Returns sched dict + per-core (S, idx) builders' inputs."""
        NG = -(-NBLK // GBX)
        cnt = np.zeros((C, NG, NWX), np.int64)
        for ci in range(C):
            b, w = percore_bwrlv[ci][0], percore_bwrlv[ci][1]
            np.add.at(cnt, (ci, b // GBX, w), 1)
        KT = -(-cnt.max(axis=0) // 128)          # [NG, NWX]
        ktoff = np.zeros((NG, NWX), np.int64)
        acc = 0
        for g in range(NG):
            for w in range(NWX):
                ktoff[g, w] = acc
                acc += KT[g, w]
        TOTKT = acc
        # per-core slot/op computation
        per_core = []
        opset = {}
        for ci in range(C):
            b, w, rel, lane, val = percore_bwrlv[ci]
            g = b // GBX
            bucket = g * NWX + w
            order = np.lexsort((np.arange(len(b)), b, bucket))
            bs, ws, gs = b[order], w[order], g[order]
            rels, lanes, vals = rel[order], lane[order], val[order]
            buck = gs * NWX + ws
            segchange = np.r_[True, buck[1:] != buck[:-1]]
            segstart = np.maximum.accumulate(
                np.where(segchange, np.arange(len(buck)), 0))
            pos = np.arange(len(buck)) - segstart
            kt = ktoff[gs, ws] + pos // 128
            sit = pos % 128
            per_core.append((kt, sit, bs, rels, lanes, vals))
            for key in set(zip(kt.tolist(), bs.tolist())):
                opset[key] = True
        ops = sorted(opset.keys())               # (kt, blk) in emission order
        opidx = {key: o for o, key in enumerate(ops)}
        NOP = len(ops)
        # group structure for emission
        groups = []
        for g in range(NG):
            gops = [(w, int(ktoff[g, w]), int(KT[g, w]))
                    for w in range(NWX) if KT[g, w] > 0]
            kt_lo = int(ktoff[g].min()) if gops else 0
            kt_hi = kt_lo + sum(nk for _, _, nk in gops)
            mops = [(kt, blk, opidx[(kt, blk)]) for (kt, blk) in ops
                    if kt_lo <= kt < kt_hi] if gops else []
            # start/stop per block within this group
            first, last = {}, {}
            for i, (kt, blk, o) in enumerate(mops):
                if blk not in first:
                    first[blk] = i
                last[blk] = i
            flags = [(kt, blk, o, first[blk] == i, last[blk] == i)
                     for i, (kt, blk, o) in enumerate(mops)]
            groups.append(dict(gops=gops, mops=flags, kt_lo=kt_lo,
                               nk=kt_hi - kt_lo,
                               blocks=list(range(g * GBX,
                                                 min((g + 1) * GBX, NBLK)))))
        return dict(KT=KT, ktoff=ktoff, TOTKT=TOTKT, NOP=NOP, groups=groups,
                    per_core=per_core, opidx=opidx, cnt=cnt)

    def fill_slots(sched, ci, sdtype):
        kt, sit, bs, rels, lanes, vals = sched["per_core"][ci]
        nslots = sched["TOTKT"] * 128
        idx_flat = np.zeros(nslots, np.int64)
        idx_flat[kt * 128 + sit] = rels
        assert rels.max(initial=0) < WIN
        S = np.zeros((128, sched["NOP"] * 128), np.float32)
        o = np.array([sched["opidx"][(int(k), int(b))]
                      for k, b in zip(kt, bs)], np.int64)
        np.add.at(S, (sit, o * 128 + lanes), vals)
        return (S.astype(sdtype),
                _rep16(idx_flat.astype(np.int16), nslots))

    # ---- L1 schedules ----------------------------------------------------
    sched1 = []
    for d in range(2):
        percore = []
        for ci in range(C):
            adst, asrc = core_edges[ci][d]
            dpos = pos_own[ci][adst]
            row = zrow[ci][asrc]
            assert (row >= 0).all()
            percore.append((dpos // 128, row // WIN, row % WIN, dpos % 128,
                            dinv[d][asrc].astype(np.float32)))
        sched1.append(build_flat(percore, NB, GB1, NW1))

    # ---- L2 mask schedules (halo only; self via direct slice) -----------
    mk_rank = []          # [C] array [OWN] -> rank in masked list or -1
    for ci in range(C):
        lo = ci * OWN
        r = np.full(OWN, -1, np.int64)
        r[mk_nodes[ci] - lo] = np.arange(MK[ci])
        mk_rank.append(r)

    sched2 = []
    for d in range(2):
        percore = []
        for ci in range(C):
            lo = ci * OWN
            ad_g, as_g = (dst, src) if d == 0 else (src, dst)
            sel = ((ad_g >= lo) & (ad_g < lo + OWN)
                   & is_masked[np.clip(ad_g, 0, N - 1)])
            adst = ad_g[sel] - lo
            md = mk_rank[ci][adst]
            sj = as_g[sel] // OWN        # owner core of source
            srow = sj * OWNP + pos_own_of(sj, as_g[sel] - sj * OWN, pos_own)
            percore.append((md // 128, srow // WIN, srow % WIN, md % 128,
                            dinv[d][lo + adst].astype(np.float32)))
        sched2.append(build_flat(percore, MB, GB2, NW2))

    # ---- per-core inputs -------------------------------------------------
    w1all = (np.concatenate([W["on_td_W1"], W["tgt_td_W1"],
                             W["on_bu_W1"], W["tgt_bu_W1"]], axis=1)
             * W1SCALE).astype(f8)
    w2_td = np.concatenate([W["on_td_W2"], W["tgt_td_W2"]], axis=1).astype(bf16)
    w2_bu = np.concatenate([W["on_bu_W2"], W["tgt_bu_W2"]], axis=1).astype(bf16)
    ton = np.concatenate([token @ W["on_td_W1"], token @ W["on_bu_W1"]])
    tonbc = _bcast(ton).astype(bf16)
    b1bc_td = _bcast(np.concatenate([W["on_td_b1"], W["tgt_td_b1"]]))
    b1bc_bu = _bcast(np.concatenate([W["on_bu_b1"], W["tgt_bu_b1"]]))
    b2bc_td = _bcast(np.concatenate([W["on_td_b2"], W["tgt_td_b2"]]))
    b2bc_bu = _bcast(np.concatenate([W["on_bu_b2"], W["tgt_bu_b2"]]))
    b2col = np.stack(
        [W["on_td_b2"], W["tgt_td_b2"], W["on_bu_b2"], W["tgt_bu_b2"]],
        axis=1).astype(np.float32)                         # [64, 4]
    ones = np.ones((128, 1), np.float32)
    gcount = np.bincount(batch, minlength=G).astype(np.float32)
    cntbc = np.broadcast_to(gcount[None, :128], (128, 128)).copy()

    in_maps = []
    for ci in range(C):
        lo = ci * OWN
        # xT in z-row order
        xT = np.zeros((512, RT), f8)
        xT[:, 0:UM[ci]] = x[um_nodes[ci]].T
        xT[:, UMPAD:UMPAD + MK[ci]] = x[mk_nodes[ci]].T
        xT[:, OWNP:OWNP + HU[ci]] = x[halo_um[ci]].T
        xT[:, OWNP + HUPAD:OWNP + HUPAD + HM[ci]] = x[halo_mk[ci]].T

        def dstarr(dv):
            a = np.ones(OWNP, np.float32)
            a[0:UM[ci]] = dv[um_nodes[ci]]
            a[UMPAD:UMPAD + MK[ci]] = dv[mk_nodes[ci]]
            return np.ascontiguousarray(a.reshape(-1, 128).T)

        def colarr(vals_mk, fill=0.0):
            a = np.full(MKPAD, fill, np.float32)
            a[0:MK[ci]] = vals_mk
            return np.ascontiguousarray(a.reshape(-1, 128).T)  # [128, MB]

        m = dict(xT=xT,
                 ddst_td=dstarr(dinv[0]), ddst_bu=dstarr(dinv[1]),
                 swv_td=colarr(dinv[0][mk_nodes[ci]]),
                 swv_bu=colarr(dinv[1][mk_nodes[ci]]),
                 mcvw=colarr(mcnt_global[mk_nodes[ci]]))
        for d, nm in ((0, "td"), (1, "bu")):
            S, idx = fill_slots(sched1[d], ci, f8)
            m[f"s_{nm}1"], m[f"i_{nm}1"] = S, idx
            S2, idx2 = fill_slots(sched2[d], ci, bf16)
            m[f"s2_{nm}"], m[f"i2_{nm}"] = S2, idx2
            # pool S: out-edges of own nodes + self, grouped by graph
            ad, as_ = (dst, src) if d == 0 else (src, dst)
            dv = dinv[d]
            sel = (as_ >= lo) & (as_ < lo + OWN)
            j = pos_own[ci][as_[sel] - lo]
            gg = batch[ad[sel]]
            v = dv[ad[sel]]
            pp = np.zeros((128, NB * 128), np.float32)
            np.add.at(pp, (j % 128, (j // 128) * 128 + gg), v)
            jj = pos_own[ci]
            np.add.at(pp, (jj % 128, (jj // 128) * 128 + batch[lo:lo + OWN]),
                      dv[lo:lo + OWN])
            m[f"pools_{nm}"] = pp.astype(bf16)
        m.update(w1all=w1all, w2_td=w2_td, w2_bu=w2_bu, tonbc=tonbc,
                 b1bc_td=b1bc_td, b1bc_bu=b1bc_bu,
                 b2bc_td=b2bc_td, b2bc_bu=b2bc_bu, b2col=b2col,
                 ones=ones, cntbc=cntbc)
        in_maps.append(m)

    meta = dict(RT=RT, NW1=NW1, NW2=NW2, NB=NB, MB=MB, OWNP=OWNP,
                UMPAD=UMPAD, MKPAD=MKPAD, NPAD=NPAD,
                sections=sections, sched1=sched1, sched2=sched2)
    return meta, in_maps


def pos_own_of(owner_cores, local_idx, pos_own):
    """vectorized pos_own lookup across owner cores"""
    out = np.empty(len(local_idx), np.int64)
    for j in np.unique(owner_cores):
        sel = owner_cores == j
        out[sel] = pos_own[j][local_idx[sel]]
    return out


# ---------------------------------------------------------------- program

def build_program(meta):
    import concourse.bass as bass
    import concourse.bacc as bacc
    import concourse.mybir as mybir
    import concourse.tile as tile
    from concourse.masks import make_identity

    RT, NB, MB = meta["RT"], meta["NB"], meta["MB"]
    NW1, NW2 = meta["NW1"], meta["NW2"]
    OWNP, UMPAD, NPAD = meta["OWNP"], meta["UMPAD"], meta["NPAD"]
    f32, bf, i16 = mybir.dt.float32, mybir.dt.bfloat16, mybir.dt.int16
    f8 = mybir.dt.float8e4
    MUL, ADD = mybir.AluOpType.mult, mybir.AluOpType.add

    nc = bacc.Bacc("TRN2", target_bir_lowering=False, debug=False,
                   num_devices=C)

    def din(name, shape, dt):
        return nc.dram_tensor(name, shape, dt, kind="ExternalInput")

    xT = din("xT", [512, RT], f8)
    ddst = [din("ddst_td", [128, NB], f32), din("ddst_bu", [128, NB], f32)]
    s1 = [din("s_td1", [128, meta["sched1"][0]["NOP"] * 128], f8),
          din("s_bu1", [128, meta["sched1"][1]["NOP"] * 128], f8)]
    i1 = [din("i_td1", [128, meta["sched1"][0]["TOTKT"] * 8], i16),
          din("i_bu1", [128, meta["sched1"][1]["TOTKT"] * 8], i16)]
    s2 = [din("s2_td", [128, meta["sched2"][0]["NOP"] * 128], bf),
          din("s2_bu", [128, meta["sched2"][1]["NOP"] * 128], bf)]
    i2 = [din("i2_td", [128, meta["sched2"][0]["TOTKT"] * 8], i16),
          din("i2_bu", [128, meta["sched2"][1]["TOTKT"] * 8], i16)]
    pools_t = [din("pools_td", [128, NB * 128], bf),
               din("pools_bu", [128, NB * 128], bf)]
    swv_t = [din("swv_td", [128, MB], f32), din("swv_bu", [128, MB], f32)]
    mcvw_t = din("mcvw", [128, MB], f32)
    w1all = din("w1all", [512, 512], f8)
    w2 = [din("w2_td", [128, 128], bf), din("w2_bu", [128, 128], bf)]
    tonbc = din("tonbc", [128, 256], bf)
    b1bc = [din("b1bc_td", [128, 256], f32), din("b1bc_bu", [128, 256], f32)]
    b2bc = [din("b2bc_td", [128, 128], f32), din("b2bc_bu", [128, 128], f32)]
    b2col_t = din("b2col", [64, 4], f32)
    ones_t = din("ones", [128, 1], f32)
    cntbc_t = din("cntbc", [128, 128], f32)
    loss_t = nc.dram_tensor("loss", [1, 1], f32, kind="ExternalOutput")

    z_ws = [nc.dram_tensor(f"zarr{w}", [min(WIN, RT - w * WIN), 512], f8,
                           kind="Internal")
            for w in range(NW1)]

    with tile.TileContext(nc) as tc:
        with (
            tc.tile_pool(name="const", bufs=1) as cpool,
            tc.tile_pool(name="dram", bufs=1, space="DRAM") as dpool,
        ):
            z2own = [dpool.tile([OWNP, 128], bf, tag=f"z2own{d}",
                                name=f"z2own{d}") for d in range(2)]
            z2full = [dpool.tile([NPAD, 128], bf, addr_space="Shared",
                                 tag=f"z2full{d}", name=f"z2full{d}")
                      for d in range(2)]
            ar_in = dpool.tile([128, 520], f32, tag="arin", name="arin")
            ar_out = dpool.tile([128, 520], f32, addr_space="Shared",
                                tag="arout", name="arout")

            ident = cpool.tile([128, 128], bf)
            make_identity(nc, ident[:])
            w1sb = cpool.tile([128, 4 * 512], f8)
            for k in range(4):
                nc.sync.dma_start(out=w1sb[:, k * 512:(k + 1) * 512],
                                  in_=w1all[k * 128:(k + 1) * 128, :])
            w2sb = [cpool.tile([128, 128], bf, tag=f"w2_{d}", name=f"w2_{d}")
                    for d in range(2)]
            tonsb = cpool.tile([128, 256], bf)
            b1sb = [cpool.tile([128, 256], f32, tag=f"b1_{d}", name=f"b1_{d}")
                    for d in range(2)]
            b2sb = [cpool.tile([128, 128], f32, tag=f"b2_{d}", name=f"b2_{d}")
                    for d in range(2)]
            ddsb = [cpool.tile([128, NB], f32, tag=f"dd_{d}", name=f"dd_{d}")
                    for d in range(2)]
            swsb = [cpool.tile([128, MB], f32, tag=f"sw_{d}", name=f"sw_{d}")
                    for d in range(2)]
            mcsb = cpool.tile([128, MB], f32)
            onesb = cpool.tile([128, 1], f32)
            nc.sync.dma_start(out=tonsb[:], in_=tonbc[:, :])
            nc.sync.dma_start(out=onesb[:], in_=ones_t[:, :])
            nc.sync.dma_start(out=mcsb[:], in_=mcvw_t[:, :])
            for d in range(2):
                nc.sync.dma_start(out=w2sb[d][:], in_=w2[d][:, :])
                nc.sync.dma_start(out=b1sb[d][:], in_=b1bc[d][:, :])
                nc.sync.dma_start(out=b2sb[d][:], in_=b2bc[d][:, :])
                nc.sync.dma_start(out=ddsb[d][:], in_=ddst[d][:, :])
                nc.sync.dma_start(out=swsb[d][:], in_=swv_t[d][:, :])

            # ================= P1: z = scaled([x1|x] @ W1-fused) ==========
            with (
                tc.tile_pool(name="xk", bufs=2) as xkp,
                tc.tile_pool(name="zsb", bufs=3) as zsp,
                tc.tile_pool(name="pz", bufs=2, space="PSUM") as pzp,
            ):
                DR = mybir.MatmulPerfMode.DoubleRow
                jpar = 0
                for (r0, rlen, msk) in meta["sections"]:
                    for off in range(0, rlen, NF):
                        nf = min(NF, rlen - off)
                        xk = xkp.tile([128, 4 * NF], f8, tag="xk", name="xk")
                        for k in range(4):
                            nc.sync.dma_start(
                                out=xk[:, k * NF:k * NF + nf],
                                in_=xT[k * 128:(k + 1) * 128,
                                       r0 + off:r0 + off + nf])
                        xk3 = xk[:].rearrange("p (k n) -> p k n", k=4, n=NF)
                        w13 = w1sb[:].rearrange("p (k n) -> p k n", k=4,
                                                n=512)
                        for j in range(nf // 128):
                            row = r0 + off + j * 128
                            jpar += 1
                            zs = zsp.tile([128, 512], f8, tag="zs", name="zs")
                            if not msk:
                                ps = pzp.tile([128, 512], f32, tag="pz",
                                              name="pz")
                                for k in range(0, 4, 2):
                                    nc.tensor.matmul(
                                        out=ps[:],
                                        lhsT=xk3[:, k:k + 2,
                                                 j * 128:(j + 1) * 128],
                                        rhs=w13[:, k:k + 2, :],
                                        start=(k == 0), stop=(k == 2),
                                        perf_mode=DR)
                                if jpar % 3 == 0:
                                    nc.scalar.activation(
                                        out=zs[:], in_=ps[:],
                                        func=mybir.ActivationFunctionType.Copy,
                                        scale=1.0 / W1SCALE)
                                else:
                                    nc.vector.tensor_scalar(
                                        out=zs[:], in0=ps[:],
                                        scalar1=1.0 / W1SCALE,
                                        scalar2=None, op0=MUL)
                            else:
                                ps = pzp.tile([128, 512], f32, tag="pz",
                                              name="pz")
                                for h in range(2):
                                    c0 = h * 256 + 128
                                    for k in range(0, 4, 2):
                                        nc.tensor.matmul(
                                            out=ps[:, h * 128:(h + 1) * 128],
                                            lhsT=xk3[:, k:k + 2,
                                                     j * 128:(j + 1) * 128],
                                            rhs=w13[:, k:k + 2, c0:c0 + 128],
                                            start=(k == 0), stop=(k == 2),
                                            perf_mode=DR)
                                for h in range(2):
                                    nc.vector.tensor_copy(
                                        out=zs[:, h * 256:h * 256 + 128],
                                        in_=tonsb[:, h * 128:(h + 1) * 128])
                                    if jpar % 3 == 0:
                                        nc.scalar.activation(
                                            out=zs[:, h * 256 + 128:
                                                   (h + 1) * 256],
                                            in_=ps[:, h * 128:(h + 1) * 128],
                                            func=mybir.ActivationFunctionType.Copy,
                                            scale=1.0 / W1SCALE)
                                    else:
                                        nc.vector.tensor_scalar(
                                            out=zs[:, h * 256 + 128:
                                                   (h + 1) * 256],
                                            in0=ps[:, h * 128:(h + 1) * 128],
                                            scalar1=1.0 / W1SCALE,
                                            scalar2=None, op0=MUL)
                            zw = row // WIN
                            zr = row - zw * WIN
                            nc.sync.dma_start(out=z_ws[zw][zr:zr + 128, :],
                                              in_=zs[:])

            # ===== L1 agg + finalize (z2 + fused pool), per direction =====
            poolpool_cm = tc.tile_pool(name="plps", bufs=1, space="PSUM")
            poolpool = poolpool_cm.__enter__()
            poolps = poolpool.tile([128, 512], f32, tag="pl", name="pl")

            def l1_dir(d):
                sch = meta["sched1"][d]
                wlen = lambda w: min(WIN, RT - w * WIN)
                with (
                    tc.tile_pool(name=f"g1{d}", bufs=15) as gp,
                    tc.tile_pool(name=f"sI1{d}", bufs=3) as sp,
                    tc.tile_pool(name=f"ix1{d}", bufs=1) as ip,
                    tc.tile_pool(name=f"ps1{d}", bufs=2) as pwp,
                    tc.tile_pool(name=f"fin1{d}", bufs=3) as fp,
                    tc.tile_pool(name=f"h1q{d}", bufs=20) as h1p,
                    tc.tile_pool(name=f"zrb{d}", bufs=3) as zrp,
                    tc.tile_pool(name=f"agg{d}", bufs=1, space="PSUM") as ap,
                    tc.tile_pool(name=f"tr{d}", bufs=2, space="PSUM") as trp,
                    tc.tile_pool(name=f"z2p{d}", bufs=1, space="PSUM") as z2p,
                ):
                    # stage B (transpose -> @W2 -> scale -> z2own write +
                    # fused pool matmuls), decoupled from the agg pipeline
                    # via the deep h1 tile pool and one-group emission skew.
                    def stage_b(blk, bi, h1, pst):
                        trt = trp.tile([128, 256], bf, tag="t", name="t")
                        nc.tensor.transpose(
                            out=trt[:, 0:128], in_=h1[:, 0:128],
                            identity=ident[:])
                        nc.tensor.transpose(
                            out=trt[:, 128:256], in_=h1[:, 128:256],
                            identity=ident[:])
                        h1T = fp.tile([128, 256], bf, tag="h1T", name="h1T")
                        nc.scalar.copy(out=h1T[:], in_=trt[:])
                        z2ps = z2p.tile([128, 128], f32, tag="z2", name="z2")
                        nc.tensor.matmul(out=z2ps[:, 0:64],
                                         lhsT=h1T[:, 0:128],
                                         rhs=w2sb[d][:, 0:64],
                                         start=True, stop=True)
                        nc.tensor.matmul(out=z2ps[:, 64:128],
                                         lhsT=h1T[:, 128:256],
                                         rhs=w2sb[d][:, 64:128],
                                         start=True, stop=True)
                        z2sb = fp.tile([128, 128], bf, tag="z2sb",
                                       name="z2sb")
                        nc.vector.tensor_scalar(
                            out=z2sb[:], in0=z2ps[:],
                            scalar1=ddsb[d][:, blk:blk + 1],
                            scalar2=None, op0=MUL)
                        nc.sync.dma_start(
                            out=z2own[d][blk * 128:(blk + 1) * 128, :],
                            in_=z2sb[:])
                        nc.tensor.matmul(
                            out=poolps[0:64, d * 256:d * 256 + 128],
                            lhsT=z2sb[:, 0:64],
                            rhs=pst[:, bi * 128:(bi + 1) * 128],
                            start=(blk == 0), stop=(blk == NB - 1),
                            skip_group_check=True)
                        nc.tensor.matmul(
                            out=poolps[0:64, d * 256 + 128:d * 256 + 256],
                            lhsT=z2sb[:, 64:128],
                            rhs=pst[:, bi * 128:(bi + 1) * 128],
                            start=(blk == 0), stop=(blk == NB - 1),
                            skip_group_check=True)

                    itall = ip.tile([128, max(sch["TOTKT"], 1) * 8], i16,
                                    tag="ia", name="ia")
                    nc.gpsimd.dma_start(out=itall[:], in_=i1[d][:, :])
                    pending = []
                    groups = list(enumerate(sch["groups"]))
                    WAVE = 13
                    for w0 in range(0, len(groups), WAVE):
                        wave = groups[w0:w0 + WAVE]
                        slabs = {}
                        for g, grp in wave:
                            if grp["gops"]:
                                slabs[g] = gp.tile([128, grp["nk"] * 256],
                                                   f8, tag="g", name="g")
                        # window-major gather emission: w0 gathers only wait
                        # for z window 0 (ready ~35% into P1)
                        for w in range(meta["NW1"]):
                            for g, grp in wave:
                                for (ww, ktb, nkw) in grp["gops"]:
                                    if ww != w:
                                        continue
                                    o = ktb - grp["kt_lo"]
                                    nc.gpsimd.dma_gather(
                                        slabs[g][:, o * 256:(o + nkw) * 256]
                                        .rearrange("p (k e) -> p k e",
                                                   k=nkw, e=256),
                                        z_ws[w][0:wlen(w),
                                                256 * d:256 * d + 256],
                                        itall[:, ktb * 8:(ktb + nkw) * 8],
                                        nkw * 128, nkw * 128, 256,
                                        elem_step=512, single_packet=False)
                        for g, grp in wave:
                            blocks = grp["blocks"]
                            nops = len(grp["mops"])
                            gt = slabs.get(g)
                            if nops:
                                st = sp.tile([128, nops * 128], f8, tag="s",
                                             name="s")
                                nc.scalar.dma_start(
                                    out=st[:],
                                    in_=s1[d][:, grp["mops"][0][2] * 128:
                                              (grp["mops"][0][2] + nops)
                                              * 128])
                                aps = ap.tile([128, len(blocks) * 256], f32,
                                              tag="a", name="a")
                                o0 = grp["mops"][0][2]
                                for (kt, blk, o, st_f, sp_f) in grp["mops"]:
                                    bi = blk - blocks[0]
                                    nc.tensor.matmul(
                                        out=aps[:, bi * 256:(bi + 1) * 256],
                                        lhsT=st[:, (o - o0) * 128:
                                                (o - o0 + 1) * 128],
                                        rhs=gt[:, (kt - grp["kt_lo"]) * 256:
                                               (kt - grp["kt_lo"] + 1)
                                               * 256],
                                        start=st_f, stop=sp_f,
                                        skip_group_check=True)
                            has = {blk for (_, blk, _, _, _) in grp["mops"]}
                            # pool S slab for this group
                            pst = pwp.tile([128, len(blocks) * 128], bf,
                                           tag="ps", name="ps")
                            nc.sync.dma_start(
                                out=pst[:],
                                in_=pools_t[d][:, blocks[0] * 128:
                                               (blocks[0] + len(blocks))
                                               * 128])
                            newly = []
                            for blk in blocks:
                                bi = blk - blocks[0]
                                zrb = zrp.tile([128, 256], f8, tag="zr",
                                               name="zr")
                                nc.sync.dma_start(
                                    out=zrb[:],
                                    in_=z_ws[0][blk * 128:(blk + 1) * 128,
                                                256 * d:256 * d + 256])
                                hs = fp.tile([128, 256], f32, tag="hs",
                                             name="hs")
                                if blk in has:
                                    # hs = h_self*dinv_dst + agg
                                    nc.vector.scalar_tensor_tensor(
                                        out=hs[:], in0=zrb[:],
                                        scalar=ddsb[d][:, blk:blk + 1],
                                        in1=aps[:, bi * 256:(bi + 1) * 256],
                                        op0=MUL, op1=ADD)
                                else:
                                    nc.vector.tensor_scalar(
                                        out=hs[:], in0=zrb[:],
                                        scalar1=ddsb[d][:, blk:blk + 1],
                                        scalar2=None, op0=MUL)
                                # h1 = relu(hs*ddst + b1)
                                nc.vector.scalar_tensor_tensor(
                                    out=hs[:], in0=hs[:],
                                    scalar=ddsb[d][:, blk:blk + 1],
                                    in1=b1sb[d][:, 0:256], op0=MUL, op1=ADD)
                                h1 = h1p.tile([128, 256], bf, tag="h1",
                                              name="h1")
                                nc.scalar.activation(
                                    out=h1[:], in_=hs[:],
                                    func=mybir.ActivationFunctionType.Relu)
                                newly.append((blk, bi, h1, pst))
                            for item in pending:
                                stage_b(*item)
                            pending = newly
                    for item in pending:
                        stage_b(*item)

            def allgather(d):
                nc.gpsimd.collective_compute(
                    "AllGather", mybir.AluOpType.bypass,
                    replica_groups=[list(range(C))],
                    ins=[z2own[d].opt()], outs=[z2full[d].opt()])

            l1_dir(0)
            allgather(0)
            l1_dir(1)
            allgather(1)

            # drain pooled sums
            arsb = cpool.tile([128, 520], f32)
            nc.vector.memset(arsb[:], 0.0)
            for d in range(2):
                nc.vector.tensor_copy(out=arsb[0:64, d * 256:(d + 1) * 256],
                                      in_=poolps[0:64, d * 256:(d + 1) * 256])
            poolpool_cm.__exit__(None, None, None)

            # ========== L2 mask aggregation (node-major) ==================
            # wide per-dir product tiles
            prodw = [[cpool.tile([128, MB], f32, tag=f"pw{d}{q}",
                                 name=f"pw{d}{q}") for q in range(3)]
                     for d in range(2)]
            for d in range(2):
                for q in range(3):
                    nc.vector.memset(prodw[d][q][:], 0.0)

            def l2_dir(d):
                sch = meta["sched2"][d]
                wlen = lambda w: min(WIN, NPAD - w * WIN)
                with (
                    tc.tile_pool(name=f"g2{d}", bufs=3) as gp,
                    tc.tile_pool(name=f"sI2{d}", bufs=3) as sp,
                    tc.tile_pool(name=f"ix2{d}", bufs=3) as ip,
                    tc.tile_pool(name=f"fin2{d}", bufs=3) as fp,
                    tc.tile_pool(name=f"zsl{d}", bufs=3) as zp,
                    tc.tile_pool(name=f"mag{d}", bufs=2, space="PSUM") as ap,
                ):
                    for g, grp in enumerate(sch["groups"]):
                        blocks = grp["blocks"]
                        nops = len(grp["mops"])
                        nk = grp["nk"]
                        if nops:
                            st = sp.tile([128, nops * 128], bf, tag="s",
                                         name="s")
                            nc.sync.dma_start(
                                out=st[:],
                                in_=s2[d][:, grp["mops"][0][2] * 128:
                                          (grp["mops"][0][2] + nops) * 128])
                            it = ip.tile([128, nk * 8], i16, tag="i", name="i")
                            nc.sync.dma_start(
                                out=it[:], in_=i2[d][:, grp["kt_lo"] * 8:
                                                     (grp["kt_lo"] + nk) * 8])
                            gt = gp.tile([128, nk * 128], bf, tag="g",
                                         name="g")
                            for w, ktb, nkw in grp["gops"]:
                                o = ktb - grp["kt_lo"]
                                nc.gpsimd.dma_gather(
                                    gt[:, o * 128:(o + nkw) * 128].rearrange(
                                        "p (k e) -> p k e", k=nkw, e=128),
                                    z2full[d][w * WIN:w * WIN + wlen(w), :],
                                    it[:, o * 8:(o + nkw) * 8],
                                    nkw * 128, nkw * 128, 128,
                                    elem_step=None, single_packet=False)
                            aps = ap.tile([128, len(blocks) * 128], f32,
                                          tag="a", name="a")
                            o0 = grp["mops"][0][2]
                            for (kt, blk, o, st_f, sp_f) in grp["mops"]:
                                bi = blk - blocks[0]
                                nc.tensor.matmul(
                                    out=aps[:, bi * 128:(bi + 1) * 128],
                                    lhsT=st[:, (o - o0) * 128:
                                            (o - o0 + 1) * 128],
                                    rhs=gt[:, (kt - grp["kt_lo"]) * 128:
                                           (kt - grp["kt_lo"] + 1) * 128],
                                    start=st_f, stop=sp_f,
                                    skip_group_check=True)
                        has = {blk for (_, blk, _, _, _) in grp["mops"]}
                        for blk in blocks:
                            bi = blk - blocks[0]
                            zsl = zp.tile([128, 128], bf, tag="zs", name="zs")
                            nc.sync.dma_start(
                                out=zsl[:],
                                in_=z2own[d][UMPAD + blk * 128:
                                             UMPAD + (blk + 1) * 128, :])
                            hs = fp.tile([128, 128], f32, tag="hs", name="hs")
                            # hs = z_self*swv (+ agg)
                            if blk in has:
                                nc.vector.scalar_tensor_tensor(
                                    out=hs[:], in0=zsl[:],
                                    scalar=swsb[d][:, blk:blk + 1],
                                    in1=aps[:, bi * 128:(bi + 1) * 128],
                                    op0=MUL, op1=ADD)
                            else:
                                nc.vector.tensor_scalar(
                                    out=hs[:], in0=zsl[:],
                                    scalar1=swsb[d][:, blk:blk + 1],
                                    scalar2=None, op0=MUL)
                            nc.vector.tensor_tensor(
                                out=hs[:], in0=hs[:], in1=b2sb[d][:, 0:128],
                                op=ADD)
                            # products (accumulate over 64-feat free dim)
                            scr = fp.tile([128, 64], f32, tag="sc", name="sc")
                            for q, (p0, p1) in enumerate(
                                    ((0, 64), (0, 0), (64, 64))):
                                nc.vector.scalar_tensor_tensor(
                                    out=scr[:], in0=hs[:, p0:p0 + 64],
                                    scalar=1.0, in1=hs[:, p1:p1 + 64],
                                    op0=MUL, op1=MUL,
                                    accum_out=prodw[d][q][:, blk:blk + 1])

            l2_dir(0)
            l2_dir(1)

            # ========== masked SCE epilogue (wide) ========================
            with tc.tile_pool(name="ep", bufs=1) as ep:
                su = [ep.tile([128, MB], f32, tag=f"su{q}", name=f"su{q}")
                      for q in range(3)]
                for q in range(3):
                    nc.vector.tensor_tensor(out=su[q][:], in0=prodw[0][q][:],
                                            in1=prodw[1][q][:], op=ADD)

                def rsq(n, tag):
                    r = ep.tile([128, MB], f32, tag=tag, name=tag)
                    nc.scalar.sqrt(out=r[:], in_=n[:])
                    nc.vector.tensor_scalar_max(out=r[:], in0=r[:],
                                                scalar1=1e-12)
                    nc.vector.reciprocal(out=r[:], in_=r[:])
                    return r

                r1 = rsq(su[1], "r1")
                r2 = rsq(su[2], "r2")
                tt = ep.tile([128, MB], f32, tag="tt", name="tt")
                nc.vector.tensor_tensor(out=tt[:], in0=su[0][:], in1=r1[:],
                                        op=MUL)
                nc.vector.tensor_tensor(out=tt[:], in0=tt[:], in1=r2[:],
                                        op=MUL)
                nc.vector.tensor_tensor(out=tt[:], in0=tt[:], in1=mcsb[:],
                                        op=MUL)
                scr = ep.tile([128, MB], f32, tag="scr", name="scr")
                colsum = ep.tile([128, 1], f32, tag="cs", name="cs")
                nc.vector.scalar_tensor_tensor(
                    out=scr[:], in0=tt[:], scalar=-1.0, in1=mcsb[:],
                    op0=MUL, op1=ADD, accum_out=colsum[:])
                with tc.tile_pool(name="eps", bufs=1, space="PSUM") as epp:
                    macc_ps = epp.tile([1, 1], f32, tag="mp", name="mp")
                    nc.tensor.matmul(out=macc_ps[:], lhsT=colsum[:],
                                     rhs=onesb[:], start=True, stop=True)
                    nc.vector.tensor_copy(out=arsb[0:1, 512:513],
                                          in_=macc_ps[:])

            # ========== AllReduce (pools + mask partial) =================
            nc.sync.dma_start(out=ar_in[:, :], in_=arsb[:])
            nc.gpsimd.collective_compute(
                "AllReduce", mybir.AluOpType.add,
                replica_groups=[list(range(C))],
                ins=[ar_in.opt()], outs=[ar_out.opt()])

            # ========== pooled cosine + final loss =======================
            with (
                tc.tile_pool(name="fin3", bufs=2) as f2,
                tc.tile_pool(name="fps", bufs=2, space="PSUM") as fpp,
            ):
                ar2 = f2.tile([128, 520], f32, tag="ar2", name="ar2")
                nc.sync.dma_start(out=ar2[:], in_=ar_out[:, :])
                cntsb = f2.tile([128, 128], f32, tag="cnt", name="cnt")
                nc.sync.dma_start(out=cntsb[:], in_=cntbc_t[:, :])
                b2t = f2.tile([64, 4], f32, tag="b2tf", name="b2tf")
                nc.sync.dma_start(out=b2t[:], in_=b2col_t[:, :])
                pf = {}
                for d in range(2):
                    for h in range(2):
                        po = f2.tile([64, 128], f32, tag=f"po{d}{h}",
                                     name=f"po{d}{h}")
                        nc.vector.scalar_tensor_tensor(
                            out=po[:], in0=cntsb[0:64, :],
                            scalar=b2t[0:64, 2 * d + h:2 * d + h + 1],
                            in1=ar2[0:64, d * 256 + h * 128:
                                    d * 256 + (h + 1) * 128],
                            op0=MUL, op1=ADD)
                        pf[(d, h)] = po
                gsums = []
                for qi, pick in enumerate(((0, 1), (0, 0), (1, 1))):
                    qp = fpp.tile([1, 128], f32, tag="gqp", name="gqp")
                    for d in range(2):
                        pr = f2.tile([64, 128], f32, tag=f"gpr{d}",
                                     name=f"gpr{d}")
                        nc.vector.tensor_tensor(
                            out=pr[:], in0=pf[(d, pick[0])][:],
                            in1=pf[(d, pick[1])][:], op=MUL)
                        nc.tensor.matmul(
                            out=qp[:], lhsT=onesb[0:64, 0:1], rhs=pr[:],
                            start=(d == 0), stop=(d == 1),
                            skip_group_check=True)
                    sq = f2.tile([1, 128], f32, tag=f"gsq{qi}",
                                 name=f"gsq{qi}")
                    nc.vector.tensor_copy(out=sq[:], in_=qp[:])
                    gsums.append(sq)
                gdot, gn1, gn2 = gsums

                def rguard2(n, tag):
                    r = f2.tile([1, 128], f32, tag=tag, name=tag)
                    nc.scalar.sqrt(out=r[:], in_=n[:])
                    nc.vector.tensor_scalar_max(out=r[:], in0=r[:],
                                                scalar1=1e-12)
                    nc.vector.reciprocal(out=r[:], in_=r[:])
                    return r

                g1 = rguard2(gn1, "g1")
                g2 = rguard2(gn2, "g2")
                cosg = f2.tile([1, 128], f32, tag="cosg", name="cosg")
                nc.vector.tensor_tensor(out=cosg[:], in0=gdot[:], in1=g1[:],
                                        op=MUL)
                nc.vector.tensor_tensor(out=cosg[:], in0=cosg[:], in1=g2[:],
                                        op=MUL)
                onesrow = f2.tile([1, 128], f32, tag="onesr", name="onesr")
                nc.vector.memset(onesrow[:], 1.0)
                gterm = f2.tile([1, 128], f32, tag="gterm", name="gterm")
                gs = f2.tile([1, 1], f32, tag="gs", name="gs")
                nc.vector.scalar_tensor_tensor(
                    out=gterm[:], in0=cosg[:], scalar=-1.0, in1=onesrow[:],
                    op0=MUL, op1=ADD, accum_out=gs[:])
                l1t = f2.tile([1, 1], f32, tag="l1", name="l1")
                nc.scalar.activation(out=l1t[:], in_=gs[:],
                                     func=mybir.ActivationFunctionType.Copy,
                                     scale=1.0 / G)
                l2t = f2.tile([1, 1], f32, tag="l2", name="l2")
                nc.scalar.activation(out=l2t[:], in_=ar2[0:1, 512:513],
                                     func=mybir.ActivationFunctionType.Copy,
                                     scale=1.0 / M)
                nc.vector.tensor_tensor(out=l1t[:], in0=l1t[:], in1=l2t[:],
                                        op=ADD)
                nc.sync.dma_start(out=loss_t[:, :], in_=l1t[:])

    return nc


# ---------------------------------------------------------------- entry

LAST_RESULT = None


def _install_trace_hook():
    """The agent image's antenv lacks axon_hooks; synthesize it from
    trn_boot's ctypes NTFF hook so trace=True works under axon."""
    import types
    try:
        from antenv import axon_hooks  # noqa: F401
        return
    except ImportError:
        pass
    try:
        import antenv
        import trn_agent_boot.trn_boot as tb
        hook = tb._ntff_profile_via_ctypes("/opt/axon/libaxon_pjrt.so")
        mod = types.ModuleType("antenv.axon_hooks")
        mod.get_axon_ntff_profile_hook = lambda: hook
        mod.set_axon_ntff_profile_hook = lambda h: None
        sys.modules["antenv.axon_hooks"] = mod
        antenv.axon_hooks = mod
    except Exception as e:
        print(f"[kernel] trace hook install failed: {e}", file=sys.stderr)


def kernel(_trace=False, **inputs):
    global LAST_RESULT
    import time
    from concourse import bass_utils
    if _trace:
        _install_trace_hook()
    t0 = time.monotonic()
    meta, in_maps = host_prep(inputs)
    t1 = time.monotonic()
    nc = build_program(meta)
    t2 = time.monotonic()
    nc.compile()
    t3 = time.monotonic()
    res = bass_utils.run_bass_kernel_spmd(
        nc, in_maps, core_ids=list(range(C)),
        trace=_trace, trace_cores=[0] if _trace else None)
    t4 = time.monotonic()
    print(f"[kernel] prep {t1-t0:.1f}s build {t2-t1:.1f}s "
          f"compile {t3-t2:.1f}s run {t4-t3:.1f}s", file=sys.stderr)
    LAST_RESULT = res
    return np.float32(res.results[0]["loss"][0, 0])


# revision 37
# speedup vs baseline: 1.0021x; 1.0021x over previous
"""Trainium2 Bass kernel for the rumor-GCN masked-autoencoder loss.

Strategy (8 NeuronCores, SPMD single NEFF):
  - Nodes partitioned into 8 contiguous ranges (25000 each), then per-core
    RE-ORDERED: unmasked own nodes first [0, UM), masked own compact at
    [UMPAD, UMPAD+MK).  All host-side index maps are relabeled, so the
    permutation is free at runtime and makes (a) mask-aggregation self terms
    a contiguous z2own slice and (b) L1 self-loop terms a contiguous z
    readback -- neither needs dma_gather (~8ns/idx on GpSimd, the dominant
    cost; see /root/problem/microbench.py).
  - z = [x1|x] @ W1 for all 4 GCN heads in one fused [512->512] bf16 matmul
    over the per-core needed set (own + halo, pre-gathered by host).  Row
    scales dinv[src] folded at the copy-out, dinv[dst] at finalize.
  - L1 edge aggregation: flat slot schedule bucketed by (group-of-8-dst-
    blocks, z-window).  Slots sorted by dst block inside each bucket, padded
    only at bucket tails; one dma_gather per bucket; one matmul per
    (K-tile x dst-block-segment) with host-built one-hot S.  Tiles may span
    dst blocks (extra matmul, no extra gather).  Self-loop term z[own]
    added at finalize via direct DMA readback.  global_add_pool is fused
    into the finalize: pool[g] += z2sb^T @ poolS (src-side rewrite).
  - L2 is only needed at masked nodes.  Mask aggregation is node-major
    ([128 masked nodes, 128 feat(on|tgt)] PSUM per block): halo edges
    gathered from the AllGathered z2full with the same flat scheduling;
    self term + b2 bias added at finalize from the contiguous z2own slice.
    Cosine terms reduce along the free dim via accum_out into per-block
    columns; one short wide chain finishes the masked SCE.
  - Each direction's z2 AllGather is issued as soon as that direction's L1
    finishes, overlapping the other direction's aggregation; pooled sums +
    the mask partial go through one small AllReduce.
"""

import sys

import numpy as np

sys.path.insert(0, "/opt/trn_rl_repo")

# ---------------------------------------------------------------- config

WIN = 32768
GB1 = 8       # L1 dst blocks per PSUM group
GB2 = 8       # L2 mask blocks per PSUM group
NF = 2048     # P1 column chunk

N, E, G, M, C = 200000, 400000, 128, 100000, 8
OWN = N // C

_WNAMES = [p + s for p in ("on_td", "on_bu", "tgt_td", "tgt_bu")
           for s in ("_W1", "_b1", "_W2", "_b2")]


def _rep16(idx_flat, nslots):
    """int16 index list -> [128, nslots//16] layout (16-part wrap, 8x rep)."""
    blk = np.zeros((16, nslots // 16), dtype=np.int16)
    k = np.arange(len(idx_flat))
    blk[k % 16, k // 16] = idx_flat
    return np.tile(blk, (8, 1))


def _bcast(vec, parts=128):
    return np.broadcast_to(np.asarray(vec)[None, :], (parts, len(vec))).copy()


def _pad128(n):
    return -(-n // 128) * 128


# ---------------------------------------------------------------- host prep

W1SCALE = 16.0  # lift fp8 W1 out of the subnormal range; undone in dloc


def host_prep(inp):
    import ml_dtypes
    bf16 = ml_dtypes.bfloat16
    f8 = ml_dtypes.float8_e4m3
    x = np.asarray(inp["x"], np.float32)
    token = np.asarray(inp["enc_mask_token"], np.float32).reshape(-1)
    ei = np.asarray(inp["edge_index"])
    src, dst = ei[0].astype(np.int64), ei[1].astype(np.int64)
    batch = np.asarray(inp["batch"]).astype(np.int64)
    mask_nodes = np.asarray(inp["mask_nodes"]).astype(np.int64)
    W = {k: np.asarray(inp[k], np.float32) for k in _WNAMES}

    dinv = [
        (1.0 / np.sqrt(np.bincount(dst, minlength=N) + 1.0)).astype(np.float32),
        (1.0 / np.sqrt(np.bincount(src, minlength=N) + 1.0)).astype(np.float32),
    ]
    is_masked = np.zeros(N, bool)
    is_masked[mask_nodes] = True
    mcnt_global = np.bincount(mask_nodes, minlength=N).astype(np.float32)
    xbf = x.astype(bf16)

    # ---- per-core own-node permutation: unmasked first, masked at tail
    um_nodes, mk_nodes = [], []
    for ci in range(C):
        lo = ci * OWN
        m = is_masked[lo:lo + OWN]
        um_nodes.append(np.where(~m)[0] + lo)
        mk_nodes.append(np.where(m)[0] + lo)
    UM = [len(a) for a in um_nodes]
    MK = [len(a) for a in mk_nodes]
    UMPAD = _pad128(max(UM))
    MKPAD = _pad128(max(MK))
    OWNP = UMPAD + MKPAD
    NB = OWNP // 128
    MB = MKPAD // 128
    NPAD = C * OWNP
    NW2 = -(-NPAD // WIN)

    pos_own = []          # [C] array [OWN] -> p-order position
    for ci in range(C):
        lo = ci * OWN
        p = np.empty(OWN, np.int64)
        p[um_nodes[ci] - lo] = np.arange(UM[ci])
        p[mk_nodes[ci] - lo] = UMPAD + np.arange(MK[ci])
        pos_own.append(p)

    # ---- per-core edge lists (dir 0 = TD: dst-agg; dir 1 = BU: src-agg)
    core_edges = []       # [core][dir] -> (adst_local, asrc_global)
    for ci in range(C):
        lo, hi = ci * OWN, (ci + 1) * OWN
        per = []
        for d in range(2):
            ad, as_ = (dst, src) if d == 0 else (src, dst)
            sel = (ad >= lo) & (ad < hi)
            per.append((ad[sel] - lo, as_[sel]))
        core_edges.append(per)

    # ---- halo sets (union over both dirs), split unmasked/masked
    halo_um, halo_mk = [], []
    for ci in range(C):
        lo, hi = ci * OWN, (ci + 1) * OWN
        srcs = np.unique(np.concatenate(
            [core_edges[ci][0][1], core_edges[ci][1][1]]))
        srcs = srcs[(srcs < lo) | (srcs >= hi)]
        halo_um.append(srcs[~is_masked[srcs]])
        halo_mk.append(srcs[is_masked[srcs]])
    HU = [len(a) for a in halo_um]
    HM = [len(a) for a in halo_mk]
    HUPAD = _pad128(max(HU))
    HMPAD = _pad128(max(HM))
    RT = OWNP + HUPAD + HMPAD
    NW1 = -(-RT // WIN)

    # z-row map per core: global node -> z row (own p-order | halo)
    zrow = []
    for ci in range(C):
        lo = ci * OWN
        zm = np.full(N, -1, np.int64)
        zm[lo + np.arange(OWN)] = pos_own[ci]
        zm[halo_um[ci]] = OWNP + np.arange(HU[ci])
        zm[halo_mk[ci]] = OWNP + HUPAD + np.arange(HM[ci])
        zrow.append(zm)

    # P1 sections: (row0, rowlen, is_masked_section)
    sections = [(0, UMPAD, False), (UMPAD, MKPAD, True),
                (OWNP, HUPAD, False), (OWNP + HUPAD, HMPAD, True)]

    # ---- generic flat scheduler -----------------------------------------
    def build_flat(percore_bwrlv, NBLK, GBX, NWX):
        """percore_bwrlv: per core (blk, win, rel, lane, val) arrays.
        Returns sched dict + per-core (S, idx) builders' inputs."""
        NG = -(-NBLK // GBX)
        cnt = np.zeros((C, NG, NWX), np.int64)
        for ci in range(C):
            b, w = percore_bwrlv[ci][0], percore_bwrlv[ci][1]
            np.add.at(cnt, (ci, b // GBX, w), 1)
        KT = -(-cnt.max(axis=0) // 128)          # [NG, NWX]
        ktoff = np.zeros((NG, NWX), np.int64)
        acc = 0
        for g in range(NG):
            for w in range(NWX):
                ktoff[g, w] = acc
                acc += KT[g, w]
        TOTKT = acc
        # per-core slot/op computation
        per_core = []
        opset = {}
        for ci in range(C):
            b, w, rel, lane, val = percore_bwrlv[ci]
            g = b // GBX
            bucket = g * NWX + w
            order = np.lexsort((np.arange(len(b)), b, bucket))
            bs, ws, gs = b[order], w[order], g[order]
            rels, lanes, vals = rel[order], lane[order], val[order]
            buck = gs * NWX + ws
            segchange = np.r_[True, buck[1:] != buck[:-1]]
            segstart = np.maximum.accumulate(
                np.where(segchange, np.arange(len(buck)), 0))
            pos = np.arange(len(buck)) - segstart
            kt = ktoff[gs, ws] + pos // 128
            sit = pos % 128
            per_core.append((kt, sit, bs, rels, lanes, vals))
            for key in set(zip(kt.tolist(), bs.tolist())):
                opset[key] = True
        ops = sorted(opset.keys())               # (kt, blk) in emission order
        opidx = {key: o for o, key in enumerate(ops)}
        NOP = len(ops)
        # group structure for emission
        groups = []
        for g in range(NG):
            gops = [(w, int(ktoff[g, w]), int(KT[g, w]))
                    for w in range(NWX) if KT[g, w] > 0]
            kt_lo = int(ktoff[g].min()) if gops else 0
            kt_hi = kt_lo + sum(nk for _, _, nk in gops)
            mops = [(kt, blk, opidx[(kt, blk)]) for (kt, blk) in ops
                    if kt_lo <= kt < kt_hi] if gops else []
            # start/stop per block within this group
            first, last = {}, {}
            for i, (kt, blk, o) in enumerate(mops):
                if blk not in first:
                    first[blk] = i
                last[blk] = i
            flags = [(kt, blk, o, first[blk] == i, last[blk] == i)
                     for i, (kt, blk, o) in enumerate(mops)]
            groups.append(dict(gops=gops, mops=flags, kt_lo=kt_lo,
                               nk=kt_hi - kt_lo,
                               blocks=list(range(g * GBX,
                                                 min((g + 1) * GBX, NBLK)))))
        return dict(KT=KT, ktoff=ktoff, TOTKT=TOTKT, NOP=NOP, groups=groups,
                    per_core=per_core, opidx=opidx, cnt=cnt)

    def fill_slots(sched, ci, sdtype):
        kt, sit, bs, rels, lanes, vals = sched["per_core"][ci]
        nslots = sched["TOTKT"] * 128
        idx_flat = np.zeros(nslots, np.int64)
        idx_flat[kt * 128 + sit] = rels
        assert rels.max(initial=0) < WIN
        S = np.zeros((128, sched["NOP"] * 128), np.float32)
        o = np.array([sched["opidx"][(int(k), int(b))]
                      for k, b in zip(kt, bs)], np.int64)
        np.add.at(S, (sit, o * 128 + lanes), vals)
        return (S.astype(sdtype),
                _rep16(idx_flat.astype(np.int16), nslots))

    # ---- L1 schedules ----------------------------------------------------
    sched1 = []
    for d in range(2):
        percore = []
        for ci in range(C):
            adst, asrc = core_edges[ci][d]
            dpos = pos_own[ci][adst]
            row = zrow[ci][asrc]
            assert (row >= 0).all()
            percore.append((dpos // 128, row // WIN, row % WIN, dpos % 128,
                            dinv[d][asrc].astype(np.float32)))
        sched1.append(build_flat(percore, NB, GB1, NW1))

    # ---- L2 mask schedules (halo only; self via direct slice) -----------
    mk_rank = []          # [C] array [OWN] -> rank in masked list or -1
    for ci in range(C):
        lo = ci * OWN
        r = np.full(OWN, -1, np.int64)
        r[mk_nodes[ci] - lo] = np.arange(MK[ci])
        mk_rank.append(r)

    sched2 = []
    for d in range(2):
        percore = []
        for ci in range(C):
            lo = ci * OWN
            ad_g, as_g = (dst, src) if d == 0 else (src, dst)
            sel = ((ad_g >= lo) & (ad_g < lo + OWN)
                   & is_masked[np.clip(ad_g, 0, N - 1)])
            adst = ad_g[sel] - lo
            md = mk_rank[ci][adst]
            sj = as_g[sel] // OWN        # owner core of source
            srow = sj * OWNP + pos_own_of(sj, as_g[sel] - sj * OWN, pos_own)
            percore.append((md // 128, srow // WIN, srow % WIN, md % 128,
                            dinv[d][lo + adst].astype(np.float32)))
        sched2.append(build_flat(percore, MB, GB2, NW2))

    # ---- per-core inputs -------------------------------------------------
    w1all = (np.concatenate([W["on_td_W1"], W["tgt_td_W1"],
                             W["on_bu_W1"], W["tgt_bu_W1"]], axis=1)
             * W1SCALE).astype(f8)
    w2_td = np.concatenate([W["on_td_W2"], W["tgt_td_W2"]], axis=1).astype(bf16)
    w2_bu = np.concatenate([W["on_bu_W2"], W["tgt_bu_W2"]], axis=1).astype(bf16)
    ton = np.concatenate([token @ W["on_td_W1"], token @ W["on_bu_W1"]])
    tonbc = _bcast(ton).astype(bf16)
    b1bc_td = _bcast(np.concatenate([W["on_td_b1"], W["tgt_td_b1"]]))
    b1bc_bu = _bcast(np.concatenate([W["on_bu_b1"], W["tgt_bu_b1"]]))
    b2bc_td = _bcast(np.concatenate([W["on_td_b2"], W["tgt_td_b2"]]))
    b2bc_bu = _bcast(np.concatenate([W["on_bu_b2"], W["tgt_bu_b2"]]))
    b2col = np.stack(
        [W["on_td_b2"], W["tgt_td_b2"], W["on_bu_b2"], W["tgt_bu_b2"]],
        axis=1).astype(np.float32)                         # [64, 4]
    ones = np.ones((128, 1), np.float32)
    gcount = np.bincount(batch, minlength=G).astype(np.float32)
    cntbc = np.broadcast_to(gcount[None, :128], (128, 128)).copy()

    in_maps = []
    for ci in range(C):
        lo = ci * OWN
        # xT in z-row order
        xT = np.zeros((512, RT), f8)
        xT[:, 0:UM[ci]] = x[um_nodes[ci]].T
        xT[:, UMPAD:UMPAD + MK[ci]] = x[mk_nodes[ci]].T
        xT[:, OWNP:OWNP + HU[ci]] = x[halo_um[ci]].T
        xT[:, OWNP + HUPAD:OWNP + HUPAD + HM[ci]] = x[halo_mk[ci]].T

        def dstarr(dv):
            a = np.ones(OWNP, np.float32)
            a[0:UM[ci]] = dv[um_nodes[ci]]
            a[UMPAD:UMPAD + MK[ci]] = dv[mk_nodes[ci]]
            return np.ascontiguousarray(a.reshape(-1, 128).T)

        def colarr(vals_mk, fill=0.0):
            a = np.full(MKPAD, fill, np.float32)
            a[0:MK[ci]] = vals_mk
            return np.ascontiguousarray(a.reshape(-1, 128).T)  # [128, MB]

        m = dict(xT=xT,
                 ddst_td=dstarr(dinv[0]), ddst_bu=dstarr(dinv[1]),
                 swv_td=colarr(dinv[0][mk_nodes[ci]]),
                 swv_bu=colarr(dinv[1][mk_nodes[ci]]),
                 mcvw=colarr(mcnt_global[mk_nodes[ci]]))
        for d, nm in ((0, "td"), (1, "bu")):
            S, idx = fill_slots(sched1[d], ci, f8)
            m[f"s_{nm}1"], m[f"i_{nm}1"] = S, idx
            S2, idx2 = fill_slots(sched2[d], ci, bf16)
            m[f"s2_{nm}"], m[f"i2_{nm}"] = S2, idx2
            # pool S: out-edges of own nodes + self, grouped by graph
            ad, as_ = (dst, src) if d == 0 else (src, dst)
            dv = dinv[d]
            sel = (as_ >= lo) & (as_ < lo + OWN)
            j = pos_own[ci][as_[sel] - lo]
            gg = batch[ad[sel]]
            v = dv[ad[sel]]
            pp = np.zeros((128, NB * 128), np.float32)
            np.add.at(pp, (j % 128, (j // 128) * 128 + gg), v)
            jj = pos_own[ci]
            np.add.at(pp, (jj % 128, (jj // 128) * 128 + batch[lo:lo + OWN]),
                      dv[lo:lo + OWN])
            m[f"pools_{nm}"] = pp.astype(bf16)
        m.update(w1all=w1all, w2_td=w2_td, w2_bu=w2_bu, tonbc=tonbc,
                 b1bc_td=b1bc_td, b1bc_bu=b1bc_bu,
                 b2bc_td=b2bc_td, b2bc_bu=b2bc_bu, b2col=b2col,
                 ones=ones, cntbc=cntbc)
        in_maps.append(m)

    meta = dict(RT=RT, NW1=NW1, NW2=NW2, NB=NB, MB=MB, OWNP=OWNP,
                UMPAD=UMPAD, MKPAD=MKPAD, NPAD=NPAD,
                sections=sections, sched1=sched1, sched2=sched2)
    return meta, in_maps


def pos_own_of(owner_cores, local_idx, pos_own):
    """vectorized pos_own lookup across owner cores"""
    out = np.empty(len(local_idx), np.int64)
    for j in np.unique(owner_cores):
        sel = owner_cores == j
        out[sel] = pos_own[j][local_idx[sel]]
    return out


# ---------------------------------------------------------------- program

def build_program(meta):
    import concourse.bass as bass
    import concourse.bacc as bacc
    import concourse.mybir as mybir
    import concourse.tile as tile
    from concourse.masks import make_identity

    RT, NB, MB = meta["RT"], meta["NB"], meta["MB"]
    NW1, NW2 = meta["NW1"], meta["NW2"]
    OWNP, UMPAD, NPAD = meta["OWNP"], meta["UMPAD"], meta["NPAD"]
    f32, bf, i16 = mybir.dt.float32, mybir.dt.bfloat16, mybir.dt.int16
    f8 = mybir.dt.float8e4
    MUL, ADD = mybir.AluOpType.mult, mybir.AluOpType.add

    nc = bacc.Bacc("TRN2", target_bir_lowering=False, debug=False,
                   num_devices=C)

    def din(name, shape, dt):
        return nc.dram_tensor(name, shape, dt, kind="ExternalInput")

    xT = din("xT", [512, RT], f8)
    ddst = [din("ddst_td", [128, NB], f32), din("ddst_bu", [128, NB], f32)]
    s1 = [din("s_td1", [128, meta["sched1"][0]["NOP"] * 128], f8),
          din("s_bu1", [128, meta["sched1"][1]["NOP"] * 128], f8)]
    i1 = [din("i_td1", [128, meta["sched1"][0]["TOTKT"] * 8], i16),
          din("i_bu1", [128, meta["sched1"][1]["TOTKT"] * 8], i16)]
    s2 = [din("s2_td", [128, meta["sched2"][0]["NOP"] * 128], bf),
          din("s2_bu", [128, meta["sched2"][1]["NOP"] * 128], bf)]
    i2 = [din("i2_td", [128, meta["sched2"][0]["TOTKT"] * 8], i16),
          din("i2_bu", [128, meta["sched2"][1]["TOTKT"] * 8], i16)]
    pools_t = [din("pools_td", [128, NB * 128], bf),
               din("pools_bu", [128, NB * 128], bf)]
    swv_t = [din("swv_td", [128, MB], f32), din("swv_bu", [128, MB], f32)]
    mcvw_t = din("mcvw", [128, MB], f32)
    w1all = din("w1all", [512, 512], f8)
    w2 = [din("w2_td", [128, 128], bf), din("w2_bu", [128, 128], bf)]
    tonbc = din("tonbc", [128, 256], bf)
    b1bc = [din("b1bc_td", [128, 256], f32), din("b1bc_bu", [128, 256], f32)]
    b2bc = [din("b2bc_td", [128, 128], f32), din("b2bc_bu", [128, 128], f32)]
    b2col_t = din("b2col", [64, 4], f32)
    ones_t = din("ones", [128, 1], f32)
    cntbc_t = din("cntbc", [128, 128], f32)
    loss_t = nc.dram_tensor("loss", [1, 1], f32, kind="ExternalOutput")

    z_ws = [nc.dram_tensor(f"zarr{w}", [min(WIN, RT - w * WIN), 512], f8,
                           kind="Internal")
            for w in range(NW1)]

    with tile.TileContext(nc) as tc:
        with (
            tc.tile_pool(name="const", bufs=1) as cpool,
            tc.tile_pool(name="dram", bufs=1, space="DRAM") as dpool,
        ):
            z2own = [dpool.tile([OWNP, 128], bf, tag=f"z2own{d}",
                                name=f"z2own{d}") for d in range(2)]
            z2full = [dpool.tile([NPAD, 128], bf, addr_space="Shared",
                                 tag=f"z2full{d}", name=f"z2full{d}")
                      for d in range(2)]
            ar_in = dpool.tile([128, 520], f32, tag="arin", name="arin")
            ar_out = dpool.tile([128, 520], f32, addr_space="Shared",
                                tag="arout", name="arout")

            ident = cpool.tile([128, 128], bf)
            make_identity(nc, ident[:])
            w1sb = cpool.tile([128, 4 * 512], f8)
            for k in range(4):
                nc.sync.dma_start(out=w1sb[:, k * 512:(k + 1) * 512],
                                  in_=w1all[k * 128:(k + 1) * 128, :])
            w2sb = [cpool.tile([128, 128], bf, tag=f"w2_{d}", name=f"w2_{d}")
                    for d in range(2)]
            tonsb = cpool.tile([128, 256], bf)
            b1sb = [cpool.tile([128, 256], f32, tag=f"b1_{d}", name=f"b1_{d}")
                    for d in range(2)]
            b2sb = [cpool.tile([128, 128], f32, tag=f"b2_{d}", name=f"b2_{d}")
                    for d in range(2)]
            ddsb = [cpool.tile([128, NB], f32, tag=f"dd_{d}", name=f"dd_{d}")
                    for d in range(2)]
            swsb = [cpool.tile([128, MB], f32, tag=f"sw_{d}", name=f"sw_{d}")
                    for d in range(2)]
            mcsb = cpool.tile([128, MB], f32)
            onesb = cpool.tile([128, 1], f32)
            nc.sync.dma_start(out=tonsb[:], in_=tonbc[:, :])
            nc.sync.dma_start(out=onesb[:], in_=ones_t[:, :])
            nc.sync.dma_start(out=mcsb[:], in_=mcvw_t[:, :])
            for d in range(2):
                nc.sync.dma_start(out=w2sb[d][:], in_=w2[d][:, :])
                nc.sync.dma_start(out=b1sb[d][:], in_=b1bc[d][:, :])
                nc.sync.dma_start(out=b2sb[d][:], in_=b2bc[d][:, :])
                nc.sync.dma_start(out=ddsb[d][:], in_=ddst[d][:, :])
                nc.sync.dma_start(out=swsb[d][:], in_=swv_t[d][:, :])

            # ================= P1: z = scaled([x1|x] @ W1-fused) ==========
            with (
                tc.tile_pool(name="xk", bufs=2) as xkp,
                tc.tile_pool(name="zsb", bufs=3) as zsp,
                tc.tile_pool(name="pz", bufs=2, space="PSUM") as pzp,
            ):
                DR = mybir.MatmulPerfMode.DoubleRow
                jpar = 0
                for (r0, rlen, msk) in meta["sections"]:
                    for off in range(0, rlen, NF):
                        nf = min(NF, rlen - off)
                        xk = xkp.tile([128, 4 * NF], f8, tag="xk", name="xk")
                        for k in range(4):
                            nc.sync.dma_start(
                                out=xk[:, k * NF:k * NF + nf],
                                in_=xT[k * 128:(k + 1) * 128,
                                       r0 + off:r0 + off + nf])
                        xk3 = xk[:].rearrange("p (k n) -> p k n", k=4, n=NF)
                        w13 = w1sb[:].rearrange("p (k n) -> p k n", k=4,
                                                n=512)
                        for j in range(nf // 128):
                            row = r0 + off + j * 128
                            jpar += 1
                            zs = zsp.tile([128, 512], f8, tag="zs", name="zs")
                            if not msk:
                                ps = pzp.tile([128, 512], f32, tag="pz",
                                              name="pz")
                                for k in range(0, 4, 2):
                                    nc.tensor.matmul(
                                        out=ps[:],
                                        lhsT=xk3[:, k:k + 2,
                                                 j * 128:(j + 1) * 128],
                                        rhs=w13[:, k:k + 2, :],
                                        start=(k == 0), stop=(k == 2),
                                        perf_mode=DR)
                                if jpar % 3 == 0:
                                    nc.scalar.activation(
                                        out=zs[:], in_=ps[:],
                                        func=mybir.ActivationFunctionType.Copy,
                                        scale=1.0 / W1SCALE)
                                else:
                                    nc.vector.tensor_scalar(
                                        out=zs[:], in0=ps[:],
                                        scalar1=1.0 / W1SCALE,
                                        scalar2=None, op0=MUL)
                            else:
                                ps = pzp.tile([128, 512], f32, tag="pz",
                                              name="pz")
                                for h in range(2):
                                    c0 = h * 256 + 128
                                    for k in range(0, 4, 2):
                                        nc.tensor.matmul(
                                            out=ps[:, h * 128:(h + 1) * 128],
                                            lhsT=xk3[:, k:k + 2,
                                                     j * 128:(j + 1) * 128],
                                            rhs=w13[:, k:k + 2, c0:c0 + 128],
                                            start=(k == 0), stop=(k == 2),
                                            perf_mode=DR)
                                for h in range(2):
                                    nc.vector.tensor_copy(
                                        out=zs[:, h * 256:h * 256 + 128],
                                        in_=tonsb[:, h * 128:(h + 1) * 128])
                                    if jpar % 3 == 0:
                                        nc.scalar.activation(
                                            out=zs[:, h * 256 + 128:
                                                   (h + 1) * 256],
                                            in_=ps[:, h * 128:(h + 1) * 128],
                                            func=mybir.ActivationFunctionType.Copy,
                                            scale=1.0 / W1SCALE)
                                    else:
                                        nc.vector.tensor_scalar(
                                            out=zs[:, h * 256 + 128:
                                                   (h + 1) * 256],
                                            in0=ps[:, h * 128:(h + 1) * 128],
                                            scalar1=1.0 / W1SCALE,
                                            scalar2=None, op0=MUL)
                            zw = row // WIN
                            zr = row - zw * WIN
                            weng = nc.scalar if jpar % 3 == 1 else nc.sync
                            weng.dma_start(out=z_ws[zw][zr:zr + 128, :],
                                           in_=zs[:])

            # ===== L1 agg + finalize (z2 + fused pool), per direction =====
            poolpool_cm = tc.tile_pool(name="plps", bufs=1, space="PSUM")
            poolpool = poolpool_cm.__enter__()
            poolps = poolpool.tile([128, 512], f32, tag="pl", name="pl")

            def l1_dir(d):
                sch = meta["sched1"][d]
                wlen = lambda w: min(WIN, RT - w * WIN)
                with (
                    tc.tile_pool(name=f"g1{d}", bufs=15) as gp,
                    tc.tile_pool(name=f"sI1{d}", bufs=3) as sp,
                    tc.tile_pool(name=f"ix1{d}", bufs=1) as ip,
                    tc.tile_pool(name=f"ps1{d}", bufs=2) as pwp,
                    tc.tile_pool(name=f"fin1{d}", bufs=3) as fp,
                    tc.tile_pool(name=f"h1q{d}", bufs=20) as h1p,
                    tc.tile_pool(name=f"zrb{d}", bufs=3) as zrp,
                    tc.tile_pool(name=f"agg{d}", bufs=1, space="PSUM") as ap,
                    tc.tile_pool(name=f"tr{d}", bufs=2, space="PSUM") as trp,
                    tc.tile_pool(name=f"z2p{d}", bufs=1, space="PSUM") as z2p,
                ):
                    # stage B (transpose -> @W2 -> scale -> z2own write +
                    # fused pool matmuls), decoupled from the agg pipeline
                    # via the deep h1 tile pool and one-group emission skew.
                    def stage_b(blk, bi, h1, pst):
                        trt = trp.tile([128, 256], bf, tag="t", name="t")
                        nc.tensor.transpose(
                            out=trt[:, 0:128], in_=h1[:, 0:128],
                            identity=ident[:])
                        nc.tensor.transpose(
                            out=trt[:, 128:256], in_=h1[:, 128:256],
                            identity=ident[:])
                        h1T = fp.tile([128, 256], bf, tag="h1T", name="h1T")
                        nc.scalar.copy(out=h1T[:], in_=trt[:])
                        z2ps = z2p.tile([128, 128], f32, tag="z2", name="z2")
                        nc.tensor.matmul(out=z2ps[:, 0:64],
                                         lhsT=h1T[:, 0:128],
                                         rhs=w2sb[d][:, 0:64],
                                         start=True, stop=True)
                        nc.tensor.matmul(out=z2ps[:, 64:128],
                                         lhsT=h1T[:, 128:256],
                                         rhs=w2sb[d][:, 64:128],
                                         start=True, stop=True)
                        z2sb = fp.tile([128, 128], bf, tag="z2sb",
                                       name="z2sb")
                        nc.vector.tensor_scalar(
                            out=z2sb[:], in0=z2ps[:],
                            scalar1=ddsb[d][:, blk:blk + 1],
                            scalar2=None, op0=MUL)
                        nc.sync.dma_start(
                            out=z2own[d][blk * 128:(blk + 1) * 128, :],
                            in_=z2sb[:])
                        nc.tensor.matmul(
                            out=poolps[0:64, d * 256:d * 256 + 128],
                            lhsT=z2sb[:, 0:64],
                            rhs=pst[:, bi * 128:(bi + 1) * 128],
                            start=(blk == 0), stop=(blk == NB - 1),
                            skip_group_check=True)
                        nc.tensor.matmul(
                            out=poolps[0:64, d * 256 + 128:d * 256 + 256],
                            lhsT=z2sb[:, 64:128],
                            rhs=pst[:, bi * 128:(bi + 1) * 128],
                            start=(blk == 0), stop=(blk == NB - 1),
                            skip_group_check=True)

                    itall = ip.tile([128, max(sch["TOTKT"], 1) * 8], i16,
                                    tag="ia", name="ia")
                    nc.gpsimd.dma_start(out=itall[:], in_=i1[d][:, :])
                    pending = []
                    groups = list(enumerate(sch["groups"]))
                    WAVE = 13
                    for w0 in range(0, len(groups), WAVE):
                        wave = groups[w0:w0 + WAVE]
                        slabs = {}
                        for g, grp in wave:
                            if grp["gops"]:
                                slabs[g] = gp.tile([128, grp["nk"] * 256],
                                                   f8, tag="g", name="g")
                        # window-major gather emission: w0 gathers only wait
                        # for z window 0 (ready ~35% into P1)
                        for w in range(meta["NW1"]):
                            for g, grp in wave:
                                for (ww, ktb, nkw) in grp["gops"]:
                                    if ww != w:
                                        continue
                                    o = ktb - grp["kt_lo"]
                                    nc.gpsimd.dma_gather(
                                        slabs[g][:, o * 256:(o + nkw) * 256]
                                        .rearrange("p (k e) -> p k e",
                                                   k=nkw, e=256),
                                        z_ws[w][0:wlen(w),
                                                256 * d:256 * d + 256],
                                        itall[:, ktb * 8:(ktb + nkw) * 8],
                                        nkw * 128, nkw * 128, 256,
                                        elem_step=512, single_packet=False)
                        for g, grp in wave:
                            blocks = grp["blocks"]
                            nops = len(grp["mops"])
                            gt = slabs.get(g)
                            if nops:
                                st = sp.tile([128, nops * 128], f8, tag="s",
                                             name="s")
                                nc.scalar.dma_start(
                                    out=st[:],
                                    in_=s1[d][:, grp["mops"][0][2] * 128:
                                              (grp["mops"][0][2] + nops)
                                              * 128])
                                aps = ap.tile([128, len(blocks) * 256], f32,
                                              tag="a", name="a")
                                o0 = grp["mops"][0][2]
                                for (kt, blk, o, st_f, sp_f) in grp["mops"]:
                                    bi = blk - blocks[0]
                                    nc.tensor.matmul(
                                        out=aps[:, bi * 256:(bi + 1) * 256],
                                        lhsT=st[:, (o - o0) * 128:
                                                (o - o0 + 1) * 128],
                                        rhs=gt[:, (kt - grp["kt_lo"]) * 256:
                                               (kt - grp["kt_lo"] + 1)
                                               * 256],
                                        start=st_f, stop=sp_f,
                                        skip_group_check=True)
                            has = {blk for (_, blk, _, _, _) in grp["mops"]}
                            # pool S slab for this group
                            pst = pwp.tile([128, len(blocks) * 128], bf,
                                           tag="ps", name="ps")
                            nc.sync.dma_start(
                                out=pst[:],
                                in_=pools_t[d][:, blocks[0] * 128:
                                               (blocks[0] + len(blocks))
                                               * 128])
                            newly = []
                            for blk in blocks:
                                bi = blk - blocks[0]
                                zrb = zrp.tile([128, 256], f8, tag="zr",
                                               name="zr")
                                nc.sync.dma_start(
                                    out=zrb[:],
                                    in_=z_ws[0][blk * 128:(blk + 1) * 128,
                                                256 * d:256 * d + 256])
                                hs = fp.tile([128, 256], f32, tag="hs",
                                             name="hs")
                                if blk in has:
                                    # hs = h_self*dinv_dst + agg
                                    nc.vector.scalar_tensor_tensor(
                                        out=hs[:], in0=zrb[:],
                                        scalar=ddsb[d][:, blk:blk + 1],
                                        in1=aps[:, bi * 256:(bi + 1) * 256],
                                        op0=MUL, op1=ADD)
                                else:
                                    nc.vector.tensor_scalar(
                                        out=hs[:], in0=zrb[:],
                                        scalar1=ddsb[d][:, blk:blk + 1],
                                        scalar2=None, op0=MUL)
                                # h1 = relu(hs*ddst + b1)
                                nc.vector.scalar_tensor_tensor(
                                    out=hs[:], in0=hs[:],
                                    scalar=ddsb[d][:, blk:blk + 1],
                                    in1=b1sb[d][:, 0:256], op0=MUL, op1=ADD)
                                h1 = h1p.tile([128, 256], bf, tag="h1",
                                              name="h1")
                                nc.scalar.activation(
                                    out=h1[:], in_=hs[:],
                                    func=mybir.ActivationFunctionType.Relu)
                                newly.append((blk, bi, h1, pst))
                            for item in pending:
                                stage_b(*item)
                            pending = newly
                    for item in pending:
                        stage_b(*item)

            def allgather(d):
                nc.gpsimd.collective_compute(
                    "AllGather", mybir.AluOpType.bypass,
                    replica_groups=[list(range(C))],
                    ins=[z2own[d].opt()], outs=[z2full[d].opt()])

            l1_dir(0)
            allgather(0)
            l1_dir(1)
            allgather(1)

            # drain pooled sums
            arsb = cpool.tile([128, 520], f32)
            nc.vector.memset(arsb[:], 0.0)
            for d in range(2):
                nc.vector.tensor_copy(out=arsb[0:64, d * 256:(d + 1) * 256],
                                      in_=poolps[0:64, d * 256:(d + 1) * 256])
            poolpool_cm.__exit__(None, None, None)

            # ========== L2 mask aggregation (node-major) ==================
            # wide per-dir product tiles
            prodw = [[cpool.tile([128, MB], f32, tag=f"pw{d}{q}",
                                 name=f"pw{d}{q}") for q in range(3)]
                     for d in range(2)]
            for d in range(2):
                for q in range(3):
                    nc.vector.memset(prodw[d][q][:], 0.0)

            def l2_dir(d):
                sch = meta["sched2"][d]
                wlen = lambda w: min(WIN, NPAD - w * WIN)
                with (
                    tc.tile_pool(name=f"g2{d}", bufs=3) as gp,
                    tc.tile_pool(name=f"sI2{d}", bufs=3) as sp,
                    tc.tile_pool(name=f"ix2{d}", bufs=3) as ip,
                    tc.tile_pool(name=f"fin2{d}", bufs=3) as fp,
                    tc.tile_pool(name=f"zsl{d}", bufs=3) as zp,
                    tc.tile_pool(name=f"mag{d}", bufs=2, space="PSUM") as ap,
                ):
                    for g, grp in enumerate(sch["groups"]):
                        blocks = grp["blocks"]
                        nops = len(grp["mops"])
                        nk = grp["nk"]
                        if nops:
                            st = sp.tile([128, nops * 128], bf, tag="s",
                                         name="s")
                            nc.sync.dma_start(
                                out=st[:],
                                in_=s2[d][:, grp["mops"][0][2] * 128:
                                          (grp["mops"][0][2] + nops) * 128])
                            it = ip.tile([128, nk * 8], i16, tag="i", name="i")
                            nc.sync.dma_start(
                                out=it[:], in_=i2[d][:, grp["kt_lo"] * 8:
                                                     (grp["kt_lo"] + nk) * 8])
                            gt = gp.tile([128, nk * 128], bf, tag="g",
                                         name="g")
                            for w, ktb, nkw in grp["gops"]:
                                o = ktb - grp["kt_lo"]
                                nc.gpsimd.dma_gather(
                                    gt[:, o * 128:(o + nkw) * 128].rearrange(
                                        "p (k e) -> p k e", k=nkw, e=128),
                                    z2full[d][w * WIN:w * WIN + wlen(w), :],
                                    it[:, o * 8:(o + nkw) * 8],
                                    nkw * 128, nkw * 128, 128,
                                    elem_step=None, single_packet=False)
                            aps = ap.tile([128, len(blocks) * 128], f32,
                                          tag="a", name="a")
                            o0 = grp["mops"][0][2]
                            for (kt, blk, o, st_f, sp_f) in grp["mops"]:
                                bi = blk - blocks[0]
                                nc.tensor.matmul(
                                    out=aps[:, bi * 128:(bi + 1) * 128],
                                    lhsT=st[:, (o - o0) * 128:
                                            (o - o0 + 1) * 128],
                                    rhs=gt[:, (kt - grp["kt_lo"]) * 128:
                                           (kt - grp["kt_lo"] + 1) * 128],
                                    start=st_f, stop=sp_f,
                                    skip_group_check=True)
                        has = {blk for (_, blk, _, _, _) in grp["mops"]}
                        for blk in blocks:
                            bi = blk - blocks[0]
                            zsl = zp.tile([128, 128], bf, tag="zs", name="zs")
                            nc.sync.dma_start(
                                out=zsl[:],
                                in_=z2own[d][UMPAD + blk * 128:
                                             UMPAD + (blk + 1) * 128, :])
                            hs = fp.tile([128, 128], f32, tag="hs", name="hs")
                            # hs = z_self*swv (+ agg)
                            if blk in has:
                                nc.vector.scalar_tensor_tensor(
                                    out=hs[:], in0=zsl[:],
                                    scalar=swsb[d][:, blk:blk + 1],
                                    in1=aps[:, bi * 128:(bi + 1) * 128],
                                    op0=MUL, op1=ADD)
                            else:
                                nc.vector.tensor_scalar(
                                    out=hs[:], in0=zsl[:],
                                    scalar1=swsb[d][:, blk:blk + 1],
                                    scalar2=None, op0=MUL)
                            nc.vector.tensor_tensor(
                                out=hs[:], in0=hs[:], in1=b2sb[d][:, 0:128],
                                op=ADD)
                            # products (accumulate over 64-feat free dim)
                            scr = fp.tile([128, 64], f32, tag="sc", name="sc")
                            for q, (p0, p1) in enumerate(
                                    ((0, 64), (0, 0), (64, 64))):
                                nc.vector.scalar_tensor_tensor(
                                    out=scr[:], in0=hs[:, p0:p0 + 64],
                                    scalar=1.0, in1=hs[:, p1:p1 + 64],
                                    op0=MUL, op1=MUL,
                                    accum_out=prodw[d][q][:, blk:blk + 1])

            l2_dir(0)
            l2_dir(1)

            # ========== masked SCE epilogue (wide) ========================
            with tc.tile_pool(name="ep", bufs=1) as ep:
                su = [ep.tile([128, MB], f32, tag=f"su{q}", name=f"su{q}")
                      for q in range(3)]
                for q in range(3):
                    nc.vector.tensor_tensor(out=su[q][:], in0=prodw[0][q][:],
                                            in1=prodw[1][q][:], op=ADD)

                def rsq(n, tag):
                    r = ep.tile([128, MB], f32, tag=tag, name=tag)
                    nc.scalar.sqrt(out=r[:], in_=n[:])
                    nc.vector.tensor_scalar_max(out=r[:], in0=r[:],
                                                scalar1=1e-12)
                    nc.vector.reciprocal(out=r[:], in_=r[:])
                    return r

                r1 = rsq(su[1], "r1")
                r2 = rsq(su[2], "r2")
                tt = ep.tile([128, MB], f32, tag="tt", name="tt")
                nc.vector.tensor_tensor(out=tt[:], in0=su[0][:], in1=r1[:],
                                        op=MUL)
                nc.vector.tensor_tensor(out=tt[:], in0=tt[:], in1=r2[:],
                                        op=MUL)
                nc.vector.tensor_tensor(out=tt[:], in0=tt[:], in1=mcsb[:],
                                        op=MUL)
                scr = ep.tile([128, MB], f32, tag="scr", name="scr")
                colsum = ep.tile([128, 1], f32, tag="cs", name="cs")
                nc.vector.scalar_tensor_tensor(
                    out=scr[:], in0=tt[:], scalar=-1.0, in1=mcsb[:],
                    op0=MUL, op1=ADD, accum_out=colsum[:])
                with tc.tile_pool(name="eps", bufs=1, space="PSUM") as epp:
                    macc_ps = epp.tile([1, 1], f32, tag="mp", name="mp")
                    nc.tensor.matmul(out=macc_ps[:], lhsT=colsum[:],
                                     rhs=onesb[:], start=True, stop=True)
                    nc.vector.tensor_copy(out=arsb[0:1, 512:513],
                                          in_=macc_ps[:])

            # ========== AllReduce (pools + mask partial) =================
            nc.sync.dma_start(out=ar_in[:, :], in_=arsb[:])
            nc.gpsimd.collective_compute(
                "AllReduce", mybir.AluOpType.add,
                replica_groups=[list(range(C))],
                ins=[ar_in.opt()], outs=[ar_out.opt()])

            # ========== pooled cosine + final loss =======================
            with (
                tc.tile_pool(name="fin3", bufs=2) as f2,
                tc.tile_pool(name="fps", bufs=2, space="PSUM") as fpp,
            ):
                ar2 = f2.tile([128, 520], f32, tag="ar2", name="ar2")
                nc.sync.dma_start(out=ar2[:], in_=ar_out[:, :])
                cntsb = f2.tile([128, 128], f32, tag="cnt", name="cnt")
                nc.sync.dma_start(out=cntsb[:], in_=cntbc_t[:, :])
                b2t = f2.tile([64, 4], f32, tag="b2tf", name="b2tf")
                nc.sync.dma_start(out=b2t[:], in_=b2col_t[:, :])
                pf = {}
                for d in range(2):
                    for h in range(2):
                        po = f2.tile([64, 128], f32, tag=f"po{d}{h}",
                                     name=f"po{d}{h}")
                        nc.vector.scalar_tensor_tensor(
                            out=po[:], in0=cntsb[0:64, :],
                            scalar=b2t[0:64, 2 * d + h:2 * d + h + 1],
                            in1=ar2[0:64, d * 256 + h * 128:
                                    d * 256 + (h + 1) * 128],
                            op0=MUL, op1=ADD)
                        pf[(d, h)] = po
                gsums = []
                for qi, pick in enumerate(((0, 1), (0, 0), (1, 1))):
                    qp = fpp.tile([1, 128], f32, tag="gqp", name="gqp")
                    for d in range(2):
                        pr = f2.tile([64, 128], f32, tag=f"gpr{d}",
                                     name=f"gpr{d}")
                        nc.vector.tensor_tensor(
                            out=pr[:], in0=pf[(d, pick[0])][:],
                            in1=pf[(d, pick[1])][:], op=MUL)
                        nc.tensor.matmul(
                            out=qp[:], lhsT=onesb[0:64, 0:1], rhs=pr[:],
                            start=(d == 0), stop=(d == 1),
                            skip_group_check=True)
                    sq = f2.tile([1, 128], f32, tag=f"gsq{qi}",
                                 name=f"gsq{qi}")
                    nc.vector.tensor_copy(out=sq[:], in_=qp[:])
                    gsums.append(sq)
                gdot, gn1, gn2 = gsums

                def rguard2(n, tag):
                    r = f2.tile([1, 128], f32, tag=tag, name=tag)
                    nc.scalar.sqrt(out=r[:], in_=n[:])
                    nc.vector.tensor_scalar_max(out=r[:], in0=r[:],
                                                scalar1=1e-12)
                    nc.vector.reciprocal(out=r[:], in_=r[:])
                    return r

                g1 = rguard2(gn1, "g1")
                g2 = rguard2(gn2, "g2")
                cosg = f2.tile([1, 128], f32, tag="cosg", name="cosg")
                nc.vector.tensor_tensor(out=cosg[:], in0=gdot[:], in1=g1[:],
                                        op=MUL)
                nc.vector.tensor_tensor(out=cosg[:], in0=cosg[:], in1=g2[:],
                                        op=MUL)
                onesrow = f2.tile([1, 128], f32, tag="onesr", name="onesr")
                nc.vector.memset(onesrow[:], 1.0)
                gterm = f2.tile([1, 128], f32, tag="gterm", name="gterm")
                gs = f2.tile([1, 1], f32, tag="gs", name="gs")
                nc.vector.scalar_tensor_tensor(
                    out=gterm[:], in0=cosg[:], scalar=-1.0, in1=onesrow[:],
                    op0=MUL, op1=ADD, accum_out=gs[:])
                l1t = f2.tile([1, 1], f32, tag="l1", name="l1")
                nc.scalar.activation(out=l1t[:], in_=gs[:],
                                     func=mybir.ActivationFunctionType.Copy,
                                     scale=1.0 / G)
                l2t = f2.tile([1, 1], f32, tag="l2", name="l2")
                nc.scalar.activation(out=l2t[:], in_=ar2[0:1, 512:513],
                                     func=mybir.ActivationFunctionType.Copy,
                                     scale=1.0 / M)
                nc.vector.tensor_tensor(out=l1t[:], in0=l1t[:], in1=l2t[:],
                                        op=ADD)
                nc.sync.dma_start(out=loss_t[:, :], in_=l1t[:])

    return nc


# ---------------------------------------------------------------- entry

LAST_RESULT = None


def _install_trace_hook():
    """The agent image's antenv lacks axon_hooks; synthesize it from
    trn_boot's ctypes NTFF hook so trace=True works under axon."""
    import types
    try:
        from antenv import axon_hooks  # noqa: F401
        return
    except ImportError:
        pass
    try:
        import antenv
        import trn_agent_boot.trn_boot as tb
        hook = tb._ntff_profile_via_ctypes("/opt/axon/libaxon_pjrt.so")
        mod = types.ModuleType("antenv.axon_hooks")
        mod.get_axon_ntff_profile_hook = lambda: hook
        mod.set_axon_ntff_profile_hook = lambda h: None
        sys.modules["antenv.axon_hooks"] = mod
        antenv.axon_hooks = mod
    except Exception as e:
        print(f"[kernel] trace hook install failed: {e}", file=sys.stderr)


def kernel(_trace=False, **inputs):
    global LAST_RESULT
    import time
    from concourse import bass_utils
    if _trace:
        _install_trace_hook()
    t0 = time.monotonic()
    meta, in_maps = host_prep(inputs)
    t1 = time.monotonic()
    nc = build_program(meta)
    t2 = time.monotonic()
    nc.compile()
    t3 = time.monotonic()
    res = bass_utils.run_bass_kernel_spmd(
        nc, in_maps, core_ids=list(range(C)),
        trace=_trace, trace_cores=[0] if _trace else None)
    t4 = time.monotonic()
    print(f"[kernel] prep {t1-t0:.1f}s build {t2-t1:.1f}s "
          f"compile {t3-t2:.1f}s run {t4-t3:.1f}s", file=sys.stderr)
    LAST_RESULT = res
    return np.float32(res.results[0]["loss"][0, 0])


# revision 38
# speedup vs baseline: 1.1010x; 1.0987x over previous
"""Trainium2 Bass kernel for the rumor-GCN masked-autoencoder loss.

Strategy (8 NeuronCores, SPMD single NEFF):
  - Nodes partitioned into 8 contiguous ranges (25000 each), then per-core
    RE-ORDERED: unmasked own nodes first [0, UM), masked own compact at
    [UMPAD, UMPAD+MK).  All host-side index maps are relabeled, so the
    permutation is free at runtime and makes (a) mask-aggregation self terms
    a contiguous z2own slice and (b) L1 self-loop terms a contiguous z
    readback -- neither needs dma_gather (~8ns/idx on GpSimd, the dominant
    cost; see /root/problem/microbench.py).
  - z = [x1|x] @ W1 for all 4 GCN heads in one fused [512->512] bf16 matmul
    over the per-core needed set (own + halo, pre-gathered by host).  Row
    scales dinv[src] folded at the copy-out, dinv[dst] at finalize.
  - L1 edge aggregation: flat slot schedule bucketed by (group-of-8-dst-
    blocks, z-window).  Slots sorted by dst block inside each bucket, padded
    only at bucket tails; one dma_gather per bucket; one matmul per
    (K-tile x dst-block-segment) with host-built one-hot S.  Tiles may span
    dst blocks (extra matmul, no extra gather).  Self-loop term z[own]
    added at finalize via direct DMA readback.  global_add_pool is fused
    into the finalize: pool[g] += z2sb^T @ poolS (src-side rewrite).
  - L2 is only needed at masked nodes.  Mask aggregation is node-major
    ([128 masked nodes, 128 feat(on|tgt)] PSUM per block): halo edges
    gathered from the AllGathered z2full with the same flat scheduling;
    self term + b2 bias added at finalize from the contiguous z2own slice.
    Cosine terms reduce along the free dim via accum_out into per-block
    columns; one short wide chain finishes the masked SCE.
  - Each direction's z2 AllGather is issued as soon as that direction's L1
    finishes, overlapping the other direction's aggregation; pooled sums +
    the mask partial go through one small AllReduce.
"""

import sys

import numpy as np

sys.path.insert(0, "/opt/trn_rl_repo")

# ---------------------------------------------------------------- config

WIN = 32768
GB1 = 8       # L1 dst blocks per PSUM group
GB2 = 8       # L2 mask blocks per PSUM group
NF = 2048     # P1 column chunk

N, E, G, M, C = 200000, 400000, 128, 100000, 8
OWN = N // C

_WNAMES = [p + s for p in ("on_td", "on_bu", "tgt_td", "tgt_bu")
           for s in ("_W1", "_b1", "_W2", "_b2")]


def _rep16(idx_flat, nslots):
    """int16 index list -> [128, nslots//16] layout (16-part wrap, 8x rep)."""
    blk = np.zeros((16, nslots // 16), dtype=np.int16)
    k = np.arange(len(idx_flat))
    blk[k % 16, k // 16] = idx_flat
    return np.tile(blk, (8, 1))


def _bcast(vec, parts=128):
    return np.broadcast_to(np.asarray(vec)[None, :], (parts, len(vec))).copy()


def _pad128(n):
    return -(-n // 128) * 128


# ---------------------------------------------------------------- host prep

W1SCALE = 16.0  # lift fp8 W1 out of the subnormal range; undone in dloc


def host_prep(inp):
    import ml_dtypes
    bf16 = ml_dtypes.bfloat16
    f8 = ml_dtypes.float8_e4m3
    x = np.asarray(inp["x"], np.float32)
    token = np.asarray(inp["enc_mask_token"], np.float32).reshape(-1)
    ei = np.asarray(inp["edge_index"])
    src, dst = ei[0].astype(np.int64), ei[1].astype(np.int64)
    batch = np.asarray(inp["batch"]).astype(np.int64)
    mask_nodes = np.asarray(inp["mask_nodes"]).astype(np.int64)
    W = {k: np.asarray(inp[k], np.float32) for k in _WNAMES}

    dinv = [
        (1.0 / np.sqrt(np.bincount(dst, minlength=N) + 1.0)).astype(np.float32),
        (1.0 / np.sqrt(np.bincount(src, minlength=N) + 1.0)).astype(np.float32),
    ]
    is_masked = np.zeros(N, bool)
    is_masked[mask_nodes] = True
    mcnt_global = np.bincount(mask_nodes, minlength=N).astype(np.float32)
    xbf = x.astype(bf16)

    # ---- per-core own-node permutation: unmasked first, masked at tail
    um_nodes, mk_nodes = [], []
    for ci in range(C):
        lo = ci * OWN
        m = is_masked[lo:lo + OWN]
        um_nodes.append(np.where(~m)[0] + lo)
        mk_nodes.append(np.where(m)[0] + lo)
    UM = [len(a) for a in um_nodes]
    MK = [len(a) for a in mk_nodes]
    UMPAD = _pad128(max(UM))
    MKPAD = _pad128(max(MK))
    OWNP = UMPAD + MKPAD
    NB = OWNP // 128
    MB = MKPAD // 128
    NPAD = C * OWNP
    NW2 = -(-NPAD // WIN)

    pos_own = []          # [C] array [OWN] -> p-order position
    for ci in range(C):
        lo = ci * OWN
        p = np.empty(OWN, np.int64)
        p[um_nodes[ci] - lo] = np.arange(UM[ci])
        p[mk_nodes[ci] - lo] = UMPAD + np.arange(MK[ci])
        pos_own.append(p)

    # ---- per-core edge lists (dir 0 = TD: dst-agg; dir 1 = BU: src-agg)
    core_edges = []       # [core][dir] -> (adst_local, asrc_global)
    for ci in range(C):
        lo, hi = ci * OWN, (ci + 1) * OWN
        per = []
        for d in range(2):
            ad, as_ = (dst, src) if d == 0 else (src, dst)
            sel = (ad >= lo) & (ad < hi)
            per.append((ad[sel] - lo, as_[sel]))
        core_edges.append(per)

    # ---- halo sets (union over both dirs), split unmasked/masked
    halo_um, halo_mk = [], []
    for ci in range(C):
        lo, hi = ci * OWN, (ci + 1) * OWN
        srcs = np.unique(np.concatenate(
            [core_edges[ci][0][1], core_edges[ci][1][1]]))
        srcs = srcs[(srcs < lo) | (srcs >= hi)]
        halo_um.append(srcs[~is_masked[srcs]])
        halo_mk.append(srcs[is_masked[srcs]])
    HU = [len(a) for a in halo_um]
    HM = [len(a) for a in halo_mk]
    HUPAD = _pad128(max(HU))
    HMPAD = _pad128(max(HM))
    RT = OWNP + HUPAD + HMPAD
    NW1 = -(-RT // WIN)

    # z-row map per core: global node -> z row (own p-order | halo)
    zrow = []
    for ci in range(C):
        lo = ci * OWN
        zm = np.full(N, -1, np.int64)
        zm[lo + np.arange(OWN)] = pos_own[ci]
        zm[halo_um[ci]] = OWNP + np.arange(HU[ci])
        zm[halo_mk[ci]] = OWNP + HUPAD + np.arange(HM[ci])
        zrow.append(zm)

    # P1 sections: (row0, rowlen, is_masked_section)
    sections = [(0, UMPAD, False), (UMPAD, MKPAD, True),
                (OWNP, HUPAD, False), (OWNP + HUPAD, HMPAD, True)]

    # ---- generic flat scheduler -----------------------------------------
    def build_flat(percore_bwrlv, NBLK, GBX, NWX):
        """percore_bwrlv: per core (blk, win, rel, lane, val) arrays.
        Returns sched dict + per-core (S, idx) builders' inputs."""
        NG = -(-NBLK // GBX)
        cnt = np.zeros((C, NG, NWX), np.int64)
        for ci in range(C):
            b, w = percore_bwrlv[ci][0], percore_bwrlv[ci][1]
            np.add.at(cnt, (ci, b // GBX, w), 1)
        KT = -(-cnt.max(axis=0) // 128)          # [NG, NWX]
        ktoff = np.zeros((NG, NWX), np.int64)
        acc = 0
        for g in range(NG):
            for w in range(NWX):
                ktoff[g, w] = acc
                acc += KT[g, w]
        TOTKT = acc
        # per-core slot/op computation
        per_core = []
        opset = {}
        for ci in range(C):
            b, w, rel, lane, val = percore_bwrlv[ci]
            g = b // GBX
            bucket = g * NWX + w
            order = np.lexsort((np.arange(len(b)), b, bucket))
            bs, ws, gs = b[order], w[order], g[order]
            rels, lanes, vals = rel[order], lane[order], val[order]
            buck = gs * NWX + ws
            segchange = np.r_[True, buck[1:] != buck[:-1]]
            segstart = np.maximum.accumulate(
                np.where(segchange, np.arange(len(buck)), 0))
            pos = np.arange(len(buck)) - segstart
            kt = ktoff[gs, ws] + pos // 128
            sit = pos % 128
            per_core.append((kt, sit, bs, rels, lanes, vals))
            for key in set(zip(kt.tolist(), bs.tolist())):
                opset[key] = True
        ops = sorted(opset.keys())               # (kt, blk) in emission order
        opidx = {key: o for o, key in enumerate(ops)}
        NOP = len(ops)
        # group structure for emission
        groups = []
        for g in range(NG):
            gops = [(w, int(ktoff[g, w]), int(KT[g, w]))
                    for w in range(NWX) if KT[g, w] > 0]
            kt_lo = int(ktoff[g].min()) if gops else 0
            kt_hi = kt_lo + sum(nk for _, _, nk in gops)
            mops = [(kt, blk, opidx[(kt, blk)]) for (kt, blk) in ops
                    if kt_lo <= kt < kt_hi] if gops else []
            # start/stop per block within this group
            first, last = {}, {}
            for i, (kt, blk, o) in enumerate(mops):
                if blk not in first:
                    first[blk] = i
                last[blk] = i
            flags = [(kt, blk, o, first[blk] == i, last[blk] == i)
                     for i, (kt, blk, o) in enumerate(mops)]
            groups.append(dict(gops=gops, mops=flags, kt_lo=kt_lo,
                               nk=kt_hi - kt_lo,
                               blocks=list(range(g * GBX,
                                                 min((g + 1) * GBX, NBLK)))))
        return dict(KT=KT, ktoff=ktoff, TOTKT=TOTKT, NOP=NOP, groups=groups,
                    per_core=per_core, opidx=opidx, cnt=cnt)

    def fill_slots(sched, ci, sdtype):
        kt, sit, bs, rels, lanes, vals = sched["per_core"][ci]
        nslots = sched["TOTKT"] * 128
        idx_flat = np.zeros(nslots, np.int64)
        idx_flat[kt * 128 + sit] = rels
        assert rels.max(initial=0) < WIN
        S = np.zeros((128, sched["NOP"] * 128), np.float32)
        o = np.array([sched["opidx"][(int(k), int(b))]
                      for k, b in zip(kt, bs)], np.int64)
        np.add.at(S, (sit, o * 128 + lanes), vals)
        return (S.astype(sdtype),
                _rep16(idx_flat.astype(np.int16), nslots))

    # ---- L1 schedules ----------------------------------------------------
    sched1 = []
    for d in range(2):
        percore = []
        for ci in range(C):
            adst, asrc = core_edges[ci][d]
            dpos = pos_own[ci][adst]
            row = zrow[ci][asrc]
            assert (row >= 0).all()
            percore.append((dpos // 128, row // WIN, row % WIN, dpos % 128,
                            dinv[d][asrc].astype(np.float32)))
        sched1.append(build_flat(percore, NB, GB1, NW1))

    # ---- L2 mask schedules (halo only; self via direct slice) -----------
    mk_rank = []          # [C] array [OWN] -> rank in masked list or -1
    for ci in range(C):
        lo = ci * OWN
        r = np.full(OWN, -1, np.int64)
        r[mk_nodes[ci] - lo] = np.arange(MK[ci])
        mk_rank.append(r)

    sched2 = []
    for d in range(2):
        percore = []
        for ci in range(C):
            lo = ci * OWN
            ad_g, as_g = (dst, src) if d == 0 else (src, dst)
            sel = ((ad_g >= lo) & (ad_g < lo + OWN)
                   & is_masked[np.clip(ad_g, 0, N - 1)])
            adst = ad_g[sel] - lo
            md = mk_rank[ci][adst]
            sj = as_g[sel] // OWN        # owner core of source
            srow = sj * OWNP + pos_own_of(sj, as_g[sel] - sj * OWN, pos_own)
            percore.append((md // 128, srow // WIN, srow % WIN, md % 128,
                            dinv[d][lo + adst].astype(np.float32)))
        sched2.append(build_flat(percore, MB, GB2, NW2))

    # ---- per-core inputs -------------------------------------------------
    w1all = (np.concatenate([W["on_td_W1"], W["tgt_td_W1"],
                             W["on_bu_W1"], W["tgt_bu_W1"]], axis=1)
             * W1SCALE).astype(f8)
    w2_td = np.concatenate([W["on_td_W2"], W["tgt_td_W2"]], axis=1).astype(bf16)
    w2_bu = np.concatenate([W["on_bu_W2"], W["tgt_bu_W2"]], axis=1).astype(bf16)
    ton = np.concatenate([token @ W["on_td_W1"], token @ W["on_bu_W1"]])
    tonbc = _bcast(ton).astype(bf16)
    b1bc_td = _bcast(np.concatenate([W["on_td_b1"], W["tgt_td_b1"]]))
    b1bc_bu = _bcast(np.concatenate([W["on_bu_b1"], W["tgt_bu_b1"]]))
    b2bc_td = _bcast(np.concatenate([W["on_td_b2"], W["tgt_td_b2"]]))
    b2bc_bu = _bcast(np.concatenate([W["on_bu_b2"], W["tgt_bu_b2"]]))
    b2col = np.stack(
        [W["on_td_b2"], W["tgt_td_b2"], W["on_bu_b2"], W["tgt_bu_b2"]],
        axis=1).astype(np.float32)                         # [64, 4]
    ones = np.ones((128, 1), np.float32)
    gcount = np.bincount(batch, minlength=G).astype(np.float32)
    cntbc = np.broadcast_to(gcount[None, :128], (128, 128)).copy()

    in_maps = []
    for ci in range(C):
        lo = ci * OWN
        # xT in z-row order
        xT = np.zeros((512, RT), f8)
        xT[:, 0:UM[ci]] = x[um_nodes[ci]].T
        xT[:, UMPAD:UMPAD + MK[ci]] = x[mk_nodes[ci]].T
        xT[:, OWNP:OWNP + HU[ci]] = x[halo_um[ci]].T
        xT[:, OWNP + HUPAD:OWNP + HUPAD + HM[ci]] = x[halo_mk[ci]].T

        def dstarr(dv):
            a = np.ones(OWNP, np.float32)
            a[0:UM[ci]] = dv[um_nodes[ci]]
            a[UMPAD:UMPAD + MK[ci]] = dv[mk_nodes[ci]]
            return np.ascontiguousarray(a.reshape(-1, 128).T)

        def colarr(vals_mk, fill=0.0):
            a = np.full(MKPAD, fill, np.float32)
            a[0:MK[ci]] = vals_mk
            return np.ascontiguousarray(a.reshape(-1, 128).T)  # [128, MB]

        m = dict(xT=xT,
                 ddst_td=dstarr(dinv[0]), ddst_bu=dstarr(dinv[1]),
                 swv_td=colarr(dinv[0][mk_nodes[ci]]),
                 swv_bu=colarr(dinv[1][mk_nodes[ci]]),
                 mcvw=colarr(mcnt_global[mk_nodes[ci]]))
        for d, nm in ((0, "td"), (1, "bu")):
            S, idx = fill_slots(sched1[d], ci, f8)
            m[f"s_{nm}1"], m[f"i_{nm}1"] = S, idx
            S2, idx2 = fill_slots(sched2[d], ci, bf16)
            m[f"s2_{nm}"], m[f"i2_{nm}"] = S2, idx2
            # pool S: out-edges of own nodes + self, grouped by graph
            ad, as_ = (dst, src) if d == 0 else (src, dst)
            dv = dinv[d]
            sel = (as_ >= lo) & (as_ < lo + OWN)
            j = pos_own[ci][as_[sel] - lo]
            gg = batch[ad[sel]]
            v = dv[ad[sel]]
            pp = np.zeros((128, NB * 128), np.float32)
            np.add.at(pp, (j % 128, (j // 128) * 128 + gg), v)
            jj = pos_own[ci]
            np.add.at(pp, (jj % 128, (jj // 128) * 128 + batch[lo:lo + OWN]),
                      dv[lo:lo + OWN])
            m[f"pools_{nm}"] = pp.astype(bf16)
        m.update(w1all=w1all, w2_td=w2_td, w2_bu=w2_bu, tonbc=tonbc,
                 b1bc_td=b1bc_td, b1bc_bu=b1bc_bu,
                 b2bc_td=b2bc_td, b2bc_bu=b2bc_bu, b2col=b2col,
                 ones=ones, cntbc=cntbc)
        in_maps.append(m)

    meta = dict(RT=RT, NW1=NW1, NW2=NW2, NB=NB, MB=MB, OWNP=OWNP,
                UMPAD=UMPAD, MKPAD=MKPAD, NPAD=NPAD,
                sections=sections, sched1=sched1, sched2=sched2)
    return meta, in_maps


def pos_own_of(owner_cores, local_idx, pos_own):
    """vectorized pos_own lookup across owner cores"""
    out = np.empty(len(local_idx), np.int64)
    for j in np.unique(owner_cores):
        sel = owner_cores == j
        out[sel] = pos_own[j][local_idx[sel]]
    return out


# ---------------------------------------------------------------- program

def build_program(meta):
    import concourse.bass as bass
    import concourse.bacc as bacc
    import concourse.mybir as mybir
    import concourse.tile as tile
    from concourse.masks import make_identity

    RT, NB, MB = meta["RT"], meta["NB"], meta["MB"]
    NW1, NW2 = meta["NW1"], meta["NW2"]
    OWNP, UMPAD, NPAD = meta["OWNP"], meta["UMPAD"], meta["NPAD"]
    f32, bf, i16 = mybir.dt.float32, mybir.dt.bfloat16, mybir.dt.int16
    f8 = mybir.dt.float8e4
    MUL, ADD = mybir.AluOpType.mult, mybir.AluOpType.add

    nc = bacc.Bacc("TRN2", target_bir_lowering=False, debug=False,
                   num_devices=C)

    def din(name, shape, dt):
        return nc.dram_tensor(name, shape, dt, kind="ExternalInput")

    xT = din("xT", [512, RT], f8)
    ddst = [din("ddst_td", [128, NB], f32), din("ddst_bu", [128, NB], f32)]
    s1 = [din("s_td1", [128, meta["sched1"][0]["NOP"] * 128], f8),
          din("s_bu1", [128, meta["sched1"][1]["NOP"] * 128], f8)]
    i1 = [din("i_td1", [128, meta["sched1"][0]["TOTKT"] * 8], i16),
          din("i_bu1", [128, meta["sched1"][1]["TOTKT"] * 8], i16)]
    s2 = [din("s2_td", [128, meta["sched2"][0]["NOP"] * 128], bf),
          din("s2_bu", [128, meta["sched2"][1]["NOP"] * 128], bf)]
    i2 = [din("i2_td", [128, meta["sched2"][0]["TOTKT"] * 8], i16),
          din("i2_bu", [128, meta["sched2"][1]["TOTKT"] * 8], i16)]
    pools_t = [din("pools_td", [128, NB * 128], bf),
               din("pools_bu", [128, NB * 128], bf)]
    swv_t = [din("swv_td", [128, MB], f32), din("swv_bu", [128, MB], f32)]
    mcvw_t = din("mcvw", [128, MB], f32)
    w1all = din("w1all", [512, 512], f8)
    w2 = [din("w2_td", [128, 128], bf), din("w2_bu", [128, 128], bf)]
    tonbc = din("tonbc", [128, 256], bf)
    b1bc = [din("b1bc_td", [128, 256], f32), din("b1bc_bu", [128, 256], f32)]
    b2bc = [din("b2bc_td", [128, 128], f32), din("b2bc_bu", [128, 128], f32)]
    b2col_t = din("b2col", [64, 4], f32)
    ones_t = din("ones", [128, 1], f32)
    cntbc_t = din("cntbc", [128, 128], f32)
    loss_t = nc.dram_tensor("loss", [1, 1], f32, kind="ExternalOutput")

    z_ws = [nc.dram_tensor(f"zarr{w}", [min(WIN, RT - w * WIN), 512], f8,
                           kind="Internal")
            for w in range(NW1)]

    with tile.TileContext(nc) as tc:
        with (
            tc.tile_pool(name="const", bufs=1) as cpool,
            tc.tile_pool(name="dram", bufs=1, space="DRAM") as dpool,
        ):
            z2own = [dpool.tile([OWNP, 128], bf, tag=f"z2own{d}",
                                name=f"z2own{d}") for d in range(2)]
            z2full = [dpool.tile([NPAD, 128], bf, addr_space="Shared",
                                 tag=f"z2full{d}", name=f"z2full{d}")
                      for d in range(2)]
            ar_in = dpool.tile([128, 520], f32, tag="arin", name="arin")
            ar_out = dpool.tile([128, 520], f32, addr_space="Shared",
                                tag="arout", name="arout")

            ident = cpool.tile([128, 128], bf)
            make_identity(nc, ident[:])
            w1sb = cpool.tile([128, 4 * 512], f8)
            for k in range(4):
                nc.sync.dma_start(out=w1sb[:, k * 512:(k + 1) * 512],
                                  in_=w1all[k * 128:(k + 1) * 128, :])
            w2sb = [cpool.tile([128, 128], bf, tag=f"w2_{d}", name=f"w2_{d}")
                    for d in range(2)]
            tonsb = cpool.tile([128, 256], bf)
            b1sb = [cpool.tile([128, 256], f32, tag=f"b1_{d}", name=f"b1_{d}")
                    for d in range(2)]
            b2sb = [cpool.tile([128, 128], f32, tag=f"b2_{d}", name=f"b2_{d}")
                    for d in range(2)]
            ddsb = [cpool.tile([128, NB], f32, tag=f"dd_{d}", name=f"dd_{d}")
                    for d in range(2)]
            swsb = [cpool.tile([128, MB], f32, tag=f"sw_{d}", name=f"sw_{d}")
                    for d in range(2)]
            mcsb = cpool.tile([128, MB], f32)
            onesb = cpool.tile([128, 1], f32)
            nc.sync.dma_start(out=tonsb[:], in_=tonbc[:, :])
            nc.sync.dma_start(out=onesb[:], in_=ones_t[:, :])
            nc.sync.dma_start(out=mcsb[:], in_=mcvw_t[:, :])
            for d in range(2):
                nc.sync.dma_start(out=w2sb[d][:], in_=w2[d][:, :])
                nc.sync.dma_start(out=b1sb[d][:], in_=b1bc[d][:, :])
                nc.sync.dma_start(out=b2sb[d][:], in_=b2bc[d][:, :])
                nc.sync.dma_start(out=ddsb[d][:], in_=ddst[d][:, :])
                nc.sync.dma_start(out=swsb[d][:], in_=swv_t[d][:, :])

            # ================= P1: z = scaled([x1|x] @ W1-fused) ==========
            with (
                tc.tile_pool(name="xk", bufs=2) as xkp,
                tc.tile_pool(name="zsb", bufs=3) as zsp,
                tc.tile_pool(name="pz", bufs=2, space="PSUM") as pzp,
            ):
                DR = mybir.MatmulPerfMode.DoubleRow
                jpar = 0
                for (r0, rlen, msk) in meta["sections"]:
                    for off in range(0, rlen, NF):
                        nf = min(NF, rlen - off)
                        xk = xkp.tile([128, 4 * NF], f8, tag="xk", name="xk")
                        for k in range(4):
                            nc.sync.dma_start(
                                out=xk[:, k * NF:k * NF + nf],
                                in_=xT[k * 128:(k + 1) * 128,
                                       r0 + off:r0 + off + nf])
                        xk3 = xk[:].rearrange("p (k n) -> p k n", k=4, n=NF)
                        w13 = w1sb[:].rearrange("p (k n) -> p k n", k=4,
                                                n=512)
                        for j in range(nf // 128):
                            row = r0 + off + j * 128
                            jpar += 1
                            zs = zsp.tile([128, 512], f8, tag="zs", name="zs")
                            if not msk:
                                ps = pzp.tile([128, 512], f32, tag="pz",
                                              name="pz")
                                for k in range(0, 4, 2):
                                    nc.tensor.matmul(
                                        out=ps[:],
                                        lhsT=xk3[:, k:k + 2,
                                                 j * 128:(j + 1) * 128],
                                        rhs=w13[:, k:k + 2, :],
                                        start=(k == 0), stop=(k == 2),
                                        perf_mode=DR)
                                if jpar % 3 == 0:
                                    nc.scalar.activation(
                                        out=zs[:], in_=ps[:],
                                        func=mybir.ActivationFunctionType.Copy,
                                        scale=1.0 / W1SCALE)
                                else:
                                    nc.vector.tensor_scalar(
                                        out=zs[:], in0=ps[:],
                                        scalar1=1.0 / W1SCALE,
                                        scalar2=None, op0=MUL)
                            else:
                                ps = pzp.tile([128, 512], f32, tag="pz",
                                              name="pz")
                                for h in range(2):
                                    c0 = h * 256 + 128
                                    for k in range(0, 4, 2):
                                        nc.tensor.matmul(
                                            out=ps[:, h * 128:(h + 1) * 128],
                                            lhsT=xk3[:, k:k + 2,
                                                     j * 128:(j + 1) * 128],
                                            rhs=w13[:, k:k + 2, c0:c0 + 128],
                                            start=(k == 0), stop=(k == 2),
                                            perf_mode=DR)
                                for h in range(2):
                                    nc.vector.tensor_copy(
                                        out=zs[:, h * 256:h * 256 + 128],
                                        in_=tonsb[:, h * 128:(h + 1) * 128])
                                    if jpar % 3 == 0:
                                        nc.scalar.activation(
                                            out=zs[:, h * 256 + 128:
                                                   (h + 1) * 256],
                                            in_=ps[:, h * 128:(h + 1) * 128],
                                            func=mybir.ActivationFunctionType.Copy,
                                            scale=1.0 / W1SCALE)
                                    else:
                                        nc.vector.tensor_scalar(
                                            out=zs[:, h * 256 + 128:
                                                   (h + 1) * 256],
                                            in0=ps[:, h * 128:(h + 1) * 128],
                                            scalar1=1.0 / W1SCALE,
                                            scalar2=None, op0=MUL)
                            zw = row // WIN
                            zr = row - zw * WIN
                            weng = nc.scalar if jpar % 3 == 1 else nc.sync
                            weng.dma_start(out=z_ws[zw][zr:zr + 128, :],
                                           in_=zs[:])

            # ===== L1 agg + finalize (z2 + fused pool), per direction =====
            poolpool_cm = tc.tile_pool(name="plps", bufs=1, space="PSUM")
            poolpool = poolpool_cm.__enter__()
            poolps = poolpool.tile([128, 512], f32, tag="pl", name="pl")

            def l1_dir(d):
                sch = meta["sched1"][d]
                wlen = lambda w: min(WIN, RT - w * WIN)
                with (
                    tc.tile_pool(name=f"g1{d}", bufs=4) as gp,
                    tc.tile_pool(name=f"sI1{d}", bufs=3) as sp,
                    tc.tile_pool(name=f"ix1{d}", bufs=1) as ip,
                    tc.tile_pool(name=f"ps1{d}", bufs=2) as pwp,
                    tc.tile_pool(name=f"fin1{d}", bufs=3) as fp,
                    tc.tile_pool(name=f"h1q{d}", bufs=20) as h1p,
                    tc.tile_pool(name=f"zrb{d}", bufs=3) as zrp,
                    tc.tile_pool(name=f"agg{d}", bufs=1, space="PSUM") as ap,
                    tc.tile_pool(name=f"tr{d}", bufs=2, space="PSUM") as trp,
                    tc.tile_pool(name=f"z2p{d}", bufs=1, space="PSUM") as z2p,
                ):
                    # stage B (transpose -> @W2 -> scale -> z2own write +
                    # fused pool matmuls), decoupled from the agg pipeline
                    # via the deep h1 tile pool and one-group emission skew.
                    def stage_b(blk, bi, h1, pst):
                        trt = trp.tile([128, 256], bf, tag="t", name="t")
                        nc.tensor.transpose(
                            out=trt[:, 0:128], in_=h1[:, 0:128],
                            identity=ident[:])
                        nc.tensor.transpose(
                            out=trt[:, 128:256], in_=h1[:, 128:256],
                            identity=ident[:])
                        h1T = fp.tile([128, 256], bf, tag="h1T", name="h1T")
                        nc.scalar.copy(out=h1T[:], in_=trt[:])
                        z2ps = z2p.tile([128, 128], f32, tag="z2", name="z2")
                        nc.tensor.matmul(out=z2ps[:, 0:64],
                                         lhsT=h1T[:, 0:128],
                                         rhs=w2sb[d][:, 0:64],
                                         start=True, stop=True)
                        nc.tensor.matmul(out=z2ps[:, 64:128],
                                         lhsT=h1T[:, 128:256],
                                         rhs=w2sb[d][:, 64:128],
                                         start=True, stop=True)
                        z2sb = fp.tile([128, 128], bf, tag="z2sb",
                                       name="z2sb")
                        nc.vector.tensor_scalar(
                            out=z2sb[:], in0=z2ps[:],
                            scalar1=ddsb[d][:, blk:blk + 1],
                            scalar2=None, op0=MUL)
                        nc.sync.dma_start(
                            out=z2own[d][blk * 128:(blk + 1) * 128, :],
                            in_=z2sb[:])
                        nc.tensor.matmul(
                            out=poolps[0:64, d * 256:d * 256 + 128],
                            lhsT=z2sb[:, 0:64],
                            rhs=pst[:, bi * 128:(bi + 1) * 128],
                            start=(blk == 0), stop=(blk == NB - 1),
                            skip_group_check=True)
                        nc.tensor.matmul(
                            out=poolps[0:64, d * 256 + 128:d * 256 + 256],
                            lhsT=z2sb[:, 64:128],
                            rhs=pst[:, bi * 128:(bi + 1) * 128],
                            start=(blk == 0), stop=(blk == NB - 1),
                            skip_group_check=True)

                    itall = ip.tile([128, max(sch["TOTKT"], 1) * 8], i16,
                                    tag="ia", name="ia")
                    nc.gpsimd.dma_start(out=itall[:], in_=i1[d][:, :])
                    pending = []
                    if True:
                        for g, grp in enumerate(sch["groups"]):
                            blocks = grp["blocks"]
                            nops = len(grp["mops"])
                            gt = None
                            if grp["gops"]:
                                gt = gp.tile([128, grp["nk"] * 256], f8,
                                             tag="g", name="g")
                                for (ww, ktb, nkw) in grp["gops"]:
                                    o = ktb - grp["kt_lo"]
                                    nc.gpsimd.dma_gather(
                                        gt[:, o * 256:(o + nkw) * 256]
                                        .rearrange("p (k e) -> p k e",
                                                   k=nkw, e=256),
                                        z_ws[ww][0:wlen(ww),
                                                 256 * d:256 * d + 256],
                                        itall[:, ktb * 8:(ktb + nkw) * 8],
                                        nkw * 128, nkw * 128, 256,
                                        elem_step=512, single_packet=False)
                            if nops:
                                st = sp.tile([128, nops * 128], f8, tag="s",
                                             name="s")
                                nc.scalar.dma_start(
                                    out=st[:],
                                    in_=s1[d][:, grp["mops"][0][2] * 128:
                                              (grp["mops"][0][2] + nops)
                                              * 128])
                                aps = ap.tile([128, len(blocks) * 256], f32,
                                              tag="a", name="a")
                                o0 = grp["mops"][0][2]
                                for (kt, blk, o, st_f, sp_f) in grp["mops"]:
                                    bi = blk - blocks[0]
                                    nc.tensor.matmul(
                                        out=aps[:, bi * 256:(bi + 1) * 256],
                                        lhsT=st[:, (o - o0) * 128:
                                                (o - o0 + 1) * 128],
                                        rhs=gt[:, (kt - grp["kt_lo"]) * 256:
                                               (kt - grp["kt_lo"] + 1)
                                               * 256],
                                        start=st_f, stop=sp_f,
                                        skip_group_check=True)
                            has = {blk for (_, blk, _, _, _) in grp["mops"]}
                            # pool S slab for this group
                            pst = pwp.tile([128, len(blocks) * 128], bf,
                                           tag="ps", name="ps")
                            nc.sync.dma_start(
                                out=pst[:],
                                in_=pools_t[d][:, blocks[0] * 128:
                                               (blocks[0] + len(blocks))
                                               * 128])
                            newly = []
                            for blk in blocks:
                                bi = blk - blocks[0]
                                zrb = zrp.tile([128, 256], f8, tag="zr",
                                               name="zr")
                                nc.sync.dma_start(
                                    out=zrb[:],
                                    in_=z_ws[0][blk * 128:(blk + 1) * 128,
                                                256 * d:256 * d + 256])
                                hs = fp.tile([128, 256], f32, tag="hs",
                                             name="hs")
                                if blk in has:
                                    # hs = h_self*dinv_dst + agg
                                    nc.vector.scalar_tensor_tensor(
                                        out=hs[:], in0=zrb[:],
                                        scalar=ddsb[d][:, blk:blk + 1],
                                        in1=aps[:, bi * 256:(bi + 1) * 256],
                                        op0=MUL, op1=ADD)
                                else:
                                    nc.vector.tensor_scalar(
                                        out=hs[:], in0=zrb[:],
                                        scalar1=ddsb[d][:, blk:blk + 1],
                                        scalar2=None, op0=MUL)
                                # h1 = relu(hs*ddst + b1)
                                nc.vector.scalar_tensor_tensor(
                                    out=hs[:], in0=hs[:],
                                    scalar=ddsb[d][:, blk:blk + 1],
                                    in1=b1sb[d][:, 0:256], op0=MUL, op1=ADD)
                                h1 = h1p.tile([128, 256], bf, tag="h1",
                                              name="h1")
                                nc.scalar.activation(
                                    out=h1[:], in_=hs[:],
                                    func=mybir.ActivationFunctionType.Relu)
                                newly.append((blk, bi, h1, pst))
                            for item in pending:
                                stage_b(*item)
                            pending = newly
                    for item in pending:
                        stage_b(*item)

            def allgather(d):
                nc.gpsimd.collective_compute(
                    "AllGather", mybir.AluOpType.bypass,
                    replica_groups=[list(range(C))],
                    ins=[z2own[d].opt()], outs=[z2full[d].opt()])

            l1_dir(0)
            allgather(0)
            l1_dir(1)
            allgather(1)

            # drain pooled sums
            arsb = cpool.tile([128, 520], f32)
            nc.vector.memset(arsb[:], 0.0)
            for d in range(2):
                nc.vector.tensor_copy(out=arsb[0:64, d * 256:(d + 1) * 256],
                                      in_=poolps[0:64, d * 256:(d + 1) * 256])
            poolpool_cm.__exit__(None, None, None)

            # ========== L2 mask aggregation (node-major) ==================
            # wide per-dir product tiles
            prodw = [[cpool.tile([128, MB], f32, tag=f"pw{d}{q}",
                                 name=f"pw{d}{q}") for q in range(3)]
                     for d in range(2)]
            for d in range(2):
                for q in range(3):
                    nc.vector.memset(prodw[d][q][:], 0.0)

            def l2_dir(d):
                sch = meta["sched2"][d]
                wlen = lambda w: min(WIN, NPAD - w * WIN)
                with (
                    tc.tile_pool(name=f"g2{d}", bufs=3) as gp,
                    tc.tile_pool(name=f"sI2{d}", bufs=3) as sp,
                    tc.tile_pool(name=f"ix2{d}", bufs=3) as ip,
                    tc.tile_pool(name=f"fin2{d}", bufs=3) as fp,
                    tc.tile_pool(name=f"zsl{d}", bufs=3) as zp,
                    tc.tile_pool(name=f"mag{d}", bufs=2, space="PSUM") as ap,
                ):
                    for g, grp in enumerate(sch["groups"]):
                        blocks = grp["blocks"]
                        nops = len(grp["mops"])
                        nk = grp["nk"]
                        if nops:
                            st = sp.tile([128, nops * 128], bf, tag="s",
                                         name="s")
                            nc.sync.dma_start(
                                out=st[:],
                                in_=s2[d][:, grp["mops"][0][2] * 128:
                                          (grp["mops"][0][2] + nops) * 128])
                            it = ip.tile([128, nk * 8], i16, tag="i", name="i")
                            nc.sync.dma_start(
                                out=it[:], in_=i2[d][:, grp["kt_lo"] * 8:
                                                     (grp["kt_lo"] + nk) * 8])
                            gt = gp.tile([128, nk * 128], bf, tag="g",
                                         name="g")
                            for w, ktb, nkw in grp["gops"]:
                                o = ktb - grp["kt_lo"]
                                nc.gpsimd.dma_gather(
                                    gt[:, o * 128:(o + nkw) * 128].rearrange(
                                        "p (k e) -> p k e", k=nkw, e=128),
                                    z2full[d][w * WIN:w * WIN + wlen(w), :],
                                    it[:, o * 8:(o + nkw) * 8],
                                    nkw * 128, nkw * 128, 128,
                                    elem_step=None, single_packet=False)
                            aps = ap.tile([128, len(blocks) * 128], f32,
                                          tag="a", name="a")
                            o0 = grp["mops"][0][2]
                            for (kt, blk, o, st_f, sp_f) in grp["mops"]:
                                bi = blk - blocks[0]
                                nc.tensor.matmul(
                                    out=aps[:, bi * 128:(bi + 1) * 128],
                                    lhsT=st[:, (o - o0) * 128:
                                            (o - o0 + 1) * 128],
                                    rhs=gt[:, (kt - grp["kt_lo"]) * 128:
                                           (kt - grp["kt_lo"] + 1) * 128],
                                    start=st_f, stop=sp_f,
                                    skip_group_check=True)
                        has = {blk for (_, blk, _, _, _) in grp["mops"]}
                        for blk in blocks:
                            bi = blk - blocks[0]
                            zsl = zp.tile([128, 128], bf, tag="zs", name="zs")
                            nc.sync.dma_start(
                                out=zsl[:],
                                in_=z2own[d][UMPAD + blk * 128:
                                             UMPAD + (blk + 1) * 128, :])
                            hs = fp.tile([128, 128], f32, tag="hs", name="hs")
                            # hs = z_self*swv (+ agg)
                            if blk in has:
                                nc.vector.scalar_tensor_tensor(
                                    out=hs[:], in0=zsl[:],
                                    scalar=swsb[d][:, blk:blk + 1],
                                    in1=aps[:, bi * 128:(bi + 1) * 128],
                                    op0=MUL, op1=ADD)
                            else:
                                nc.vector.tensor_scalar(
                                    out=hs[:], in0=zsl[:],
                                    scalar1=swsb[d][:, blk:blk + 1],
                                    scalar2=None, op0=MUL)
                            nc.vector.tensor_tensor(
                                out=hs[:], in0=hs[:], in1=b2sb[d][:, 0:128],
                                op=ADD)
                            # products (accumulate over 64-feat free dim)
                            scr = fp.tile([128, 64], f32, tag="sc", name="sc")
                            for q, (p0, p1) in enumerate(
                                    ((0, 64), (0, 0), (64, 64))):
                                nc.vector.scalar_tensor_tensor(
                                    out=scr[:], in0=hs[:, p0:p0 + 64],
                                    scalar=1.0, in1=hs[:, p1:p1 + 64],
                                    op0=MUL, op1=MUL,
                                    accum_out=prodw[d][q][:, blk:blk + 1])

            l2_dir(0)
            l2_dir(1)

            # ========== masked SCE epilogue (wide) ========================
            with tc.tile_pool(name="ep", bufs=1) as ep:
                su = [ep.tile([128, MB], f32, tag=f"su{q}", name=f"su{q}")
                      for q in range(3)]
                for q in range(3):
                    nc.vector.tensor_tensor(out=su[q][:], in0=prodw[0][q][:],
                                            in1=prodw[1][q][:], op=ADD)

                def rsq(n, tag):
                    r = ep.tile([128, MB], f32, tag=tag, name=tag)
                    nc.scalar.sqrt(out=r[:], in_=n[:])
                    nc.vector.tensor_scalar_max(out=r[:], in0=r[:],
                                                scalar1=1e-12)
                    nc.vector.reciprocal(out=r[:], in_=r[:])
                    return r

                r1 = rsq(su[1], "r1")
                r2 = rsq(su[2], "r2")
                tt = ep.tile([128, MB], f32, tag="tt", name="tt")
                nc.vector.tensor_tensor(out=tt[:], in0=su[0][:], in1=r1[:],
                                        op=MUL)
                nc.vector.tensor_tensor(out=tt[:], in0=tt[:], in1=r2[:],
                                        op=MUL)
                nc.vector.tensor_tensor(out=tt[:], in0=tt[:], in1=mcsb[:],
                                        op=MUL)
                scr = ep.tile([128, MB], f32, tag="scr", name="scr")
                colsum = ep.tile([128, 1], f32, tag="cs", name="cs")
                nc.vector.scalar_tensor_tensor(
                    out=scr[:], in0=tt[:], scalar=-1.0, in1=mcsb[:],
                    op0=MUL, op1=ADD, accum_out=colsum[:])
                with tc.tile_pool(name="eps", bufs=1, space="PSUM") as epp:
                    macc_ps = epp.tile([1, 1], f32, tag="mp", name="mp")
                    nc.tensor.matmul(out=macc_ps[:], lhsT=colsum[:],
                                     rhs=onesb[:], start=True, stop=True)
                    nc.vector.tensor_copy(out=arsb[0:1, 512:513],
                                          in_=macc_ps[:])

            # ========== AllReduce (pools + mask partial) =================
            nc.sync.dma_start(out=ar_in[:, :], in_=arsb[:])
            nc.gpsimd.collective_compute(
                "AllReduce", mybir.AluOpType.add,
                replica_groups=[list(range(C))],
                ins=[ar_in.opt()], outs=[ar_out.opt()])

            # ========== pooled cosine + final loss =======================
            with (
                tc.tile_pool(name="fin3", bufs=2) as f2,
                tc.tile_pool(name="fps", bufs=2, space="PSUM") as fpp,
            ):
                ar2 = f2.tile([128, 520], f32, tag="ar2", name="ar2")
                nc.sync.dma_start(out=ar2[:], in_=ar_out[:, :])
                cntsb = f2.tile([128, 128], f32, tag="cnt", name="cnt")
                nc.sync.dma_start(out=cntsb[:], in_=cntbc_t[:, :])
                b2t = f2.tile([64, 4], f32, tag="b2tf", name="b2tf")
                nc.sync.dma_start(out=b2t[:], in_=b2col_t[:, :])
                pf = {}
                for d in range(2):
                    for h in range(2):
                        po = f2.tile([64, 128], f32, tag=f"po{d}{h}",
                                     name=f"po{d}{h}")
                        nc.vector.scalar_tensor_tensor(
                            out=po[:], in0=cntsb[0:64, :],
                            scalar=b2t[0:64, 2 * d + h:2 * d + h + 1],
                            in1=ar2[0:64, d * 256 + h * 128:
                                    d * 256 + (h + 1) * 128],
                            op0=MUL, op1=ADD)
                        pf[(d, h)] = po
                gsums = []
                for qi, pick in enumerate(((0, 1), (0, 0), (1, 1))):
                    qp = fpp.tile([1, 128], f32, tag="gqp", name="gqp")
                    for d in range(2):
                        pr = f2.tile([64, 128], f32, tag=f"gpr{d}",
                                     name=f"gpr{d}")
                        nc.vector.tensor_tensor(
                            out=pr[:], in0=pf[(d, pick[0])][:],
                            in1=pf[(d, pick[1])][:], op=MUL)
                        nc.tensor.matmul(
                            out=qp[:], lhsT=onesb[0:64, 0:1], rhs=pr[:],
                            start=(d == 0), stop=(d == 1),
                            skip_group_check=True)
                    sq = f2.tile([1, 128], f32, tag=f"gsq{qi}",
                                 name=f"gsq{qi}")
                    nc.vector.tensor_copy(out=sq[:], in_=qp[:])
                    gsums.append(sq)
                gdot, gn1, gn2 = gsums

                def rguard2(n, tag):
                    r = f2.tile([1, 128], f32, tag=tag, name=tag)
                    nc.scalar.sqrt(out=r[:], in_=n[:])
                    nc.vector.tensor_scalar_max(out=r[:], in0=r[:],
                                                scalar1=1e-12)
                    nc.vector.reciprocal(out=r[:], in_=r[:])
                    return r

                g1 = rguard2(gn1, "g1")
                g2 = rguard2(gn2, "g2")
                cosg = f2.tile([1, 128], f32, tag="cosg", name="cosg")
                nc.vector.tensor_tensor(out=cosg[:], in0=gdot[:], in1=g1[:],
                                        op=MUL)
                nc.vector.tensor_tensor(out=cosg[:], in0=cosg[:], in1=g2[:],
                                        op=MUL)
                onesrow = f2.tile([1, 128], f32, tag="onesr", name="onesr")
                nc.vector.memset(onesrow[:], 1.0)
                gterm = f2.tile([1, 128], f32, tag="gterm", name="gterm")
                gs = f2.tile([1, 1], f32, tag="gs", name="gs")
                nc.vector.scalar_tensor_tensor(
                    out=gterm[:], in0=cosg[:], scalar=-1.0, in1=onesrow[:],
                    op0=MUL, op1=ADD, accum_out=gs[:])
                l1t = f2.tile([1, 1], f32, tag="l1", name="l1")
                nc.scalar.activation(out=l1t[:], in_=gs[:],
                                     func=mybir.ActivationFunctionType.Copy,
                                     scale=1.0 / G)
                l2t = f2.tile([1, 1], f32, tag="l2", name="l2")
                nc.scalar.activation(out=l2t[:], in_=ar2[0:1, 512:513],
                                     func=mybir.ActivationFunctionType.Copy,
                                     scale=1.0 / M)
                nc.vector.tensor_tensor(out=l1t[:], in0=l1t[:], in1=l2t[:],
                                        op=ADD)
                nc.sync.dma_start(out=loss_t[:, :], in_=l1t[:])

    return nc


# ---------------------------------------------------------------- entry

LAST_RESULT = None


def _install_trace_hook():
    """The agent image's antenv lacks axon_hooks; synthesize it from
    trn_boot's ctypes NTFF hook so trace=True works under axon."""
    import types
    try:
        from antenv import axon_hooks  # noqa: F401
        return
    except ImportError:
        pass
    try:
        import antenv
        import trn_agent_boot.trn_boot as tb
        hook = tb._ntff_profile_via_ctypes("/opt/axon/libaxon_pjrt.so")
        mod = types.ModuleType("antenv.axon_hooks")
        mod.get_axon_ntff_profile_hook = lambda: hook
        mod.set_axon_ntff_profile_hook = lambda h: None
        sys.modules["antenv.axon_hooks"] = mod
        antenv.axon_hooks = mod
    except Exception as e:
        print(f"[kernel] trace hook install failed: {e}", file=sys.stderr)


def kernel(_trace=False, **inputs):
    global LAST_RESULT
    import time
    from concourse import bass_utils
    if _trace:
        _install_trace_hook()
    t0 = time.monotonic()
    meta, in_maps = host_prep(inputs)
    t1 = time.monotonic()
    nc = build_program(meta)
    t2 = time.monotonic()
    nc.compile()
    t3 = time.monotonic()
    res = bass_utils.run_bass_kernel_spmd(
        nc, in_maps, core_ids=list(range(C)),
        trace=_trace, trace_cores=[0] if _trace else None)
    t4 = time.monotonic()
    print(f"[kernel] prep {t1-t0:.1f}s build {t2-t1:.1f}s "
          f"compile {t3-t2:.1f}s run {t4-t3:.1f}s", file=sys.stderr)
    LAST_RESULT = res
    return np.float32(res.results[0]["loss"][0, 0])


# revision 39
# speedup vs baseline: 1.1892x; 1.0801x over previous
"""Trainium2 Bass kernel for the rumor-GCN masked-autoencoder loss.

Strategy (8 NeuronCores, SPMD single NEFF):
  - Nodes partitioned into 8 contiguous ranges (25000 each), then per-core
    RE-ORDERED: unmasked own nodes first [0, UM), masked own compact at
    [UMPAD, UMPAD+MK).  All host-side index maps are relabeled, so the
    permutation is free at runtime and makes (a) mask-aggregation self terms
    a contiguous z2own slice and (b) L1 self-loop terms a contiguous z
    readback -- neither needs dma_gather (~8ns/idx on GpSimd, the dominant
    cost; see /root/problem/microbench.py).
  - z = [x1|x] @ W1 for all 4 GCN heads in one fused [512->512] bf16 matmul
    over the per-core needed set (own + halo, pre-gathered by host).  Row
    scales dinv[src] folded at the copy-out, dinv[dst] at finalize.
  - L1 edge aggregation: flat slot schedule bucketed by (group-of-8-dst-
    blocks, z-window).  Slots sorted by dst block inside each bucket, padded
    only at bucket tails; one dma_gather per bucket; one matmul per
    (K-tile x dst-block-segment) with host-built one-hot S.  Tiles may span
    dst blocks (extra matmul, no extra gather).  Self-loop term z[own]
    added at finalize via direct DMA readback.  global_add_pool is fused
    into the finalize: pool[g] += z2sb^T @ poolS (src-side rewrite).
  - L2 is only needed at masked nodes.  Mask aggregation is node-major
    ([128 masked nodes, 128 feat(on|tgt)] PSUM per block): halo edges
    gathered from the AllGathered z2full with the same flat scheduling;
    self term + b2 bias added at finalize from the contiguous z2own slice.
    Cosine terms reduce along the free dim via accum_out into per-block
    columns; one short wide chain finishes the masked SCE.
  - Each direction's z2 AllGather is issued as soon as that direction's L1
    finishes, overlapping the other direction's aggregation; pooled sums +
    the mask partial go through one small AllReduce.
"""

import sys

import numpy as np

sys.path.insert(0, "/opt/trn_rl_repo")

# ---------------------------------------------------------------- config

WIN = 32768
GB1 = 8       # L1 dst blocks per PSUM group
GB2 = 8       # L2 mask blocks per PSUM group
NF = 2048     # P1 column chunk

N, E, G, M, C = 200000, 400000, 128, 100000, 8
OWN = N // C

_WNAMES = [p + s for p in ("on_td", "on_bu", "tgt_td", "tgt_bu")
           for s in ("_W1", "_b1", "_W2", "_b2")]


def _rep16(idx_flat, nslots):
    """int16 index list -> [128, nslots//16] layout (16-part wrap, 8x rep)."""
    blk = np.zeros((16, nslots // 16), dtype=np.int16)
    k = np.arange(len(idx_flat))
    blk[k % 16, k // 16] = idx_flat
    return np.tile(blk, (8, 1))


def _bcast(vec, parts=128):
    return np.broadcast_to(np.asarray(vec)[None, :], (parts, len(vec))).copy()


def _pad128(n):
    return -(-n // 128) * 128


# ---------------------------------------------------------------- host prep

W1SCALE = 16.0  # lift fp8 W1 out of the subnormal range; undone in dloc


def host_prep(inp):
    import ml_dtypes
    bf16 = ml_dtypes.bfloat16
    f8 = ml_dtypes.float8_e4m3
    x = np.asarray(inp["x"], np.float32)
    token = np.asarray(inp["enc_mask_token"], np.float32).reshape(-1)
    ei = np.asarray(inp["edge_index"])
    src, dst = ei[0].astype(np.int64), ei[1].astype(np.int64)
    batch = np.asarray(inp["batch"]).astype(np.int64)
    mask_nodes = np.asarray(inp["mask_nodes"]).astype(np.int64)
    W = {k: np.asarray(inp[k], np.float32) for k in _WNAMES}

    dinv = [
        (1.0 / np.sqrt(np.bincount(dst, minlength=N) + 1.0)).astype(np.float32),
        (1.0 / np.sqrt(np.bincount(src, minlength=N) + 1.0)).astype(np.float32),
    ]
    is_masked = np.zeros(N, bool)
    is_masked[mask_nodes] = True
    mcnt_global = np.bincount(mask_nodes, minlength=N).astype(np.float32)
    xbf = x.astype(bf16)

    # ---- per-core own-node permutation: unmasked first, masked at tail
    um_nodes, mk_nodes = [], []
    for ci in range(C):
        lo = ci * OWN
        m = is_masked[lo:lo + OWN]
        um_nodes.append(np.where(~m)[0] + lo)
        mk_nodes.append(np.where(m)[0] + lo)
    UM = [len(a) for a in um_nodes]
    MK = [len(a) for a in mk_nodes]
    UMPAD = _pad128(max(UM))
    MKPAD = _pad128(max(MK))
    OWNP = UMPAD + MKPAD
    NB = OWNP // 128
    MB = MKPAD // 128
    NPAD = C * OWNP
    NW2 = -(-NPAD // WIN)

    pos_own = []          # [C] array [OWN] -> p-order position
    for ci in range(C):
        lo = ci * OWN
        p = np.empty(OWN, np.int64)
        p[um_nodes[ci] - lo] = np.arange(UM[ci])
        p[mk_nodes[ci] - lo] = UMPAD + np.arange(MK[ci])
        pos_own.append(p)

    # ---- per-core edge lists (dir 0 = TD: dst-agg; dir 1 = BU: src-agg)
    core_edges = []       # [core][dir] -> (adst_local, asrc_global)
    for ci in range(C):
        lo, hi = ci * OWN, (ci + 1) * OWN
        per = []
        for d in range(2):
            ad, as_ = (dst, src) if d == 0 else (src, dst)
            sel = (ad >= lo) & (ad < hi)
            per.append((ad[sel] - lo, as_[sel]))
        core_edges.append(per)

    # ---- halo sets (union over both dirs), split unmasked/masked
    halo_um, halo_mk = [], []
    for ci in range(C):
        lo, hi = ci * OWN, (ci + 1) * OWN
        srcs = np.unique(np.concatenate(
            [core_edges[ci][0][1], core_edges[ci][1][1]]))
        srcs = srcs[(srcs < lo) | (srcs >= hi)]
        halo_um.append(srcs[~is_masked[srcs]])
        halo_mk.append(srcs[is_masked[srcs]])
    HU = [len(a) for a in halo_um]
    HM = [len(a) for a in halo_mk]
    HUPAD = _pad128(max(HU))
    HMPAD = _pad128(max(HM))
    RT = OWNP + HUPAD + HMPAD
    NW1 = -(-RT // WIN)

    # z-row map per core: global node -> z row (own p-order | halo)
    zrow = []
    for ci in range(C):
        lo = ci * OWN
        zm = np.full(N, -1, np.int64)
        zm[lo + np.arange(OWN)] = pos_own[ci]
        zm[halo_um[ci]] = OWNP + np.arange(HU[ci])
        zm[halo_mk[ci]] = OWNP + HUPAD + np.arange(HM[ci])
        zrow.append(zm)

    # P1 sections: (row0, rowlen, is_masked_section)
    sections = [(0, UMPAD, False), (UMPAD, MKPAD, True),
                (OWNP, HUPAD, False), (OWNP + HUPAD, HMPAD, True)]

    # ---- generic flat scheduler -----------------------------------------
    def build_flat(percore_bwrlv, NBLK, GBX, NWX):
        """percore_bwrlv: per core (blk, win, rel, lane, val) arrays.
        Returns sched dict + per-core (S, idx) builders' inputs."""
        NG = -(-NBLK // GBX)
        cnt = np.zeros((C, NG, NWX), np.int64)
        for ci in range(C):
            b, w = percore_bwrlv[ci][0], percore_bwrlv[ci][1]
            np.add.at(cnt, (ci, b // GBX, w), 1)
        KT = -(-cnt.max(axis=0) // 128)          # [NG, NWX]
        ktoff = np.zeros((NG, NWX), np.int64)
        acc = 0
        for g in range(NG):
            for w in range(NWX):
                ktoff[g, w] = acc
                acc += KT[g, w]
        TOTKT = acc
        # per-core slot/op computation
        per_core = []
        opset = {}
        for ci in range(C):
            b, w, rel, lane, val = percore_bwrlv[ci]
            g = b // GBX
            bucket = g * NWX + w
            order = np.lexsort((np.arange(len(b)), b, bucket))
            bs, ws, gs = b[order], w[order], g[order]
            rels, lanes, vals = rel[order], lane[order], val[order]
            buck = gs * NWX + ws
            segchange = np.r_[True, buck[1:] != buck[:-1]]
            segstart = np.maximum.accumulate(
                np.where(segchange, np.arange(len(buck)), 0))
            pos = np.arange(len(buck)) - segstart
            kt = ktoff[gs, ws] + pos // 128
            sit = pos % 128
            per_core.append((kt, sit, bs, rels, lanes, vals))
            for key in set(zip(kt.tolist(), bs.tolist())):
                opset[key] = True
        ops = sorted(opset.keys())               # (kt, blk) in emission order
        opidx = {key: o for o, key in enumerate(ops)}
        NOP = len(ops)
        # group structure for emission
        groups = []
        for g in range(NG):
            gops = [(w, int(ktoff[g, w]), int(KT[g, w]))
                    for w in range(NWX) if KT[g, w] > 0]
            kt_lo = int(ktoff[g].min()) if gops else 0
            kt_hi = kt_lo + sum(nk for _, _, nk in gops)
            mops = [(kt, blk, opidx[(kt, blk)]) for (kt, blk) in ops
                    if kt_lo <= kt < kt_hi] if gops else []
            # start/stop per block within this group
            first, last = {}, {}
            for i, (kt, blk, o) in enumerate(mops):
                if blk not in first:
                    first[blk] = i
                last[blk] = i
            flags = [(kt, blk, o, first[blk] == i, last[blk] == i)
                     for i, (kt, blk, o) in enumerate(mops)]
            groups.append(dict(gops=gops, mops=flags, kt_lo=kt_lo,
                               nk=kt_hi - kt_lo,
                               blocks=list(range(g * GBX,
                                                 min((g + 1) * GBX, NBLK)))))
        return dict(KT=KT, ktoff=ktoff, TOTKT=TOTKT, NOP=NOP, groups=groups,
                    per_core=per_core, opidx=opidx, cnt=cnt)

    def fill_slots(sched, ci, sdtype):
        kt, sit, bs, rels, lanes, vals = sched["per_core"][ci]
        nslots = sched["TOTKT"] * 128
        idx_flat = np.zeros(nslots, np.int64)
        idx_flat[kt * 128 + sit] = rels
        assert rels.max(initial=0) < WIN
        S = np.zeros((128, sched["NOP"] * 128), np.float32)
        o = np.array([sched["opidx"][(int(k), int(b))]
                      for k, b in zip(kt, bs)], np.int64)
        np.add.at(S, (sit, o * 128 + lanes), vals)
        return (S.astype(sdtype),
                _rep16(idx_flat.astype(np.int16), nslots))

    # ---- L1 schedules ----------------------------------------------------
    sched1 = []
    for d in range(2):
        percore = []
        for ci in range(C):
            adst, asrc = core_edges[ci][d]
            dpos = pos_own[ci][adst]
            row = zrow[ci][asrc]
            assert (row >= 0).all()
            percore.append((dpos // 128, row // WIN, row % WIN, dpos % 128,
                            dinv[d][asrc].astype(np.float32)))
        sched1.append(build_flat(percore, NB, GB1, NW1))

    # ---- L2 mask schedules (halo only; self via direct slice) -----------
    mk_rank = []          # [C] array [OWN] -> rank in masked list or -1
    for ci in range(C):
        lo = ci * OWN
        r = np.full(OWN, -1, np.int64)
        r[mk_nodes[ci] - lo] = np.arange(MK[ci])
        mk_rank.append(r)

    sched2 = []
    for d in range(2):
        percore = []
        for ci in range(C):
            lo = ci * OWN
            ad_g, as_g = (dst, src) if d == 0 else (src, dst)
            sel = ((ad_g >= lo) & (ad_g < lo + OWN)
                   & is_masked[np.clip(ad_g, 0, N - 1)])
            adst = ad_g[sel] - lo
            md = mk_rank[ci][adst]
            sj = as_g[sel] // OWN        # owner core of source
            srow = sj * OWNP + pos_own_of(sj, as_g[sel] - sj * OWN, pos_own)
            percore.append((md // 128, srow // WIN, srow % WIN, md % 128,
                            dinv[d][lo + adst].astype(np.float32)))
        sched2.append(build_flat(percore, MB, GB2, NW2))

    # ---- per-core inputs -------------------------------------------------
    w1all = (np.concatenate([W["on_td_W1"], W["tgt_td_W1"],
                             W["on_bu_W1"], W["tgt_bu_W1"]], axis=1)
             * W1SCALE).astype(f8)
    w2_td = np.concatenate([W["on_td_W2"], W["tgt_td_W2"]], axis=1).astype(bf16)
    w2_bu = np.concatenate([W["on_bu_W2"], W["tgt_bu_W2"]], axis=1).astype(bf16)
    ton = np.concatenate([token @ W["on_td_W1"], token @ W["on_bu_W1"]])
    tonbc = _bcast(ton).astype(bf16)
    b1bc_td = _bcast(np.concatenate([W["on_td_b1"], W["tgt_td_b1"]]))
    b1bc_bu = _bcast(np.concatenate([W["on_bu_b1"], W["tgt_bu_b1"]]))
    b2bc_td = _bcast(np.concatenate([W["on_td_b2"], W["tgt_td_b2"]]))
    b2bc_bu = _bcast(np.concatenate([W["on_bu_b2"], W["tgt_bu_b2"]]))
    b2col = np.stack(
        [W["on_td_b2"], W["tgt_td_b2"], W["on_bu_b2"], W["tgt_bu_b2"]],
        axis=1).astype(np.float32)                         # [64, 4]
    ones = np.ones((128, 1), np.float32)
    gcount = np.bincount(batch, minlength=G).astype(np.float32)
    cntbc = np.broadcast_to(gcount[None, :128], (128, 128)).copy()

    in_maps = []
    for ci in range(C):
        lo = ci * OWN
        # xT in z-row order
        xT = np.zeros((512, RT), f8)
        xT[:, 0:UM[ci]] = x[um_nodes[ci]].T
        xT[:, UMPAD:UMPAD + MK[ci]] = x[mk_nodes[ci]].T
        xT[:, OWNP:OWNP + HU[ci]] = x[halo_um[ci]].T
        xT[:, OWNP + HUPAD:OWNP + HUPAD + HM[ci]] = x[halo_mk[ci]].T

        def dstarr(dv):
            a = np.ones(OWNP, np.float32)
            a[0:UM[ci]] = dv[um_nodes[ci]]
            a[UMPAD:UMPAD + MK[ci]] = dv[mk_nodes[ci]]
            return np.ascontiguousarray(a.reshape(-1, 128).T)

        def colarr(vals_mk, fill=0.0):
            a = np.full(MKPAD, fill, np.float32)
            a[0:MK[ci]] = vals_mk
            return np.ascontiguousarray(a.reshape(-1, 128).T)  # [128, MB]

        m = dict(xT=xT,
                 ddst_td=dstarr(dinv[0]), ddst_bu=dstarr(dinv[1]),
                 swv_td=colarr(dinv[0][mk_nodes[ci]]),
                 swv_bu=colarr(dinv[1][mk_nodes[ci]]),
                 mcvw=colarr(mcnt_global[mk_nodes[ci]]))
        for d, nm in ((0, "td"), (1, "bu")):
            S, idx = fill_slots(sched1[d], ci, f8)
            m[f"s_{nm}1"], m[f"i_{nm}1"] = S, idx
            S2, idx2 = fill_slots(sched2[d], ci, bf16)
            m[f"s2_{nm}"], m[f"i2_{nm}"] = S2, idx2
            # pool S: out-edges of own nodes + self, grouped by graph
            ad, as_ = (dst, src) if d == 0 else (src, dst)
            dv = dinv[d]
            sel = (as_ >= lo) & (as_ < lo + OWN)
            j = pos_own[ci][as_[sel] - lo]
            gg = batch[ad[sel]]
            v = dv[ad[sel]]
            pp = np.zeros((128, NB * 128), np.float32)
            np.add.at(pp, (j % 128, (j // 128) * 128 + gg), v)
            jj = pos_own[ci]
            np.add.at(pp, (jj % 128, (jj // 128) * 128 + batch[lo:lo + OWN]),
                      dv[lo:lo + OWN])
            m[f"pools_{nm}"] = pp.astype(bf16)
        m.update(w1all=w1all, w2_td=w2_td, w2_bu=w2_bu, tonbc=tonbc,
                 b1bc_td=b1bc_td, b1bc_bu=b1bc_bu,
                 b2bc_td=b2bc_td, b2bc_bu=b2bc_bu, b2col=b2col,
                 ones=ones, cntbc=cntbc)
        in_maps.append(m)

    meta = dict(RT=RT, NW1=NW1, NW2=NW2, NB=NB, MB=MB, OWNP=OWNP,
                UMPAD=UMPAD, MKPAD=MKPAD, NPAD=NPAD,
                sections=sections, sched1=sched1, sched2=sched2)
    return meta, in_maps


def pos_own_of(owner_cores, local_idx, pos_own):
    """vectorized pos_own lookup across owner cores"""
    out = np.empty(len(local_idx), np.int64)
    for j in np.unique(owner_cores):
        sel = owner_cores == j
        out[sel] = pos_own[j][local_idx[sel]]
    return out


# ---------------------------------------------------------------- program

def build_program(meta):
    import concourse.bass as bass
    import concourse.bacc as bacc
    import concourse.mybir as mybir
    import concourse.tile as tile
    from concourse.masks import make_identity

    RT, NB, MB = meta["RT"], meta["NB"], meta["MB"]
    NW1, NW2 = meta["NW1"], meta["NW2"]
    OWNP, UMPAD, NPAD = meta["OWNP"], meta["UMPAD"], meta["NPAD"]
    f32, bf, i16 = mybir.dt.float32, mybir.dt.bfloat16, mybir.dt.int16
    f8 = mybir.dt.float8e4
    MUL, ADD = mybir.AluOpType.mult, mybir.AluOpType.add

    nc = bacc.Bacc("TRN2", target_bir_lowering=False, debug=False,
                   num_devices=C)

    def din(name, shape, dt):
        return nc.dram_tensor(name, shape, dt, kind="ExternalInput")

    xT = din("xT", [512, RT], f8)
    ddst = [din("ddst_td", [128, NB], f32), din("ddst_bu", [128, NB], f32)]
    s1 = [din("s_td1", [128, meta["sched1"][0]["NOP"] * 128], f8),
          din("s_bu1", [128, meta["sched1"][1]["NOP"] * 128], f8)]
    i1 = [din("i_td1", [128, meta["sched1"][0]["TOTKT"] * 8], i16),
          din("i_bu1", [128, meta["sched1"][1]["TOTKT"] * 8], i16)]
    s2 = [din("s2_td", [128, meta["sched2"][0]["NOP"] * 128], bf),
          din("s2_bu", [128, meta["sched2"][1]["NOP"] * 128], bf)]
    i2 = [din("i2_td", [128, meta["sched2"][0]["TOTKT"] * 8], i16),
          din("i2_bu", [128, meta["sched2"][1]["TOTKT"] * 8], i16)]
    pools_t = [din("pools_td", [128, NB * 128], bf),
               din("pools_bu", [128, NB * 128], bf)]
    swv_t = [din("swv_td", [128, MB], f32), din("swv_bu", [128, MB], f32)]
    mcvw_t = din("mcvw", [128, MB], f32)
    w1all = din("w1all", [512, 512], f8)
    w2 = [din("w2_td", [128, 128], bf), din("w2_bu", [128, 128], bf)]
    tonbc = din("tonbc", [128, 256], bf)
    b1bc = [din("b1bc_td", [128, 256], f32), din("b1bc_bu", [128, 256], f32)]
    b2bc = [din("b2bc_td", [128, 128], f32), din("b2bc_bu", [128, 128], f32)]
    b2col_t = din("b2col", [64, 4], f32)
    ones_t = din("ones", [128, 1], f32)
    cntbc_t = din("cntbc", [128, 128], f32)
    loss_t = nc.dram_tensor("loss", [1, 1], f32, kind="ExternalOutput")

    z_ws = [nc.dram_tensor(f"zarr{w}", [min(WIN, RT - w * WIN), 512], f8,
                           kind="Internal")
            for w in range(NW1)]

    with tile.TileContext(nc) as tc:
        with (
            tc.tile_pool(name="const", bufs=1) as cpool,
            tc.tile_pool(name="dram", bufs=1, space="DRAM") as dpool,
        ):
            z2own = [dpool.tile([OWNP, 128], bf, tag=f"z2own{d}",
                                name=f"z2own{d}") for d in range(2)]
            z2full = [dpool.tile([NPAD, 128], bf, addr_space="Shared",
                                 tag=f"z2full{d}", name=f"z2full{d}")
                      for d in range(2)]
            ar_in = dpool.tile([128, 520], f32, tag="arin", name="arin")
            ar_out = dpool.tile([128, 520], f32, addr_space="Shared",
                                tag="arout", name="arout")

            ident = cpool.tile([128, 128], bf)
            make_identity(nc, ident[:])
            w1sb = cpool.tile([128, 4 * 512], f8)
            for k in range(4):
                nc.sync.dma_start(out=w1sb[:, k * 512:(k + 1) * 512],
                                  in_=w1all[k * 128:(k + 1) * 128, :])
            w2sb = [cpool.tile([128, 128], bf, tag=f"w2_{d}", name=f"w2_{d}")
                    for d in range(2)]
            tonsb = cpool.tile([128, 256], bf)
            b1sb = [cpool.tile([128, 256], f32, tag=f"b1_{d}", name=f"b1_{d}")
                    for d in range(2)]
            b2sb = [cpool.tile([128, 128], f32, tag=f"b2_{d}", name=f"b2_{d}")
                    for d in range(2)]
            ddsb = [cpool.tile([128, NB], f32, tag=f"dd_{d}", name=f"dd_{d}")
                    for d in range(2)]
            swsb = [cpool.tile([128, MB], f32, tag=f"sw_{d}", name=f"sw_{d}")
                    for d in range(2)]
            mcsb = cpool.tile([128, MB], f32)
            onesb = cpool.tile([128, 1], f32)
            nc.sync.dma_start(out=tonsb[:], in_=tonbc[:, :])
            nc.sync.dma_start(out=onesb[:], in_=ones_t[:, :])
            nc.sync.dma_start(out=mcsb[:], in_=mcvw_t[:, :])
            for d in range(2):
                nc.sync.dma_start(out=w2sb[d][:], in_=w2[d][:, :])
                nc.sync.dma_start(out=b1sb[d][:], in_=b1bc[d][:, :])
                nc.sync.dma_start(out=b2sb[d][:], in_=b2bc[d][:, :])
                nc.sync.dma_start(out=ddsb[d][:], in_=ddst[d][:, :])
                nc.sync.dma_start(out=swsb[d][:], in_=swv_t[d][:, :])

            # ================= P1: z = scaled([x1|x] @ W1-fused) ==========
            with (
                tc.tile_pool(name="xk", bufs=3) as xkp,
                tc.tile_pool(name="zsb", bufs=4) as zsp,
                tc.tile_pool(name="pz", bufs=3, space="PSUM") as pzp,
            ):
                DR = mybir.MatmulPerfMode.DoubleRow
                jpar = 0
                for (r0, rlen, msk) in meta["sections"]:
                    for off in range(0, rlen, NF):
                        nf = min(NF, rlen - off)
                        xk = xkp.tile([128, 4 * NF], f8, tag="xk", name="xk")
                        for k in range(4):
                            nc.sync.dma_start(
                                out=xk[:, k * NF:k * NF + nf],
                                in_=xT[k * 128:(k + 1) * 128,
                                       r0 + off:r0 + off + nf])
                        xk3 = xk[:].rearrange("p (k n) -> p k n", k=4, n=NF)
                        w13 = w1sb[:].rearrange("p (k n) -> p k n", k=4,
                                                n=512)
                        for j in range(nf // 128):
                            row = r0 + off + j * 128
                            jpar += 1
                            zs = zsp.tile([128, 512], f8, tag="zs", name="zs")
                            if not msk:
                                ps = pzp.tile([128, 512], f32, tag="pz",
                                              name="pz")
                                for k in range(0, 4, 2):
                                    nc.tensor.matmul(
                                        out=ps[:],
                                        lhsT=xk3[:, k:k + 2,
                                                 j * 128:(j + 1) * 128],
                                        rhs=w13[:, k:k + 2, :],
                                        start=(k == 0), stop=(k == 2),
                                        perf_mode=DR)
                                if jpar % 3 == 0:
                                    nc.scalar.activation(
                                        out=zs[:], in_=ps[:],
                                        func=mybir.ActivationFunctionType.Copy,
                                        scale=1.0 / W1SCALE)
                                else:
                                    nc.vector.tensor_scalar(
                                        out=zs[:], in0=ps[:],
                                        scalar1=1.0 / W1SCALE,
                                        scalar2=None, op0=MUL)
                            else:
                                ps = pzp.tile([128, 512], f32, tag="pz",
                                              name="pz")
                                for h in range(2):
                                    c0 = h * 256 + 128
                                    for k in range(0, 4, 2):
                                        nc.tensor.matmul(
                                            out=ps[:, h * 128:(h + 1) * 128],
                                            lhsT=xk3[:, k:k + 2,
                                                     j * 128:(j + 1) * 128],
                                            rhs=w13[:, k:k + 2, c0:c0 + 128],
                                            start=(k == 0), stop=(k == 2),
                                            perf_mode=DR)
                                for h in range(2):
                                    nc.vector.tensor_copy(
                                        out=zs[:, h * 256:h * 256 + 128],
                                        in_=tonsb[:, h * 128:(h + 1) * 128])
                                    if jpar % 3 == 0:
                                        nc.scalar.activation(
                                            out=zs[:, h * 256 + 128:
                                                   (h + 1) * 256],
                                            in_=ps[:, h * 128:(h + 1) * 128],
                                            func=mybir.ActivationFunctionType.Copy,
                                            scale=1.0 / W1SCALE)
                                    else:
                                        nc.vector.tensor_scalar(
                                            out=zs[:, h * 256 + 128:
                                                   (h + 1) * 256],
                                            in0=ps[:, h * 128:(h + 1) * 128],
                                            scalar1=1.0 / W1SCALE,
                                            scalar2=None, op0=MUL)
                            zw = row // WIN
                            zr = row - zw * WIN
                            weng = nc.scalar if jpar % 3 == 1 else nc.sync
                            weng.dma_start(out=z_ws[zw][zr:zr + 128, :],
                                           in_=zs[:])

            # ===== L1 agg + finalize (z2 + fused pool), per direction =====
            poolpool_cm = tc.tile_pool(name="plps", bufs=1, space="PSUM")
            poolpool = poolpool_cm.__enter__()
            poolps = poolpool.tile([128, 512], f32, tag="pl", name="pl")

            def l1_dir(d):
                sch = meta["sched1"][d]
                wlen = lambda w: min(WIN, RT - w * WIN)
                with (
                    tc.tile_pool(name=f"g1{d}", bufs=4) as gp,
                    tc.tile_pool(name=f"sI1{d}", bufs=3) as sp,
                    tc.tile_pool(name=f"ix1{d}", bufs=1) as ip,
                    tc.tile_pool(name=f"ps1{d}", bufs=2) as pwp,
                    tc.tile_pool(name=f"fin1{d}", bufs=3) as fp,
                    tc.tile_pool(name=f"h1q{d}", bufs=20) as h1p,
                    tc.tile_pool(name=f"zrb{d}", bufs=3) as zrp,
                    tc.tile_pool(name=f"agg{d}", bufs=1, space="PSUM") as ap,
                    tc.tile_pool(name=f"tr{d}", bufs=2, space="PSUM") as trp,
                    tc.tile_pool(name=f"z2p{d}", bufs=1, space="PSUM") as z2p,
                ):
                    # stage B (transpose -> @W2 -> scale -> z2own write +
                    # fused pool matmuls), decoupled from the agg pipeline
                    # via the deep h1 tile pool and one-group emission skew.
                    def stage_b(blk, bi, h1, pst):
                        trt = trp.tile([128, 256], bf, tag="t", name="t")
                        nc.tensor.transpose(
                            out=trt[:, 0:128], in_=h1[:, 0:128],
                            identity=ident[:])
                        nc.tensor.transpose(
                            out=trt[:, 128:256], in_=h1[:, 128:256],
                            identity=ident[:])
                        h1T = fp.tile([128, 256], bf, tag="h1T", name="h1T")
                        nc.scalar.copy(out=h1T[:], in_=trt[:])
                        z2ps = z2p.tile([128, 128], f32, tag="z2", name="z2")
                        nc.tensor.matmul(out=z2ps[:, 0:64],
                                         lhsT=h1T[:, 0:128],
                                         rhs=w2sb[d][:, 0:64],
                                         start=True, stop=True)
                        nc.tensor.matmul(out=z2ps[:, 64:128],
                                         lhsT=h1T[:, 128:256],
                                         rhs=w2sb[d][:, 64:128],
                                         start=True, stop=True)
                        z2sb = fp.tile([128, 128], bf, tag="z2sb",
                                       name="z2sb")
                        nc.vector.tensor_scalar(
                            out=z2sb[:], in0=z2ps[:],
                            scalar1=ddsb[d][:, blk:blk + 1],
                            scalar2=None, op0=MUL)
                        nc.sync.dma_start(
                            out=z2own[d][blk * 128:(blk + 1) * 128, :],
                            in_=z2sb[:])
                        nc.tensor.matmul(
                            out=poolps[0:64, d * 256:d * 256 + 128],
                            lhsT=z2sb[:, 0:64],
                            rhs=pst[:, bi * 128:(bi + 1) * 128],
                            start=(blk == 0), stop=(blk == NB - 1),
                            skip_group_check=True)
                        nc.tensor.matmul(
                            out=poolps[0:64, d * 256 + 128:d * 256 + 256],
                            lhsT=z2sb[:, 64:128],
                            rhs=pst[:, bi * 128:(bi + 1) * 128],
                            start=(blk == 0), stop=(blk == NB - 1),
                            skip_group_check=True)

                    itall = ip.tile([128, max(sch["TOTKT"], 1) * 8], i16,
                                    tag="ia", name="ia")
                    nc.gpsimd.dma_start(out=itall[:], in_=i1[d][:, :])
                    pending = []
                    if True:
                        for g, grp in enumerate(sch["groups"]):
                            blocks = grp["blocks"]
                            nops = len(grp["mops"])
                            gt = None
                            if grp["gops"]:
                                gt = gp.tile([128, grp["nk"] * 256], f8,
                                             tag="g", name="g")
                                for (ww, ktb, nkw) in grp["gops"]:
                                    o = ktb - grp["kt_lo"]
                                    nc.gpsimd.dma_gather(
                                        gt[:, o * 256:(o + nkw) * 256]
                                        .rearrange("p (k e) -> p k e",
                                                   k=nkw, e=256),
                                        z_ws[ww][0:wlen(ww),
                                                 256 * d:256 * d + 256],
                                        itall[:, ktb * 8:(ktb + nkw) * 8],
                                        nkw * 128, nkw * 128, 256,
                                        elem_step=512, single_packet=False)
                            if nops:
                                st = sp.tile([128, nops * 128], f8, tag="s",
                                             name="s")
                                nc.scalar.dma_start(
                                    out=st[:],
                                    in_=s1[d][:, grp["mops"][0][2] * 128:
                                              (grp["mops"][0][2] + nops)
                                              * 128])
                                aps = ap.tile([128, len(blocks) * 256], f32,
                                              tag="a", name="a")
                                o0 = grp["mops"][0][2]
                                for (kt, blk, o, st_f, sp_f) in grp["mops"]:
                                    bi = blk - blocks[0]
                                    nc.tensor.matmul(
                                        out=aps[:, bi * 256:(bi + 1) * 256],
                                        lhsT=st[:, (o - o0) * 128:
                                                (o - o0 + 1) * 128],
                                        rhs=gt[:, (kt - grp["kt_lo"]) * 256:
                                               (kt - grp["kt_lo"] + 1)
                                               * 256],
                                        start=st_f, stop=sp_f,
                                        skip_group_check=True)
                            has = {blk for (_, blk, _, _, _) in grp["mops"]}
                            # pool S slab for this group
                            pst = pwp.tile([128, len(blocks) * 128], bf,
                                           tag="ps", name="ps")
                            nc.sync.dma_start(
                                out=pst[:],
                                in_=pools_t[d][:, blocks[0] * 128:
                                               (blocks[0] + len(blocks))
                                               * 128])
                            newly = []
                            for blk in blocks:
                                bi = blk - blocks[0]
                                zrb = zrp.tile([128, 256], f8, tag="zr",
                                               name="zr")
                                nc.sync.dma_start(
                                    out=zrb[:],
                                    in_=z_ws[0][blk * 128:(blk + 1) * 128,
                                                256 * d:256 * d + 256])
                                hs = fp.tile([128, 256], f32, tag="hs",
                                             name="hs")
                                if blk in has:
                                    # hs = h_self*dinv_dst + agg
                                    nc.vector.scalar_tensor_tensor(
                                        out=hs[:], in0=zrb[:],
                                        scalar=ddsb[d][:, blk:blk + 1],
                                        in1=aps[:, bi * 256:(bi + 1) * 256],
                                        op0=MUL, op1=ADD)
                                else:
                                    nc.vector.tensor_scalar(
                                        out=hs[:], in0=zrb[:],
                                        scalar1=ddsb[d][:, blk:blk + 1],
                                        scalar2=None, op0=MUL)
                                # h1 = relu(hs*ddst + b1)
                                nc.vector.scalar_tensor_tensor(
                                    out=hs[:], in0=hs[:],
                                    scalar=ddsb[d][:, blk:blk + 1],
                                    in1=b1sb[d][:, 0:256], op0=MUL, op1=ADD)
                                h1 = h1p.tile([128, 256], bf, tag="h1",
                                              name="h1")
                                nc.scalar.activation(
                                    out=h1[:], in_=hs[:],
                                    func=mybir.ActivationFunctionType.Relu)
                                newly.append((blk, bi, h1, pst))
                            for item in pending:
                                stage_b(*item)
                            pending = newly
                    for item in pending:
                        stage_b(*item)

            def allgather(d):
                nc.gpsimd.collective_compute(
                    "AllGather", mybir.AluOpType.bypass,
                    replica_groups=[list(range(C))],
                    ins=[z2own[d].opt()], outs=[z2full[d].opt()])

            l1_dir(0)
            allgather(0)
            l1_dir(1)
            allgather(1)

            # drain pooled sums
            arsb = cpool.tile([128, 520], f32)
            nc.vector.memset(arsb[:], 0.0)
            for d in range(2):
                nc.vector.tensor_copy(out=arsb[0:64, d * 256:(d + 1) * 256],
                                      in_=poolps[0:64, d * 256:(d + 1) * 256])
            poolpool_cm.__exit__(None, None, None)

            # ========== L2 mask aggregation (node-major) ==================
            # wide per-dir product tiles
            prodw = [[cpool.tile([128, MB], f32, tag=f"pw{d}{q}",
                                 name=f"pw{d}{q}") for q in range(3)]
                     for d in range(2)]
            for d in range(2):
                for q in range(3):
                    nc.vector.memset(prodw[d][q][:], 0.0)

            def l2_dir(d):
                sch = meta["sched2"][d]
                wlen = lambda w: min(WIN, NPAD - w * WIN)
                with (
                    tc.tile_pool(name=f"g2{d}", bufs=3) as gp,
                    tc.tile_pool(name=f"sI2{d}", bufs=3) as sp,
                    tc.tile_pool(name=f"ix2{d}", bufs=3) as ip,
                    tc.tile_pool(name=f"fin2{d}", bufs=3) as fp,
                    tc.tile_pool(name=f"zsl{d}", bufs=3) as zp,
                    tc.tile_pool(name=f"mag{d}", bufs=2, space="PSUM") as ap,
                ):
                    itall2 = ip.tile([128, max(sch["TOTKT"], 1) * 8],
                                     i16, tag="ia2", name="ia2")
                    nc.gpsimd.dma_start(out=itall2[:], in_=i2[d][:, :])
                    for g, grp in enumerate(sch["groups"]):
                        blocks = grp["blocks"]
                        nops = len(grp["mops"])
                        nk = grp["nk"]
                        if nops:
                            st = sp.tile([128, nops * 128], bf, tag="s",
                                         name="s")
                            nc.scalar.dma_start(
                                out=st[:],
                                in_=s2[d][:, grp["mops"][0][2] * 128:
                                          (grp["mops"][0][2] + nops) * 128])
                            gt = gp.tile([128, nk * 128], bf, tag="g",
                                         name="g")
                            for w, ktb, nkw in grp["gops"]:
                                o = ktb - grp["kt_lo"]
                                nc.gpsimd.dma_gather(
                                    gt[:, o * 128:(o + nkw) * 128].rearrange(
                                        "p (k e) -> p k e", k=nkw, e=128),
                                    z2full[d][w * WIN:w * WIN + wlen(w), :],
                                    itall2[:, ktb * 8:(ktb + nkw) * 8],
                                    nkw * 128, nkw * 128, 128,
                                    elem_step=None, single_packet=False)
                            aps = ap.tile([128, len(blocks) * 128], f32,
                                          tag="a", name="a")
                            o0 = grp["mops"][0][2]
                            for (kt, blk, o, st_f, sp_f) in grp["mops"]:
                                bi = blk - blocks[0]
                                nc.tensor.matmul(
                                    out=aps[:, bi * 128:(bi + 1) * 128],
                                    lhsT=st[:, (o - o0) * 128:
                                            (o - o0 + 1) * 128],
                                    rhs=gt[:, (kt - grp["kt_lo"]) * 128:
                                           (kt - grp["kt_lo"] + 1) * 128],
                                    start=st_f, stop=sp_f,
                                    skip_group_check=True)
                        has = {blk for (_, blk, _, _, _) in grp["mops"]}
                        for blk in blocks:
                            bi = blk - blocks[0]
                            zsl = zp.tile([128, 128], bf, tag="zs", name="zs")
                            nc.scalar.dma_start(
                                out=zsl[:],
                                in_=z2own[d][UMPAD + blk * 128:
                                             UMPAD + (blk + 1) * 128, :])
                            hs = fp.tile([128, 128], f32, tag="hs", name="hs")
                            # hs = z_self*swv (+ agg)
                            if blk in has:
                                nc.vector.scalar_tensor_tensor(
                                    out=hs[:], in0=zsl[:],
                                    scalar=swsb[d][:, blk:blk + 1],
                                    in1=aps[:, bi * 128:(bi + 1) * 128],
                                    op0=MUL, op1=ADD)
                            else:
                                nc.vector.tensor_scalar(
                                    out=hs[:], in0=zsl[:],
                                    scalar1=swsb[d][:, blk:blk + 1],
                                    scalar2=None, op0=MUL)
                            nc.vector.tensor_tensor(
                                out=hs[:], in0=hs[:], in1=b2sb[d][:, 0:128],
                                op=ADD)
                            # products (accumulate over 64-feat free dim)
                            scr = fp.tile([128, 64], f32, tag="sc", name="sc")
                            for q, (p0, p1) in enumerate(
                                    ((0, 64), (0, 0), (64, 64))):
                                nc.vector.scalar_tensor_tensor(
                                    out=scr[:], in0=hs[:, p0:p0 + 64],
                                    scalar=1.0, in1=hs[:, p1:p1 + 64],
                                    op0=MUL, op1=MUL,
                                    accum_out=prodw[d][q][:, blk:blk + 1])

            l2_dir(0)
            l2_dir(1)

            # ========== masked SCE epilogue (wide) ========================
            with tc.tile_pool(name="ep", bufs=1) as ep:
                su = [ep.tile([128, MB], f32, tag=f"su{q}", name=f"su{q}")
                      for q in range(3)]
                for q in range(3):
                    nc.vector.tensor_tensor(out=su[q][:], in0=prodw[0][q][:],
                                            in1=prodw[1][q][:], op=ADD)

                def rsq(n, tag):
                    r = ep.tile([128, MB], f32, tag=tag, name=tag)
                    nc.scalar.sqrt(out=r[:], in_=n[:])
                    nc.vector.tensor_scalar_max(out=r[:], in0=r[:],
                                                scalar1=1e-12)
                    nc.vector.reciprocal(out=r[:], in_=r[:])
                    return r

                r1 = rsq(su[1], "r1")
                r2 = rsq(su[2], "r2")
                tt = ep.tile([128, MB], f32, tag="tt", name="tt")
                nc.vector.tensor_tensor(out=tt[:], in0=su[0][:], in1=r1[:],
                                        op=MUL)
                nc.vector.tensor_tensor(out=tt[:], in0=tt[:], in1=r2[:],
                                        op=MUL)
                nc.vector.tensor_tensor(out=tt[:], in0=tt[:], in1=mcsb[:],
                                        op=MUL)
                scr = ep.tile([128, MB], f32, tag="scr", name="scr")
                colsum = ep.tile([128, 1], f32, tag="cs", name="cs")
                nc.vector.scalar_tensor_tensor(
                    out=scr[:], in0=tt[:], scalar=-1.0, in1=mcsb[:],
                    op0=MUL, op1=ADD, accum_out=colsum[:])
                with tc.tile_pool(name="eps", bufs=1, space="PSUM") as epp:
                    macc_ps = epp.tile([1, 1], f32, tag="mp", name="mp")
                    nc.tensor.matmul(out=macc_ps[:], lhsT=colsum[:],
                                     rhs=onesb[:], start=True, stop=True)
                    nc.vector.tensor_copy(out=arsb[0:1, 512:513],
                                          in_=macc_ps[:])

            # ========== AllReduce (pools + mask partial) =================
            nc.sync.dma_start(out=ar_in[:, :], in_=arsb[:])
            nc.gpsimd.collective_compute(
                "AllReduce", mybir.AluOpType.add,
                replica_groups=[list(range(C))],
                ins=[ar_in.opt()], outs=[ar_out.opt()])

            # ========== pooled cosine + final loss =======================
            with (
                tc.tile_pool(name="fin3", bufs=2) as f2,
                tc.tile_pool(name="fps", bufs=2, space="PSUM") as fpp,
            ):
                ar2 = f2.tile([128, 520], f32, tag="ar2", name="ar2")
                nc.sync.dma_start(out=ar2[:], in_=ar_out[:, :])
                cntsb = f2.tile([128, 128], f32, tag="cnt", name="cnt")
                nc.sync.dma_start(out=cntsb[:], in_=cntbc_t[:, :])
                b2t = f2.tile([64, 4], f32, tag="b2tf", name="b2tf")
                nc.sync.dma_start(out=b2t[:], in_=b2col_t[:, :])
                pf = {}
                for d in range(2):
                    for h in range(2):
                        po = f2.tile([64, 128], f32, tag=f"po{d}{h}",
                                     name=f"po{d}{h}")
                        nc.vector.scalar_tensor_tensor(
                            out=po[:], in0=cntsb[0:64, :],
                            scalar=b2t[0:64, 2 * d + h:2 * d + h + 1],
                            in1=ar2[0:64, d * 256 + h * 128:
                                    d * 256 + (h + 1) * 128],
                            op0=MUL, op1=ADD)
                        pf[(d, h)] = po
                gsums = []
                for qi, pick in enumerate(((0, 1), (0, 0), (1, 1))):
                    qp = fpp.tile([1, 128], f32, tag="gqp", name="gqp")
                    for d in range(2):
                        pr = f2.tile([64, 128], f32, tag=f"gpr{d}",
                                     name=f"gpr{d}")
                        nc.vector.tensor_tensor(
                            out=pr[:], in0=pf[(d, pick[0])][:],
                            in1=pf[(d, pick[1])][:], op=MUL)
                        nc.tensor.matmul(
                            out=qp[:], lhsT=onesb[0:64, 0:1], rhs=pr[:],
                            start=(d == 0), stop=(d == 1),
                            skip_group_check=True)
                    sq = f2.tile([1, 128], f32, tag=f"gsq{qi}",
                                 name=f"gsq{qi}")
                    nc.vector.tensor_copy(out=sq[:], in_=qp[:])
                    gsums.append(sq)
                gdot, gn1, gn2 = gsums

                def rguard2(n, tag):
                    r = f2.tile([1, 128], f32, tag=tag, name=tag)
                    nc.scalar.sqrt(out=r[:], in_=n[:])
                    nc.vector.tensor_scalar_max(out=r[:], in0=r[:],
                                                scalar1=1e-12)
                    nc.vector.reciprocal(out=r[:], in_=r[:])
                    return r

                g1 = rguard2(gn1, "g1")
                g2 = rguard2(gn2, "g2")
                cosg = f2.tile([1, 128], f32, tag="cosg", name="cosg")
                nc.vector.tensor_tensor(out=cosg[:], in0=gdot[:], in1=g1[:],
                                        op=MUL)
                nc.vector.tensor_tensor(out=cosg[:], in0=cosg[:], in1=g2[:],
                                        op=MUL)
                onesrow = f2.tile([1, 128], f32, tag="onesr", name="onesr")
                nc.vector.memset(onesrow[:], 1.0)
                gterm = f2.tile([1, 128], f32, tag="gterm", name="gterm")
                gs = f2.tile([1, 1], f32, tag="gs", name="gs")
                nc.vector.scalar_tensor_tensor(
                    out=gterm[:], in0=cosg[:], scalar=-1.0, in1=onesrow[:],
                    op0=MUL, op1=ADD, accum_out=gs[:])
                l1t = f2.tile([1, 1], f32, tag="l1", name="l1")
                nc.scalar.activation(out=l1t[:], in_=gs[:],
                                     func=mybir.ActivationFunctionType.Copy,
                                     scale=1.0 / G)
                l2t = f2.tile([1, 1], f32, tag="l2", name="l2")
                nc.scalar.activation(out=l2t[:], in_=ar2[0:1, 512:513],
                                     func=mybir.ActivationFunctionType.Copy,
                                     scale=1.0 / M)
                nc.vector.tensor_tensor(out=l1t[:], in0=l1t[:], in1=l2t[:],
                                        op=ADD)
                nc.sync.dma_start(out=loss_t[:, :], in_=l1t[:])

    return nc


# ---------------------------------------------------------------- entry

LAST_RESULT = None


def _install_trace_hook():
    """The agent image's antenv lacks axon_hooks; synthesize it from
    trn_boot's ctypes NTFF hook so trace=True works under axon."""
    import types
    try:
        from antenv import axon_hooks  # noqa: F401
        return
    except ImportError:
        pass
    try:
        import antenv
        import trn_agent_boot.trn_boot as tb
        hook = tb._ntff_profile_via_ctypes("/opt/axon/libaxon_pjrt.so")
        mod = types.ModuleType("antenv.axon_hooks")
        mod.get_axon_ntff_profile_hook = lambda: hook
        mod.set_axon_ntff_profile_hook = lambda h: None
        sys.modules["antenv.axon_hooks"] = mod
        antenv.axon_hooks = mod
    except Exception as e:
        print(f"[kernel] trace hook install failed: {e}", file=sys.stderr)


def kernel(_trace=False, **inputs):
    global LAST_RESULT
    import time
    from concourse import bass_utils
    if _trace:
        _install_trace_hook()
    t0 = time.monotonic()
    meta, in_maps = host_prep(inputs)
    t1 = time.monotonic()
    nc = build_program(meta)
    t2 = time.monotonic()
    nc.compile()
    t3 = time.monotonic()
    res = bass_utils.run_bass_kernel_spmd(
        nc, in_maps, core_ids=list(range(C)),
        trace=_trace, trace_cores=[0] if _trace else None)
    t4 = time.monotonic()
    print(f"[kernel] prep {t1-t0:.1f}s build {t2-t1:.1f}s "
          f"compile {t3-t2:.1f}s run {t4-t3:.1f}s", file=sys.stderr)
    LAST_RESULT = res
    return np.float32(res.results[0]["loss"][0, 0])


# revision 40
# speedup vs baseline: 1.2470x; 1.0486x over previous
"""Trainium2 Bass kernel for the rumor-GCN masked-autoencoder loss.

Strategy (8 NeuronCores, SPMD single NEFF):
  - Nodes partitioned into 8 contiguous ranges (25000 each), then per-core
    RE-ORDERED: unmasked own nodes first [0, UM), masked own compact at
    [UMPAD, UMPAD+MK).  All host-side index maps are relabeled, so the
    permutation is free at runtime and makes (a) mask-aggregation self terms
    a contiguous z2own slice and (b) L1 self-loop terms a contiguous z
    readback -- neither needs dma_gather (~8ns/idx on GpSimd, the dominant
    cost; see /root/problem/microbench.py).
  - z = [x1|x] @ W1 for all 4 GCN heads in one fused [512->512] bf16 matmul
    over the per-core needed set (own + halo, pre-gathered by host).  Row
    scales dinv[src] folded at the copy-out, dinv[dst] at finalize.
  - L1 edge aggregation: flat slot schedule bucketed by (group-of-8-dst-
    blocks, z-window).  Slots sorted by dst block inside each bucket, padded
    only at bucket tails; one dma_gather per bucket; one matmul per
    (K-tile x dst-block-segment) with host-built one-hot S.  Tiles may span
    dst blocks (extra matmul, no extra gather).  Self-loop term z[own]
    added at finalize via direct DMA readback.  global_add_pool is fused
    into the finalize: pool[g] += z2sb^T @ poolS (src-side rewrite).
  - L2 is only needed at masked nodes.  Mask aggregation is node-major
    ([128 masked nodes, 128 feat(on|tgt)] PSUM per block): halo edges
    gathered from the AllGathered z2full with the same flat scheduling;
    self term + b2 bias added at finalize from the contiguous z2own slice.
    Cosine terms reduce along the free dim via accum_out into per-block
    columns; one short wide chain finishes the masked SCE.
  - Each direction's z2 AllGather is issued as soon as that direction's L1
    finishes, overlapping the other direction's aggregation; pooled sums +
    the mask partial go through one small AllReduce.
"""

import sys

import numpy as np

sys.path.insert(0, "/opt/trn_rl_repo")

# ---------------------------------------------------------------- config

WIN = 32768
GB1 = 8       # L1 dst blocks per PSUM group
GB2 = 8       # L2 mask blocks per PSUM group
NF = 2048     # P1 column chunk

N, E, G, M, C = 200000, 400000, 128, 100000, 8
OWN = N // C

_WNAMES = [p + s for p in ("on_td", "on_bu", "tgt_td", "tgt_bu")
           for s in ("_W1", "_b1", "_W2", "_b2")]


def _rep16(idx_flat, nslots):
    """int16 index list -> [128, nslots//16] layout (16-part wrap, 8x rep)."""
    blk = np.zeros((16, nslots // 16), dtype=np.int16)
    k = np.arange(len(idx_flat))
    blk[k % 16, k // 16] = idx_flat
    return np.tile(blk, (8, 1))


def _bcast(vec, parts=128):
    return np.broadcast_to(np.asarray(vec)[None, :], (parts, len(vec))).copy()


def _pad128(n):
    return -(-n // 128) * 128


# ---------------------------------------------------------------- host prep

W1SCALE = 16.0  # lift fp8 W1 out of the subnormal range; undone in dloc


def host_prep(inp):
    import ml_dtypes
    bf16 = ml_dtypes.bfloat16
    f8 = ml_dtypes.float8_e4m3
    x = np.asarray(inp["x"], np.float32)
    token = np.asarray(inp["enc_mask_token"], np.float32).reshape(-1)
    ei = np.asarray(inp["edge_index"])
    src, dst = ei[0].astype(np.int64), ei[1].astype(np.int64)
    batch = np.asarray(inp["batch"]).astype(np.int64)
    mask_nodes = np.asarray(inp["mask_nodes"]).astype(np.int64)
    W = {k: np.asarray(inp[k], np.float32) for k in _WNAMES}

    dinv = [
        (1.0 / np.sqrt(np.bincount(dst, minlength=N) + 1.0)).astype(np.float32),
        (1.0 / np.sqrt(np.bincount(src, minlength=N) + 1.0)).astype(np.float32),
    ]
    is_masked = np.zeros(N, bool)
    is_masked[mask_nodes] = True
    mcnt_global = np.bincount(mask_nodes, minlength=N).astype(np.float32)
    xbf = x.astype(bf16)

    # ---- per-core own-node permutation: unmasked first, masked at tail
    um_nodes, mk_nodes = [], []
    for ci in range(C):
        lo = ci * OWN
        m = is_masked[lo:lo + OWN]
        um_nodes.append(np.where(~m)[0] + lo)
        mk_nodes.append(np.where(m)[0] + lo)
    UM = [len(a) for a in um_nodes]
    MK = [len(a) for a in mk_nodes]
    UMPAD = _pad128(max(UM))
    MKPAD = _pad128(max(MK))
    OWNP = UMPAD + MKPAD
    NB = OWNP // 128
    MB = MKPAD // 128
    NPAD = C * OWNP
    NW2 = -(-NPAD // WIN)

    pos_own = []          # [C] array [OWN] -> p-order position
    for ci in range(C):
        lo = ci * OWN
        p = np.empty(OWN, np.int64)
        p[um_nodes[ci] - lo] = np.arange(UM[ci])
        p[mk_nodes[ci] - lo] = UMPAD + np.arange(MK[ci])
        pos_own.append(p)

    # ---- per-core edge lists (dir 0 = TD: dst-agg; dir 1 = BU: src-agg)
    core_edges = []       # [core][dir] -> (adst_local, asrc_global)
    for ci in range(C):
        lo, hi = ci * OWN, (ci + 1) * OWN
        per = []
        for d in range(2):
            ad, as_ = (dst, src) if d == 0 else (src, dst)
            sel = (ad >= lo) & (ad < hi)
            per.append((ad[sel] - lo, as_[sel]))
        core_edges.append(per)

    # ---- halo sets (union over both dirs), split unmasked/masked
    halo_um, halo_mk = [], []
    for ci in range(C):
        lo, hi = ci * OWN, (ci + 1) * OWN
        srcs = np.unique(np.concatenate(
            [core_edges[ci][0][1], core_edges[ci][1][1]]))
        srcs = srcs[(srcs < lo) | (srcs >= hi)]
        halo_um.append(srcs[~is_masked[srcs]])
        halo_mk.append(srcs[is_masked[srcs]])
    HU = [len(a) for a in halo_um]
    HM = [len(a) for a in halo_mk]
    HUPAD = _pad128(max(HU))
    HMPAD = _pad128(max(HM))
    RT = OWNP + HUPAD + HMPAD
    NW1 = -(-RT // WIN)

    # z-row map per core: global node -> z row (own p-order | halo)
    zrow = []
    for ci in range(C):
        lo = ci * OWN
        zm = np.full(N, -1, np.int64)
        zm[lo + np.arange(OWN)] = pos_own[ci]
        zm[halo_um[ci]] = OWNP + np.arange(HU[ci])
        zm[halo_mk[ci]] = OWNP + HUPAD + np.arange(HM[ci])
        zrow.append(zm)

    # P1 sections: (row0, rowlen, is_masked_section)
    sections = [(0, UMPAD, False), (UMPAD, MKPAD, True),
                (OWNP, HUPAD, False), (OWNP + HUPAD, HMPAD, True)]

    # ---- generic flat scheduler -----------------------------------------
    def build_flat(percore_bwrlv, NBLK, GBX, NWX):
        """percore_bwrlv: per core (blk, win, rel, lane, val) arrays.
        Returns sched dict + per-core (S, idx) builders' inputs."""
        NG = -(-NBLK // GBX)
        cnt = np.zeros((C, NG, NWX), np.int64)
        for ci in range(C):
            b, w = percore_bwrlv[ci][0], percore_bwrlv[ci][1]
            np.add.at(cnt, (ci, b // GBX, w), 1)
        KT = -(-cnt.max(axis=0) // 128)          # [NG, NWX]
        ktoff = np.zeros((NG, NWX), np.int64)
        acc = 0
        for g in range(NG):
            for w in range(NWX):
                ktoff[g, w] = acc
                acc += KT[g, w]
        TOTKT = acc
        # per-core slot/op computation
        per_core = []
        opset = {}
        for ci in range(C):
            b, w, rel, lane, val = percore_bwrlv[ci]
            g = b // GBX
            bucket = g * NWX + w
            order = np.lexsort((np.arange(len(b)), b, bucket))
            bs, ws, gs = b[order], w[order], g[order]
            rels, lanes, vals = rel[order], lane[order], val[order]
            buck = gs * NWX + ws
            segchange = np.r_[True, buck[1:] != buck[:-1]]
            segstart = np.maximum.accumulate(
                np.where(segchange, np.arange(len(buck)), 0))
            pos = np.arange(len(buck)) - segstart
            kt = ktoff[gs, ws] + pos // 128
            sit = pos % 128
            per_core.append((kt, sit, bs, rels, lanes, vals))
            for key in set(zip(kt.tolist(), bs.tolist())):
                opset[key] = True
        ops = sorted(opset.keys())               # (kt, blk) in emission order
        opidx = {key: o for o, key in enumerate(ops)}
        NOP = len(ops)
        # group structure for emission
        groups = []
        for g in range(NG):
            gops = [(w, int(ktoff[g, w]), int(KT[g, w]))
                    for w in range(NWX) if KT[g, w] > 0]
            kt_lo = int(ktoff[g].min()) if gops else 0
            kt_hi = kt_lo + sum(nk for _, _, nk in gops)
            mops = [(kt, blk, opidx[(kt, blk)]) for (kt, blk) in ops
                    if kt_lo <= kt < kt_hi] if gops else []
            # start/stop per block within this group
            first, last = {}, {}
            for i, (kt, blk, o) in enumerate(mops):
                if blk not in first:
                    first[blk] = i
                last[blk] = i
            flags = [(kt, blk, o, first[blk] == i, last[blk] == i)
                     for i, (kt, blk, o) in enumerate(mops)]
            groups.append(dict(gops=gops, mops=flags, kt_lo=kt_lo,
                               nk=kt_hi - kt_lo,
                               blocks=list(range(g * GBX,
                                                 min((g + 1) * GBX, NBLK)))))
        return dict(KT=KT, ktoff=ktoff, TOTKT=TOTKT, NOP=NOP, groups=groups,
                    per_core=per_core, opidx=opidx, cnt=cnt)

    def fill_slots(sched, ci, sdtype):
        kt, sit, bs, rels, lanes, vals = sched["per_core"][ci]
        nslots = sched["TOTKT"] * 128
        idx_flat = np.zeros(nslots, np.int64)
        idx_flat[kt * 128 + sit] = rels
        assert rels.max(initial=0) < WIN
        S = np.zeros((128, sched["NOP"] * 128), np.float32)
        o = np.array([sched["opidx"][(int(k), int(b))]
                      for k, b in zip(kt, bs)], np.int64)
        np.add.at(S, (sit, o * 128 + lanes), vals)
        return (S.astype(sdtype),
                _rep16(idx_flat.astype(np.int16), nslots))

    # ---- L1 schedules ----------------------------------------------------
    sched1 = []
    for d in range(2):
        percore = []
        for ci in range(C):
            adst, asrc = core_edges[ci][d]
            dpos = pos_own[ci][adst]
            row = zrow[ci][asrc]
            assert (row >= 0).all()
            percore.append((dpos // 128, row // WIN, row % WIN, dpos % 128,
                            dinv[d][asrc].astype(np.float32)))
        sched1.append(build_flat(percore, NB, GB1, NW1))

    # ---- L2 mask schedules (halo only; self via direct slice) -----------
    mk_rank = []          # [C] array [OWN] -> rank in masked list or -1
    for ci in range(C):
        lo = ci * OWN
        r = np.full(OWN, -1, np.int64)
        r[mk_nodes[ci] - lo] = np.arange(MK[ci])
        mk_rank.append(r)

    sched2 = []
    for d in range(2):
        percore = []
        for ci in range(C):
            lo = ci * OWN
            ad_g, as_g = (dst, src) if d == 0 else (src, dst)
            sel = ((ad_g >= lo) & (ad_g < lo + OWN)
                   & is_masked[np.clip(ad_g, 0, N - 1)])
            adst = ad_g[sel] - lo
            md = mk_rank[ci][adst]
            sj = as_g[sel] // OWN        # owner core of source
            srow = sj * OWNP + pos_own_of(sj, as_g[sel] - sj * OWN, pos_own)
            percore.append((md // 128, srow // WIN, srow % WIN, md % 128,
                            dinv[d][lo + adst].astype(np.float32)))
        sched2.append(build_flat(percore, MB, GB2, NW2))

    # ---- per-core inputs -------------------------------------------------
    w1all = (np.concatenate([W["on_td_W1"], W["tgt_td_W1"],
                             W["on_bu_W1"], W["tgt_bu_W1"]], axis=1)
             * W1SCALE).astype(f8)
    w2_td = np.concatenate([W["on_td_W2"], W["tgt_td_W2"]], axis=1).astype(bf16)
    w2_bu = np.concatenate([W["on_bu_W2"], W["tgt_bu_W2"]], axis=1).astype(bf16)
    ton = np.concatenate([token @ W["on_td_W1"], token @ W["on_bu_W1"]])
    tonbc = _bcast(ton).astype(bf16)
    b1bc_td = _bcast(np.concatenate([W["on_td_b1"], W["tgt_td_b1"]]))
    b1bc_bu = _bcast(np.concatenate([W["on_bu_b1"], W["tgt_bu_b1"]]))
    b2bc_td = _bcast(np.concatenate([W["on_td_b2"], W["tgt_td_b2"]]))
    b2bc_bu = _bcast(np.concatenate([W["on_bu_b2"], W["tgt_bu_b2"]]))
    b2col = np.stack(
        [W["on_td_b2"], W["tgt_td_b2"], W["on_bu_b2"], W["tgt_bu_b2"]],
        axis=1).astype(np.float32)                         # [64, 4]
    ones = np.ones((128, 1), np.float32)
    gcount = np.bincount(batch, minlength=G).astype(np.float32)
    cntbc = np.broadcast_to(gcount[None, :128], (128, 128)).copy()

    in_maps = []
    for ci in range(C):
        lo = ci * OWN
        # xT in z-row order
        xT = np.zeros((512, RT), f8)
        xT[:, 0:UM[ci]] = x[um_nodes[ci]].T
        xT[:, UMPAD:UMPAD + MK[ci]] = x[mk_nodes[ci]].T
        xT[:, OWNP:OWNP + HU[ci]] = x[halo_um[ci]].T
        xT[:, OWNP + HUPAD:OWNP + HUPAD + HM[ci]] = x[halo_mk[ci]].T

        def dstarr(dv):
            a = np.ones(OWNP, np.float32)
            a[0:UM[ci]] = dv[um_nodes[ci]]
            a[UMPAD:UMPAD + MK[ci]] = dv[mk_nodes[ci]]
            return np.ascontiguousarray(a.reshape(-1, 128).T)

        def colarr(vals_mk, fill=0.0):
            a = np.full(MKPAD, fill, np.float32)
            a[0:MK[ci]] = vals_mk
            return np.ascontiguousarray(a.reshape(-1, 128).T)  # [128, MB]

        m = dict(xT=xT,
                 ddst_td=dstarr(dinv[0]), ddst_bu=dstarr(dinv[1]),
                 swv_td=colarr(dinv[0][mk_nodes[ci]]),
                 swv_bu=colarr(dinv[1][mk_nodes[ci]]),
                 mcvw=colarr(mcnt_global[mk_nodes[ci]]))
        for d, nm in ((0, "td"), (1, "bu")):
            S, idx = fill_slots(sched1[d], ci, f8)
            m[f"s_{nm}1"], m[f"i_{nm}1"] = S, idx
            S2, idx2 = fill_slots(sched2[d], ci, bf16)
            m[f"s2_{nm}"], m[f"i2_{nm}"] = S2, idx2
            # pool S: out-edges of own nodes + self, grouped by graph
            ad, as_ = (dst, src) if d == 0 else (src, dst)
            dv = dinv[d]
            sel = (as_ >= lo) & (as_ < lo + OWN)
            j = pos_own[ci][as_[sel] - lo]
            gg = batch[ad[sel]]
            v = dv[ad[sel]]
            pp = np.zeros((128, NB * 128), np.float32)
            np.add.at(pp, (j % 128, (j // 128) * 128 + gg), v)
            jj = pos_own[ci]
            np.add.at(pp, (jj % 128, (jj // 128) * 128 + batch[lo:lo + OWN]),
                      dv[lo:lo + OWN])
            m[f"pools_{nm}"] = pp.astype(bf16)
        m.update(w1all=w1all, w2_td=w2_td, w2_bu=w2_bu, tonbc=tonbc,
                 b1bc_td=b1bc_td, b1bc_bu=b1bc_bu,
                 b2bc_td=b2bc_td, b2bc_bu=b2bc_bu, b2col=b2col,
                 ones=ones, cntbc=cntbc)
        in_maps.append(m)

    meta = dict(RT=RT, NW1=NW1, NW2=NW2, NB=NB, MB=MB, OWNP=OWNP,
                UMPAD=UMPAD, MKPAD=MKPAD, NPAD=NPAD,
                sections=sections, sched1=sched1, sched2=sched2)
    return meta, in_maps


def pos_own_of(owner_cores, local_idx, pos_own):
    """vectorized pos_own lookup across owner cores"""
    out = np.empty(len(local_idx), np.int64)
    for j in np.unique(owner_cores):
        sel = owner_cores == j
        out[sel] = pos_own[j][local_idx[sel]]
    return out


# ---------------------------------------------------------------- program

def build_program(meta):
    import concourse.bass as bass
    import concourse.bacc as bacc
    import concourse.mybir as mybir
    import concourse.tile as tile
    from concourse.masks import make_identity

    RT, NB, MB = meta["RT"], meta["NB"], meta["MB"]
    NW1, NW2 = meta["NW1"], meta["NW2"]
    OWNP, UMPAD, NPAD = meta["OWNP"], meta["UMPAD"], meta["NPAD"]
    f32, bf, i16 = mybir.dt.float32, mybir.dt.bfloat16, mybir.dt.int16
    f8 = mybir.dt.float8e4
    MUL, ADD = mybir.AluOpType.mult, mybir.AluOpType.add

    nc = bacc.Bacc("TRN2", target_bir_lowering=False, debug=False,
                   num_devices=C)

    def din(name, shape, dt):
        return nc.dram_tensor(name, shape, dt, kind="ExternalInput")

    xT = din("xT", [512, RT], f8)
    ddst = [din("ddst_td", [128, NB], f32), din("ddst_bu", [128, NB], f32)]
    s1 = [din("s_td1", [128, meta["sched1"][0]["NOP"] * 128], f8),
          din("s_bu1", [128, meta["sched1"][1]["NOP"] * 128], f8)]
    i1 = [din("i_td1", [128, meta["sched1"][0]["TOTKT"] * 8], i16),
          din("i_bu1", [128, meta["sched1"][1]["TOTKT"] * 8], i16)]
    s2 = [din("s2_td", [128, meta["sched2"][0]["NOP"] * 128], bf),
          din("s2_bu", [128, meta["sched2"][1]["NOP"] * 128], bf)]
    i2 = [din("i2_td", [128, meta["sched2"][0]["TOTKT"] * 8], i16),
          din("i2_bu", [128, meta["sched2"][1]["TOTKT"] * 8], i16)]
    pools_t = [din("pools_td", [128, NB * 128], bf),
               din("pools_bu", [128, NB * 128], bf)]
    swv_t = [din("swv_td", [128, MB], f32), din("swv_bu", [128, MB], f32)]
    mcvw_t = din("mcvw", [128, MB], f32)
    w1all = din("w1all", [512, 512], f8)
    w2 = [din("w2_td", [128, 128], bf), din("w2_bu", [128, 128], bf)]
    tonbc = din("tonbc", [128, 256], bf)
    b1bc = [din("b1bc_td", [128, 256], f32), din("b1bc_bu", [128, 256], f32)]
    b2bc = [din("b2bc_td", [128, 128], f32), din("b2bc_bu", [128, 128], f32)]
    b2col_t = din("b2col", [64, 4], f32)
    ones_t = din("ones", [128, 1], f32)
    cntbc_t = din("cntbc", [128, 128], f32)
    loss_t = nc.dram_tensor("loss", [1, 1], f32, kind="ExternalOutput")

    z_ws = [nc.dram_tensor(f"zarr{w}", [min(WIN, RT - w * WIN), 512], f8,
                           kind="Internal")
            for w in range(NW1)]

    with tile.TileContext(nc) as tc:
        with (
            tc.tile_pool(name="const", bufs=1) as cpool,
            tc.tile_pool(name="dram", bufs=1, space="DRAM") as dpool,
        ):
            z2own = [dpool.tile([OWNP, 128], bf, tag=f"z2own{d}",
                                name=f"z2own{d}") for d in range(2)]
            z2full = [dpool.tile([NPAD, 128], bf, addr_space="Shared",
                                 tag=f"z2full{d}", name=f"z2full{d}")
                      for d in range(2)]
            ar_in = dpool.tile([128, 520], f32, tag="arin", name="arin")
            ar_out = dpool.tile([128, 520], f32, addr_space="Shared",
                                tag="arout", name="arout")

            ident = cpool.tile([128, 128], bf)
            make_identity(nc, ident[:])
            w1sb = cpool.tile([128, 4 * 512], f8)
            for k in range(4):
                nc.sync.dma_start(out=w1sb[:, k * 512:(k + 1) * 512],
                                  in_=w1all[k * 128:(k + 1) * 128, :])
            w2sb = [cpool.tile([128, 128], bf, tag=f"w2_{d}", name=f"w2_{d}")
                    for d in range(2)]
            tonsb = cpool.tile([128, 256], bf)
            b1sb = [cpool.tile([128, 256], f32, tag=f"b1_{d}", name=f"b1_{d}")
                    for d in range(2)]
            b2sb = [cpool.tile([128, 128], f32, tag=f"b2_{d}", name=f"b2_{d}")
                    for d in range(2)]
            ddsb = [cpool.tile([128, NB], f32, tag=f"dd_{d}", name=f"dd_{d}")
                    for d in range(2)]
            swsb = [cpool.tile([128, MB], f32, tag=f"sw_{d}", name=f"sw_{d}")
                    for d in range(2)]
            mcsb = cpool.tile([128, MB], f32)
            onesb = cpool.tile([128, 1], f32)
            nc.sync.dma_start(out=tonsb[:], in_=tonbc[:, :])
            nc.sync.dma_start(out=onesb[:], in_=ones_t[:, :])
            nc.sync.dma_start(out=mcsb[:], in_=mcvw_t[:, :])
            for d in range(2):
                nc.sync.dma_start(out=w2sb[d][:], in_=w2[d][:, :])
                nc.sync.dma_start(out=b1sb[d][:], in_=b1bc[d][:, :])
                nc.sync.dma_start(out=b2sb[d][:], in_=b2bc[d][:, :])
                nc.sync.dma_start(out=ddsb[d][:], in_=ddst[d][:, :])
                nc.sync.dma_start(out=swsb[d][:], in_=swv_t[d][:, :])

            # ================= P1: z = scaled([x1|x] @ W1-fused) ==========
            with (
                tc.tile_pool(name="xk", bufs=4) as xkp,
                tc.tile_pool(name="zsb", bufs=6) as zsp,
                tc.tile_pool(name="pz", bufs=4, space="PSUM") as pzp,
            ):
                DR = mybir.MatmulPerfMode.DoubleRow
                jpar = 0
                for (r0, rlen, msk) in meta["sections"]:
                    for off in range(0, rlen, NF):
                        nf = min(NF, rlen - off)
                        xk = xkp.tile([128, 4 * NF], f8, tag="xk", name="xk")
                        for k in range(4):
                            nc.sync.dma_start(
                                out=xk[:, k * NF:k * NF + nf],
                                in_=xT[k * 128:(k + 1) * 128,
                                       r0 + off:r0 + off + nf])
                        xk3 = xk[:].rearrange("p (k n) -> p k n", k=4, n=NF)
                        w13 = w1sb[:].rearrange("p (k n) -> p k n", k=4,
                                                n=512)
                        for j in range(nf // 128):
                            row = r0 + off + j * 128
                            jpar += 1
                            zs = zsp.tile([128, 512], f8, tag="zs", name="zs")
                            if not msk:
                                ps = pzp.tile([128, 512], f32, tag="pz",
                                              name="pz")
                                for k in range(0, 4, 2):
                                    nc.tensor.matmul(
                                        out=ps[:],
                                        lhsT=xk3[:, k:k + 2,
                                                 j * 128:(j + 1) * 128],
                                        rhs=w13[:, k:k + 2, :],
                                        start=(k == 0), stop=(k == 2),
                                        perf_mode=DR)
                                if jpar % 3 == 0:
                                    nc.scalar.activation(
                                        out=zs[:], in_=ps[:],
                                        func=mybir.ActivationFunctionType.Copy,
                                        scale=1.0 / W1SCALE)
                                else:
                                    nc.vector.tensor_scalar(
                                        out=zs[:], in0=ps[:],
                                        scalar1=1.0 / W1SCALE,
                                        scalar2=None, op0=MUL)
                            else:
                                ps = pzp.tile([128, 512], f32, tag="pz",
                                              name="pz")
                                for h in range(2):
                                    c0 = h * 256 + 128
                                    for k in range(0, 4, 2):
                                        nc.tensor.matmul(
                                            out=ps[:, h * 128:(h + 1) * 128],
                                            lhsT=xk3[:, k:k + 2,
                                                     j * 128:(j + 1) * 128],
                                            rhs=w13[:, k:k + 2, c0:c0 + 128],
                                            start=(k == 0), stop=(k == 2),
                                            perf_mode=DR)
                                for h in range(2):
                                    nc.vector.tensor_copy(
                                        out=zs[:, h * 256:h * 256 + 128],
                                        in_=tonsb[:, h * 128:(h + 1) * 128])
                                    if jpar % 3 == 0:
                                        nc.scalar.activation(
                                            out=zs[:, h * 256 + 128:
                                                   (h + 1) * 256],
                                            in_=ps[:, h * 128:(h + 1) * 128],
                                            func=mybir.ActivationFunctionType.Copy,
                                            scale=1.0 / W1SCALE)
                                    else:
                                        nc.vector.tensor_scalar(
                                            out=zs[:, h * 256 + 128:
                                                   (h + 1) * 256],
                                            in0=ps[:, h * 128:(h + 1) * 128],
                                            scalar1=1.0 / W1SCALE,
                                            scalar2=None, op0=MUL)
                            zw = row // WIN
                            zr = row - zw * WIN
                            weng = nc.scalar if jpar % 3 == 1 else nc.sync
                            weng.dma_start(out=z_ws[zw][zr:zr + 128, :],
                                           in_=zs[:])

            # ===== L1 agg + finalize (z2 + fused pool), per direction =====
            poolpool_cm = tc.tile_pool(name="plps", bufs=1, space="PSUM")
            poolpool = poolpool_cm.__enter__()
            poolps = poolpool.tile([128, 512], f32, tag="pl", name="pl")

            def l1_dir(d):
                sch = meta["sched1"][d]
                wlen = lambda w: min(WIN, RT - w * WIN)
                with (
                    tc.tile_pool(name=f"g1{d}", bufs=4) as gp,
                    tc.tile_pool(name=f"sI1{d}", bufs=3) as sp,
                    tc.tile_pool(name=f"ix1{d}", bufs=1) as ip,
                    tc.tile_pool(name=f"ps1{d}", bufs=2) as pwp,
                    tc.tile_pool(name=f"fin1{d}", bufs=3) as fp,
                    tc.tile_pool(name=f"h1q{d}", bufs=20) as h1p,
                    tc.tile_pool(name=f"zrb{d}", bufs=3) as zrp,
                    tc.tile_pool(name=f"agg{d}", bufs=1, space="PSUM") as ap,
                    tc.tile_pool(name=f"tr{d}", bufs=2, space="PSUM") as trp,
                    tc.tile_pool(name=f"z2p{d}", bufs=1, space="PSUM") as z2p,
                ):
                    # stage B (transpose -> @W2 -> scale -> z2own write +
                    # fused pool matmuls), decoupled from the agg pipeline
                    # via the deep h1 tile pool and one-group emission skew.
                    def stage_b(blk, bi, h1, pst):
                        trt = trp.tile([128, 256], bf, tag="t", name="t")
                        nc.tensor.transpose(
                            out=trt[:, 0:128], in_=h1[:, 0:128],
                            identity=ident[:])
                        nc.tensor.transpose(
                            out=trt[:, 128:256], in_=h1[:, 128:256],
                            identity=ident[:])
                        h1T = fp.tile([128, 256], bf, tag="h1T", name="h1T")
                        nc.scalar.copy(out=h1T[:], in_=trt[:])
                        z2ps = z2p.tile([128, 128], f32, tag="z2", name="z2")
                        nc.tensor.matmul(out=z2ps[:, 0:64],
                                         lhsT=h1T[:, 0:128],
                                         rhs=w2sb[d][:, 0:64],
                                         start=True, stop=True)
                        nc.tensor.matmul(out=z2ps[:, 64:128],
                                         lhsT=h1T[:, 128:256],
                                         rhs=w2sb[d][:, 64:128],
                                         start=True, stop=True)
                        z2sb = fp.tile([128, 128], bf, tag="z2sb",
                                       name="z2sb")
                        nc.vector.tensor_scalar(
                            out=z2sb[:], in0=z2ps[:],
                            scalar1=ddsb[d][:, blk:blk + 1],
                            scalar2=None, op0=MUL)
                        nc.sync.dma_start(
                            out=z2own[d][blk * 128:(blk + 1) * 128, :],
                            in_=z2sb[:])
                        nc.tensor.matmul(
                            out=poolps[0:64, d * 256:d * 256 + 128],
                            lhsT=z2sb[:, 0:64],
                            rhs=pst[:, bi * 128:(bi + 1) * 128],
                            start=(blk == 0), stop=(blk == NB - 1),
                            skip_group_check=True)
                        nc.tensor.matmul(
                            out=poolps[0:64, d * 256 + 128:d * 256 + 256],
                            lhsT=z2sb[:, 64:128],
                            rhs=pst[:, bi * 128:(bi + 1) * 128],
                            start=(blk == 0), stop=(blk == NB - 1),
                            skip_group_check=True)

                    itall = ip.tile([128, max(sch["TOTKT"], 1) * 8], i16,
                                    tag="ia", name="ia")
                    nc.gpsimd.dma_start(out=itall[:], in_=i1[d][:, :])
                    pending = []
                    if True:
                        for g, grp in enumerate(sch["groups"]):
                            blocks = grp["blocks"]
                            nops = len(grp["mops"])
                            gt = None
                            if grp["gops"]:
                                gt = gp.tile([128, grp["nk"] * 256], f8,
                                             tag="g", name="g")
                                for (ww, ktb, nkw) in grp["gops"]:
                                    o = ktb - grp["kt_lo"]
                                    nc.gpsimd.dma_gather(
                                        gt[:, o * 256:(o + nkw) * 256]
                                        .rearrange("p (k e) -> p k e",
                                                   k=nkw, e=256),
                                        z_ws[ww][0:wlen(ww),
                                                 256 * d:256 * d + 256],
                                        itall[:, ktb * 8:(ktb + nkw) * 8],
                                        nkw * 128, nkw * 128, 256,
                                        elem_step=512, single_packet=False)
                            if nops:
                                st = sp.tile([128, nops * 128], f8, tag="s",
                                             name="s")
                                nc.scalar.dma_start(
                                    out=st[:],
                                    in_=s1[d][:, grp["mops"][0][2] * 128:
                                              (grp["mops"][0][2] + nops)
                                              * 128])
                                aps = ap.tile([128, len(blocks) * 256], f32,
                                              tag="a", name="a")
                                o0 = grp["mops"][0][2]
                                for (kt, blk, o, st_f, sp_f) in grp["mops"]:
                                    bi = blk - blocks[0]
                                    nc.tensor.matmul(
                                        out=aps[:, bi * 256:(bi + 1) * 256],
                                        lhsT=st[:, (o - o0) * 128:
                                                (o - o0 + 1) * 128],
                                        rhs=gt[:, (kt - grp["kt_lo"]) * 256:
                                               (kt - grp["kt_lo"] + 1)
                                               * 256],
                                        start=st_f, stop=sp_f,
                                        skip_group_check=True)
                            has = {blk for (_, blk, _, _, _) in grp["mops"]}
                            # pool S slab for this group
                            pst = pwp.tile([128, len(blocks) * 128], bf,
                                           tag="ps", name="ps")
                            nc.sync.dma_start(
                                out=pst[:],
                                in_=pools_t[d][:, blocks[0] * 128:
                                               (blocks[0] + len(blocks))
                                               * 128])
                            newly = []
                            for blk in blocks:
                                bi = blk - blocks[0]
                                zrb = zrp.tile([128, 256], f8, tag="zr",
                                               name="zr")
                                nc.sync.dma_start(
                                    out=zrb[:],
                                    in_=z_ws[0][blk * 128:(blk + 1) * 128,
                                                256 * d:256 * d + 256])
                                hs = fp.tile([128, 256], f32, tag="hs",
                                             name="hs")
                                if blk in has:
                                    # hs = h_self*dinv_dst + agg
                                    nc.vector.scalar_tensor_tensor(
                                        out=hs[:], in0=zrb[:],
                                        scalar=ddsb[d][:, blk:blk + 1],
                                        in1=aps[:, bi * 256:(bi + 1) * 256],
                                        op0=MUL, op1=ADD)
                                else:
                                    nc.vector.tensor_scalar(
                                        out=hs[:], in0=zrb[:],
                                        scalar1=ddsb[d][:, blk:blk + 1],
                                        scalar2=None, op0=MUL)
                                # h1 = relu(hs*ddst + b1)
                                nc.vector.scalar_tensor_tensor(
                                    out=hs[:], in0=hs[:],
                                    scalar=ddsb[d][:, blk:blk + 1],
                                    in1=b1sb[d][:, 0:256], op0=MUL, op1=ADD)
                                h1 = h1p.tile([128, 256], bf, tag="h1",
                                              name="h1")
                                nc.scalar.activation(
                                    out=h1[:], in_=hs[:],
                                    func=mybir.ActivationFunctionType.Relu)
                                newly.append((blk, bi, h1, pst))
                            for item in pending:
                                stage_b(*item)
                            pending = newly
                    for item in pending:
                        stage_b(*item)

            def allgather(d):
                nc.gpsimd.collective_compute(
                    "AllGather", mybir.AluOpType.bypass,
                    replica_groups=[list(range(C))],
                    ins=[z2own[d].opt()], outs=[z2full[d].opt()])

            l1_dir(0)
            allgather(0)
            l1_dir(1)
            allgather(1)

            # drain pooled sums
            arsb = cpool.tile([128, 520], f32)
            nc.vector.memset(arsb[:], 0.0)
            for d in range(2):
                nc.vector.tensor_copy(out=arsb[0:64, d * 256:(d + 1) * 256],
                                      in_=poolps[0:64, d * 256:(d + 1) * 256])
            poolpool_cm.__exit__(None, None, None)

            # ========== L2 mask aggregation (node-major) ==================
            # wide per-dir product tiles
            prodw = [[cpool.tile([128, MB], f32, tag=f"pw{d}{q}",
                                 name=f"pw{d}{q}") for q in range(3)]
                     for d in range(2)]
            for d in range(2):
                for q in range(3):
                    nc.vector.memset(prodw[d][q][:], 0.0)

            def l2_dir(d):
                sch = meta["sched2"][d]
                wlen = lambda w: min(WIN, NPAD - w * WIN)
                with (
                    tc.tile_pool(name=f"g2{d}", bufs=3) as gp,
                    tc.tile_pool(name=f"sI2{d}", bufs=3) as sp,
                    tc.tile_pool(name=f"ix2{d}", bufs=3) as ip,
                    tc.tile_pool(name=f"fin2{d}", bufs=3) as fp,
                    tc.tile_pool(name=f"zsl{d}", bufs=3) as zp,
                    tc.tile_pool(name=f"mag{d}", bufs=2, space="PSUM") as ap,
                ):
                    itall2 = ip.tile([128, max(sch["TOTKT"], 1) * 8],
                                     i16, tag="ia2", name="ia2")
                    nc.gpsimd.dma_start(out=itall2[:], in_=i2[d][:, :])
                    for g, grp in enumerate(sch["groups"]):
                        blocks = grp["blocks"]
                        nops = len(grp["mops"])
                        nk = grp["nk"]
                        if nops:
                            st = sp.tile([128, nops * 128], bf, tag="s",
                                         name="s")
                            nc.scalar.dma_start(
                                out=st[:],
                                in_=s2[d][:, grp["mops"][0][2] * 128:
                                          (grp["mops"][0][2] + nops) * 128])
                            gt = gp.tile([128, nk * 128], bf, tag="g",
                                         name="g")
                            for w, ktb, nkw in grp["gops"]:
                                o = ktb - grp["kt_lo"]
                                nc.gpsimd.dma_gather(
                                    gt[:, o * 128:(o + nkw) * 128].rearrange(
                                        "p (k e) -> p k e", k=nkw, e=128),
                                    z2full[d][w * WIN:w * WIN + wlen(w), :],
                                    itall2[:, ktb * 8:(ktb + nkw) * 8],
                                    nkw * 128, nkw * 128, 128,
                                    elem_step=None, single_packet=False)
                            aps = ap.tile([128, len(blocks) * 128], f32,
                                          tag="a", name="a")
                            o0 = grp["mops"][0][2]
                            for (kt, blk, o, st_f, sp_f) in grp["mops"]:
                                bi = blk - blocks[0]
                                nc.tensor.matmul(
                                    out=aps[:, bi * 128:(bi + 1) * 128],
                                    lhsT=st[:, (o - o0) * 128:
                                            (o - o0 + 1) * 128],
                                    rhs=gt[:, (kt - grp["kt_lo"]) * 128:
                                           (kt - grp["kt_lo"] + 1) * 128],
                                    start=st_f, stop=sp_f,
                                    skip_group_check=True)
                        has = {blk for (_, blk, _, _, _) in grp["mops"]}
                        for blk in blocks:
                            bi = blk - blocks[0]
                            zsl = zp.tile([128, 128], bf, tag="zs", name="zs")
                            nc.scalar.dma_start(
                                out=zsl[:],
                                in_=z2own[d][UMPAD + blk * 128:
                                             UMPAD + (blk + 1) * 128, :])
                            hs = fp.tile([128, 128], f32, tag="hs", name="hs")
                            # hs = z_self*swv (+ agg)
                            if blk in has:
                                nc.vector.scalar_tensor_tensor(
                                    out=hs[:], in0=zsl[:],
                                    scalar=swsb[d][:, blk:blk + 1],
                                    in1=aps[:, bi * 128:(bi + 1) * 128],
                                    op0=MUL, op1=ADD)
                            else:
                                nc.vector.tensor_scalar(
                                    out=hs[:], in0=zsl[:],
                                    scalar1=swsb[d][:, blk:blk + 1],
                                    scalar2=None, op0=MUL)
                            nc.vector.tensor_tensor(
                                out=hs[:], in0=hs[:], in1=b2sb[d][:, 0:128],
                                op=ADD)
                            # products (accumulate over 64-feat free dim)
                            scr = fp.tile([128, 64], f32, tag="sc", name="sc")
                            for q, (p0, p1) in enumerate(
                                    ((0, 64), (0, 0), (64, 64))):
                                nc.vector.scalar_tensor_tensor(
                                    out=scr[:], in0=hs[:, p0:p0 + 64],
                                    scalar=1.0, in1=hs[:, p1:p1 + 64],
                                    op0=MUL, op1=MUL,
                                    accum_out=prodw[d][q][:, blk:blk + 1])

            l2_dir(0)
            l2_dir(1)

            # ========== masked SCE epilogue (wide) ========================
            with tc.tile_pool(name="ep", bufs=1) as ep:
                su = [ep.tile([128, MB], f32, tag=f"su{q}", name=f"su{q}")
                      for q in range(3)]
                for q in range(3):
                    nc.vector.tensor_tensor(out=su[q][:], in0=prodw[0][q][:],
                                            in1=prodw[1][q][:], op=ADD)

                def rsq(n, tag):
                    r = ep.tile([128, MB], f32, tag=tag, name=tag)
                    nc.scalar.sqrt(out=r[:], in_=n[:])
                    nc.vector.tensor_scalar_max(out=r[:], in0=r[:],
                                                scalar1=1e-12)
                    nc.vector.reciprocal(out=r[:], in_=r[:])
                    return r

                r1 = rsq(su[1], "r1")
                r2 = rsq(su[2], "r2")
                tt = ep.tile([128, MB], f32, tag="tt", name="tt")
                nc.vector.tensor_tensor(out=tt[:], in0=su[0][:], in1=r1[:],
                                        op=MUL)
                nc.vector.tensor_tensor(out=tt[:], in0=tt[:], in1=r2[:],
                                        op=MUL)
                nc.vector.tensor_tensor(out=tt[:], in0=tt[:], in1=mcsb[:],
                                        op=MUL)
                scr = ep.tile([128, MB], f32, tag="scr", name="scr")
                colsum = ep.tile([128, 1], f32, tag="cs", name="cs")
                nc.vector.scalar_tensor_tensor(
                    out=scr[:], in0=tt[:], scalar=-1.0, in1=mcsb[:],
                    op0=MUL, op1=ADD, accum_out=colsum[:])
                with tc.tile_pool(name="eps", bufs=1, space="PSUM") as epp:
                    macc_ps = epp.tile([1, 1], f32, tag="mp", name="mp")
                    nc.tensor.matmul(out=macc_ps[:], lhsT=colsum[:],
                                     rhs=onesb[:], start=True, stop=True)
                    nc.vector.tensor_copy(out=arsb[0:1, 512:513],
                                          in_=macc_ps[:])

            # ========== AllReduce (pools + mask partial) =================
            nc.sync.dma_start(out=ar_in[:, :], in_=arsb[:])
            nc.gpsimd.collective_compute(
                "AllReduce", mybir.AluOpType.add,
                replica_groups=[list(range(C))],
                ins=[ar_in.opt()], outs=[ar_out.opt()])

            # ========== pooled cosine + final loss =======================
            with (
                tc.tile_pool(name="fin3", bufs=2) as f2,
                tc.tile_pool(name="fps", bufs=2, space="PSUM") as fpp,
            ):
                ar2 = f2.tile([128, 520], f32, tag="ar2", name="ar2")
                nc.sync.dma_start(out=ar2[:], in_=ar_out[:, :])
                cntsb = f2.tile([128, 128], f32, tag="cnt", name="cnt")
                nc.sync.dma_start(out=cntsb[:], in_=cntbc_t[:, :])
                b2t = f2.tile([64, 4], f32, tag="b2tf", name="b2tf")
                nc.sync.dma_start(out=b2t[:], in_=b2col_t[:, :])
                pf = {}
                for d in range(2):
                    for h in range(2):
                        po = f2.tile([64, 128], f32, tag=f"po{d}{h}",
                                     name=f"po{d}{h}")
                        nc.vector.scalar_tensor_tensor(
                            out=po[:], in0=cntsb[0:64, :],
                            scalar=b2t[0:64, 2 * d + h:2 * d + h + 1],
                            in1=ar2[0:64, d * 256 + h * 128:
                                    d * 256 + (h + 1) * 128],
                            op0=MUL, op1=ADD)
                        pf[(d, h)] = po
                gsums = []
                for qi, pick in enumerate(((0, 1), (0, 0), (1, 1))):
                    qp = fpp.tile([1, 128], f32, tag="gqp", name="gqp")
                    for d in range(2):
                        pr = f2.tile([64, 128], f32, tag=f"gpr{d}",
                                     name=f"gpr{d}")
                        nc.vector.tensor_tensor(
                            out=pr[:], in0=pf[(d, pick[0])][:],
                            in1=pf[(d, pick[1])][:], op=MUL)
                        nc.tensor.matmul(
                            out=qp[:], lhsT=onesb[0:64, 0:1], rhs=pr[:],
                            start=(d == 0), stop=(d == 1),
                            skip_group_check=True)
                    sq = f2.tile([1, 128], f32, tag=f"gsq{qi}",
                                 name=f"gsq{qi}")
                    nc.vector.tensor_copy(out=sq[:], in_=qp[:])
                    gsums.append(sq)
                gdot, gn1, gn2 = gsums

                def rguard2(n, tag):
                    r = f2.tile([1, 128], f32, tag=tag, name=tag)
                    nc.scalar.sqrt(out=r[:], in_=n[:])
                    nc.vector.tensor_scalar_max(out=r[:], in0=r[:],
                                                scalar1=1e-12)
                    nc.vector.reciprocal(out=r[:], in_=r[:])
                    return r

                g1 = rguard2(gn1, "g1")
                g2 = rguard2(gn2, "g2")
                cosg = f2.tile([1, 128], f32, tag="cosg", name="cosg")
                nc.vector.tensor_tensor(out=cosg[:], in0=gdot[:], in1=g1[:],
                                        op=MUL)
                nc.vector.tensor_tensor(out=cosg[:], in0=cosg[:], in1=g2[:],
                                        op=MUL)
                onesrow = f2.tile([1, 128], f32, tag="onesr", name="onesr")
                nc.vector.memset(onesrow[:], 1.0)
                gterm = f2.tile([1, 128], f32, tag="gterm", name="gterm")
                gs = f2.tile([1, 1], f32, tag="gs", name="gs")
                nc.vector.scalar_tensor_tensor(
                    out=gterm[:], in0=cosg[:], scalar=-1.0, in1=onesrow[:],
                    op0=MUL, op1=ADD, accum_out=gs[:])
                l1t = f2.tile([1, 1], f32, tag="l1", name="l1")
                nc.scalar.activation(out=l1t[:], in_=gs[:],
                                     func=mybir.ActivationFunctionType.Copy,
                                     scale=1.0 / G)
                l2t = f2.tile([1, 1], f32, tag="l2", name="l2")
                nc.scalar.activation(out=l2t[:], in_=ar2[0:1, 512:513],
                                     func=mybir.ActivationFunctionType.Copy,
                                     scale=1.0 / M)
                nc.vector.tensor_tensor(out=l1t[:], in0=l1t[:], in1=l2t[:],
                                        op=ADD)
                nc.sync.dma_start(out=loss_t[:, :], in_=l1t[:])

    return nc


# ---------------------------------------------------------------- entry

LAST_RESULT = None


def _install_trace_hook():
    """The agent image's antenv lacks axon_hooks; synthesize it from
    trn_boot's ctypes NTFF hook so trace=True works under axon."""
    import types
    try:
        from antenv import axon_hooks  # noqa: F401
        return
    except ImportError:
        pass
    try:
        import antenv
        import trn_agent_boot.trn_boot as tb
        hook = tb._ntff_profile_via_ctypes("/opt/axon/libaxon_pjrt.so")
        mod = types.ModuleType("antenv.axon_hooks")
        mod.get_axon_ntff_profile_hook = lambda: hook
        mod.set_axon_ntff_profile_hook = lambda h: None
        sys.modules["antenv.axon_hooks"] = mod
        antenv.axon_hooks = mod
    except Exception as e:
        print(f"[kernel] trace hook install failed: {e}", file=sys.stderr)


def kernel(_trace=False, **inputs):
    global LAST_RESULT
    import time
    from concourse import bass_utils
    if _trace:
        _install_trace_hook()
    t0 = time.monotonic()
    meta, in_maps = host_prep(inputs)
    t1 = time.monotonic()
    nc = build_program(meta)
    t2 = time.monotonic()
    nc.compile()
    t3 = time.monotonic()
    res = bass_utils.run_bass_kernel_spmd(
        nc, in_maps, core_ids=list(range(C)),
        trace=_trace, trace_cores=[0] if _trace else None)
    t4 = time.monotonic()
    print(f"[kernel] prep {t1-t0:.1f}s build {t2-t1:.1f}s "
          f"compile {t3-t2:.1f}s run {t4-t3:.1f}s", file=sys.stderr)
    LAST_RESULT = res
    return np.float32(res.results[0]["loss"][0, 0])


# revision 41
# speedup vs baseline: 1.2677x; 1.0166x over previous
"""Trainium2 Bass kernel for the rumor-GCN masked-autoencoder loss.

Strategy (8 NeuronCores, SPMD single NEFF):
  - Nodes partitioned into 8 contiguous ranges (25000 each), then per-core
    RE-ORDERED: unmasked own nodes first [0, UM), masked own compact at
    [UMPAD, UMPAD+MK).  All host-side index maps are relabeled, so the
    permutation is free at runtime and makes (a) mask-aggregation self terms
    a contiguous z2own slice and (b) L1 self-loop terms a contiguous z
    readback -- neither needs dma_gather (~8ns/idx on GpSimd, the dominant
    cost; see /root/problem/microbench.py).
  - z = [x1|x] @ W1 for all 4 GCN heads in one fused [512->512] bf16 matmul
    over the per-core needed set (own + halo, pre-gathered by host).  Row
    scales dinv[src] folded at the copy-out, dinv[dst] at finalize.
  - L1 edge aggregation: flat slot schedule bucketed by (group-of-8-dst-
    blocks, z-window).  Slots sorted by dst block inside each bucket, padded
    only at bucket tails; one dma_gather per bucket; one matmul per
    (K-tile x dst-block-segment) with host-built one-hot S.  Tiles may span
    dst blocks (extra matmul, no extra gather).  Self-loop term z[own]
    added at finalize via direct DMA readback.  global_add_pool is fused
    into the finalize: pool[g] += z2sb^T @ poolS (src-side rewrite).
  - L2 is only needed at masked nodes.  Mask aggregation is node-major
    ([128 masked nodes, 128 feat(on|tgt)] PSUM per block): halo edges
    gathered from the AllGathered z2full with the same flat scheduling;
    self term + b2 bias added at finalize from the contiguous z2own slice.
    Cosine terms reduce along the free dim via accum_out into per-block
    columns; one short wide chain finishes the masked SCE.
  - Each direction's z2 AllGather is issued as soon as that direction's L1
    finishes, overlapping the other direction's aggregation; pooled sums +
    the mask partial go through one small AllReduce.
"""

import sys

import numpy as np

sys.path.insert(0, "/opt/trn_rl_repo")

# ---------------------------------------------------------------- config

WIN = 32768
GB1 = 8       # L1 dst blocks per PSUM group
GB2 = 8       # L2 mask blocks per PSUM group
NF = 2048     # P1 column chunk

N, E, G, M, C = 200000, 400000, 128, 100000, 8
OWN = N // C

_WNAMES = [p + s for p in ("on_td", "on_bu", "tgt_td", "tgt_bu")
           for s in ("_W1", "_b1", "_W2", "_b2")]


def _rep16(idx_flat, nslots):
    """int16 index list -> [128, nslots//16] layout (16-part wrap, 8x rep)."""
    blk = np.zeros((16, nslots // 16), dtype=np.int16)
    k = np.arange(len(idx_flat))
    blk[k % 16, k // 16] = idx_flat
    return np.tile(blk, (8, 1))


def _bcast(vec, parts=128):
    return np.broadcast_to(np.asarray(vec)[None, :], (parts, len(vec))).copy()


def _pad128(n):
    return -(-n // 128) * 128


# ---------------------------------------------------------------- host prep

W1SCALE = 16.0  # lift fp8 W1 out of the subnormal range; undone in dloc


def host_prep(inp):
    import ml_dtypes
    bf16 = ml_dtypes.bfloat16
    f8 = ml_dtypes.float8_e4m3
    x = np.asarray(inp["x"], np.float32)
    token = np.asarray(inp["enc_mask_token"], np.float32).reshape(-1)
    ei = np.asarray(inp["edge_index"])
    src, dst = ei[0].astype(np.int64), ei[1].astype(np.int64)
    batch = np.asarray(inp["batch"]).astype(np.int64)
    mask_nodes = np.asarray(inp["mask_nodes"]).astype(np.int64)
    W = {k: np.asarray(inp[k], np.float32) for k in _WNAMES}

    dinv = [
        (1.0 / np.sqrt(np.bincount(dst, minlength=N) + 1.0)).astype(np.float32),
        (1.0 / np.sqrt(np.bincount(src, minlength=N) + 1.0)).astype(np.float32),
    ]
    is_masked = np.zeros(N, bool)
    is_masked[mask_nodes] = True
    mcnt_global = np.bincount(mask_nodes, minlength=N).astype(np.float32)
    xbf = x.astype(bf16)

    # ---- per-core own-node permutation: unmasked first, masked at tail
    um_nodes, mk_nodes = [], []
    for ci in range(C):
        lo = ci * OWN
        m = is_masked[lo:lo + OWN]
        um_nodes.append(np.where(~m)[0] + lo)
        mk_nodes.append(np.where(m)[0] + lo)
    UM = [len(a) for a in um_nodes]
    MK = [len(a) for a in mk_nodes]
    UMPAD = _pad128(max(UM))
    MKPAD = _pad128(max(MK))
    OWNP = UMPAD + MKPAD
    NB = OWNP // 128
    MB = MKPAD // 128
    NPAD = C * OWNP
    NW2 = -(-NPAD // WIN)

    pos_own = []          # [C] array [OWN] -> p-order position
    for ci in range(C):
        lo = ci * OWN
        p = np.empty(OWN, np.int64)
        p[um_nodes[ci] - lo] = np.arange(UM[ci])
        p[mk_nodes[ci] - lo] = UMPAD + np.arange(MK[ci])
        pos_own.append(p)

    # ---- per-core edge lists (dir 0 = TD: dst-agg; dir 1 = BU: src-agg)
    core_edges = []       # [core][dir] -> (adst_local, asrc_global)
    for ci in range(C):
        lo, hi = ci * OWN, (ci + 1) * OWN
        per = []
        for d in range(2):
            ad, as_ = (dst, src) if d == 0 else (src, dst)
            sel = (ad >= lo) & (ad < hi)
            per.append((ad[sel] - lo, as_[sel]))
        core_edges.append(per)

    # ---- halo sets (union over both dirs), split unmasked/masked
    halo_um, halo_mk = [], []
    for ci in range(C):
        lo, hi = ci * OWN, (ci + 1) * OWN
        srcs = np.unique(np.concatenate(
            [core_edges[ci][0][1], core_edges[ci][1][1]]))
        srcs = srcs[(srcs < lo) | (srcs >= hi)]
        halo_um.append(srcs[~is_masked[srcs]])
        halo_mk.append(srcs[is_masked[srcs]])
    HU = [len(a) for a in halo_um]
    HM = [len(a) for a in halo_mk]
    HUPAD = _pad128(max(HU))
    HMPAD = _pad128(max(HM))
    RT = OWNP + HUPAD + HMPAD
    NW1 = -(-RT // WIN)

    # z-row map per core: global node -> z row (own p-order | halo)
    zrow = []
    for ci in range(C):
        lo = ci * OWN
        zm = np.full(N, -1, np.int64)
        zm[lo + np.arange(OWN)] = pos_own[ci]
        zm[halo_um[ci]] = OWNP + np.arange(HU[ci])
        zm[halo_mk[ci]] = OWNP + HUPAD + np.arange(HM[ci])
        zrow.append(zm)

    # P1 sections: (row0, rowlen, is_masked_section)
    sections = [(0, UMPAD, False), (UMPAD, MKPAD, True),
                (OWNP, HUPAD, False), (OWNP + HUPAD, HMPAD, True)]

    # ---- generic flat scheduler -----------------------------------------
    def build_flat(percore_bwrlv, NBLK, GBX, NWX):
        """percore_bwrlv: per core (blk, win, rel, lane, val) arrays.
        Returns sched dict + per-core (S, idx) builders' inputs."""
        NG = -(-NBLK // GBX)
        cnt = np.zeros((C, NG, NWX), np.int64)
        for ci in range(C):
            b, w = percore_bwrlv[ci][0], percore_bwrlv[ci][1]
            np.add.at(cnt, (ci, b // GBX, w), 1)
        KT = -(-cnt.max(axis=0) // 128)          # [NG, NWX]
        ktoff = np.zeros((NG, NWX), np.int64)
        acc = 0
        for g in range(NG):
            for w in range(NWX):
                ktoff[g, w] = acc
                acc += KT[g, w]
        TOTKT = acc
        # per-core slot/op computation
        per_core = []
        opset = {}
        for ci in range(C):
            b, w, rel, lane, val = percore_bwrlv[ci]
            g = b // GBX
            bucket = g * NWX + w
            order = np.lexsort((np.arange(len(b)), b, bucket))
            bs, ws, gs = b[order], w[order], g[order]
            rels, lanes, vals = rel[order], lane[order], val[order]
            buck = gs * NWX + ws
            segchange = np.r_[True, buck[1:] != buck[:-1]]
            segstart = np.maximum.accumulate(
                np.where(segchange, np.arange(len(buck)), 0))
            pos = np.arange(len(buck)) - segstart
            kt = ktoff[gs, ws] + pos // 128
            sit = pos % 128
            per_core.append((kt, sit, bs, rels, lanes, vals))
            for key in set(zip(kt.tolist(), bs.tolist())):
                opset[key] = True
        ops = sorted(opset.keys())               # (kt, blk) in emission order
        opidx = {key: o for o, key in enumerate(ops)}
        NOP = len(ops)
        # group structure for emission
        groups = []
        for g in range(NG):
            gops = [(w, int(ktoff[g, w]), int(KT[g, w]))
                    for w in range(NWX) if KT[g, w] > 0]
            kt_lo = int(ktoff[g].min()) if gops else 0
            kt_hi = kt_lo + sum(nk for _, _, nk in gops)
            mops = [(kt, blk, opidx[(kt, blk)]) for (kt, blk) in ops
                    if kt_lo <= kt < kt_hi] if gops else []
            # start/stop per block within this group
            first, last = {}, {}
            for i, (kt, blk, o) in enumerate(mops):
                if blk not in first:
                    first[blk] = i
                last[blk] = i
            flags = [(kt, blk, o, first[blk] == i, last[blk] == i)
                     for i, (kt, blk, o) in enumerate(mops)]
            groups.append(dict(gops=gops, mops=flags, kt_lo=kt_lo,
                               nk=kt_hi - kt_lo,
                               blocks=list(range(g * GBX,
                                                 min((g + 1) * GBX, NBLK)))))
        return dict(KT=KT, ktoff=ktoff, TOTKT=TOTKT, NOP=NOP, groups=groups,
                    per_core=per_core, opidx=opidx, cnt=cnt)

    def fill_slots(sched, ci, sdtype):
        kt, sit, bs, rels, lanes, vals = sched["per_core"][ci]
        nslots = sched["TOTKT"] * 128
        idx_flat = np.zeros(nslots, np.int64)
        idx_flat[kt * 128 + sit] = rels
        assert rels.max(initial=0) < WIN
        S = np.zeros((128, sched["NOP"] * 128), np.float32)
        o = np.array([sched["opidx"][(int(k), int(b))]
                      for k, b in zip(kt, bs)], np.int64)
        np.add.at(S, (sit, o * 128 + lanes), vals)
        return (S.astype(sdtype),
                _rep16(idx_flat.astype(np.int16), nslots))

    # ---- L1 schedules ----------------------------------------------------
    sched1 = []
    for d in range(2):
        percore = []
        for ci in range(C):
            adst, asrc = core_edges[ci][d]
            dpos = pos_own[ci][adst]
            row = zrow[ci][asrc]
            assert (row >= 0).all()
            percore.append((dpos // 128, row // WIN, row % WIN, dpos % 128,
                            dinv[d][asrc].astype(np.float32)))
        sched1.append(build_flat(percore, NB, GB1, NW1))

    # ---- L2 mask schedules (halo only; self via direct slice) -----------
    mk_rank = []          # [C] array [OWN] -> rank in masked list or -1
    for ci in range(C):
        lo = ci * OWN
        r = np.full(OWN, -1, np.int64)
        r[mk_nodes[ci] - lo] = np.arange(MK[ci])
        mk_rank.append(r)

    sched2 = []
    for d in range(2):
        percore = []
        for ci in range(C):
            lo = ci * OWN
            ad_g, as_g = (dst, src) if d == 0 else (src, dst)
            sel = ((ad_g >= lo) & (ad_g < lo + OWN)
                   & is_masked[np.clip(ad_g, 0, N - 1)])
            adst = ad_g[sel] - lo
            md = mk_rank[ci][adst]
            sj = as_g[sel] // OWN        # owner core of source
            srow = sj * OWNP + pos_own_of(sj, as_g[sel] - sj * OWN, pos_own)
            percore.append((md // 128, srow // WIN, srow % WIN, md % 128,
                            dinv[d][lo + adst].astype(np.float32)))
        sched2.append(build_flat(percore, MB, GB2, NW2))

    # ---- per-core inputs -------------------------------------------------
    w1all = (np.concatenate([W["on_td_W1"], W["tgt_td_W1"],
                             W["on_bu_W1"], W["tgt_bu_W1"]], axis=1)
             * W1SCALE).astype(f8)
    w2_td = np.concatenate([W["on_td_W2"], W["tgt_td_W2"]], axis=1).astype(bf16)
    w2_bu = np.concatenate([W["on_bu_W2"], W["tgt_bu_W2"]], axis=1).astype(bf16)
    ton = np.concatenate([token @ W["on_td_W1"], token @ W["on_bu_W1"]])
    tonbc = _bcast(ton).astype(bf16)
    b1bc_td = _bcast(np.concatenate([W["on_td_b1"], W["tgt_td_b1"]]))
    b1bc_bu = _bcast(np.concatenate([W["on_bu_b1"], W["tgt_bu_b1"]]))
    b2bc_td = _bcast(np.concatenate([W["on_td_b2"], W["tgt_td_b2"]]))
    b2bc_bu = _bcast(np.concatenate([W["on_bu_b2"], W["tgt_bu_b2"]]))
    b2col = np.stack(
        [W["on_td_b2"], W["tgt_td_b2"], W["on_bu_b2"], W["tgt_bu_b2"]],
        axis=1).astype(np.float32)                         # [64, 4]
    ones = np.ones((128, 1), np.float32)
    gcount = np.bincount(batch, minlength=G).astype(np.float32)
    cntbc = np.broadcast_to(gcount[None, :128], (128, 128)).copy()

    in_maps = []
    for ci in range(C):
        lo = ci * OWN
        # xT in z-row order
        xT = np.zeros((512, RT), f8)
        xT[:, 0:UM[ci]] = x[um_nodes[ci]].T
        xT[:, UMPAD:UMPAD + MK[ci]] = x[mk_nodes[ci]].T
        xT[:, OWNP:OWNP + HU[ci]] = x[halo_um[ci]].T
        xT[:, OWNP + HUPAD:OWNP + HUPAD + HM[ci]] = x[halo_mk[ci]].T

        def dstarr(dv):
            a = np.ones(OWNP, np.float32)
            a[0:UM[ci]] = dv[um_nodes[ci]]
            a[UMPAD:UMPAD + MK[ci]] = dv[mk_nodes[ci]]
            return np.ascontiguousarray(a.reshape(-1, 128).T)

        def colarr(vals_mk, fill=0.0):
            a = np.full(MKPAD, fill, np.float32)
            a[0:MK[ci]] = vals_mk
            return np.ascontiguousarray(a.reshape(-1, 128).T)  # [128, MB]

        m = dict(xT=xT,
                 ddst_td=dstarr(dinv[0]), ddst_bu=dstarr(dinv[1]),
                 swv_td=colarr(dinv[0][mk_nodes[ci]]),
                 swv_bu=colarr(dinv[1][mk_nodes[ci]]),
                 mcvw=colarr(mcnt_global[mk_nodes[ci]]))
        for d, nm in ((0, "td"), (1, "bu")):
            S, idx = fill_slots(sched1[d], ci, f8)
            m[f"s_{nm}1"], m[f"i_{nm}1"] = S, idx
            S2, idx2 = fill_slots(sched2[d], ci, bf16)
            m[f"s2_{nm}"], m[f"i2_{nm}"] = S2, idx2
            # pool S: out-edges of own nodes + self, grouped by graph
            ad, as_ = (dst, src) if d == 0 else (src, dst)
            dv = dinv[d]
            sel = (as_ >= lo) & (as_ < lo + OWN)
            j = pos_own[ci][as_[sel] - lo]
            gg = batch[ad[sel]]
            v = dv[ad[sel]]
            pp = np.zeros((128, NB * 128), np.float32)
            np.add.at(pp, (j % 128, (j // 128) * 128 + gg), v)
            jj = pos_own[ci]
            np.add.at(pp, (jj % 128, (jj // 128) * 128 + batch[lo:lo + OWN]),
                      dv[lo:lo + OWN])
            m[f"pools_{nm}"] = pp.astype(bf16)
        m.update(w1all=w1all, w2_td=w2_td, w2_bu=w2_bu, tonbc=tonbc,
                 b1bc_td=b1bc_td, b1bc_bu=b1bc_bu,
                 b2bc_td=b2bc_td, b2bc_bu=b2bc_bu, b2col=b2col,
                 ones=ones, cntbc=cntbc)
        in_maps.append(m)

    meta = dict(RT=RT, NW1=NW1, NW2=NW2, NB=NB, MB=MB, OWNP=OWNP,
                UMPAD=UMPAD, MKPAD=MKPAD, NPAD=NPAD,
                sections=sections, sched1=sched1, sched2=sched2)
    return meta, in_maps


def pos_own_of(owner_cores, local_idx, pos_own):
    """vectorized pos_own lookup across owner cores"""
    out = np.empty(len(local_idx), np.int64)
    for j in np.unique(owner_cores):
        sel = owner_cores == j
        out[sel] = pos_own[j][local_idx[sel]]
    return out


# ---------------------------------------------------------------- program

def build_program(meta):
    import concourse.bass as bass
    import concourse.bacc as bacc
    import concourse.mybir as mybir
    import concourse.tile as tile
    from concourse.masks import make_identity

    RT, NB, MB = meta["RT"], meta["NB"], meta["MB"]
    NW1, NW2 = meta["NW1"], meta["NW2"]
    OWNP, UMPAD, NPAD = meta["OWNP"], meta["UMPAD"], meta["NPAD"]
    f32, bf, i16 = mybir.dt.float32, mybir.dt.bfloat16, mybir.dt.int16
    f8 = mybir.dt.float8e4
    MUL, ADD = mybir.AluOpType.mult, mybir.AluOpType.add

    nc = bacc.Bacc("TRN2", target_bir_lowering=False, debug=False,
                   num_devices=C)

    def din(name, shape, dt):
        return nc.dram_tensor(name, shape, dt, kind="ExternalInput")

    xT = din("xT", [512, RT], f8)
    ddst = [din("ddst_td", [128, NB], f32), din("ddst_bu", [128, NB], f32)]
    s1 = [din("s_td1", [128, meta["sched1"][0]["NOP"] * 128], f8),
          din("s_bu1", [128, meta["sched1"][1]["NOP"] * 128], f8)]
    i1 = [din("i_td1", [128, meta["sched1"][0]["TOTKT"] * 8], i16),
          din("i_bu1", [128, meta["sched1"][1]["TOTKT"] * 8], i16)]
    s2 = [din("s2_td", [128, meta["sched2"][0]["NOP"] * 128], bf),
          din("s2_bu", [128, meta["sched2"][1]["NOP"] * 128], bf)]
    i2 = [din("i2_td", [128, meta["sched2"][0]["TOTKT"] * 8], i16),
          din("i2_bu", [128, meta["sched2"][1]["TOTKT"] * 8], i16)]
    pools_t = [din("pools_td", [128, NB * 128], bf),
               din("pools_bu", [128, NB * 128], bf)]
    swv_t = [din("swv_td", [128, MB], f32), din("swv_bu", [128, MB], f32)]
    mcvw_t = din("mcvw", [128, MB], f32)
    w1all = din("w1all", [512, 512], f8)
    w2 = [din("w2_td", [128, 128], bf), din("w2_bu", [128, 128], bf)]
    tonbc = din("tonbc", [128, 256], bf)
    b1bc = [din("b1bc_td", [128, 256], f32), din("b1bc_bu", [128, 256], f32)]
    b2bc = [din("b2bc_td", [128, 128], f32), din("b2bc_bu", [128, 128], f32)]
    b2col_t = din("b2col", [64, 4], f32)
    ones_t = din("ones", [128, 1], f32)
    cntbc_t = din("cntbc", [128, 128], f32)
    loss_t = nc.dram_tensor("loss", [1, 1], f32, kind="ExternalOutput")

    z_ws = [nc.dram_tensor(f"zarr{w}", [min(WIN, RT - w * WIN), 512], f8,
                           kind="Internal")
            for w in range(NW1)]

    with tile.TileContext(nc) as tc:
        with (
            tc.tile_pool(name="const", bufs=1) as cpool,
            tc.tile_pool(name="dram", bufs=1, space="DRAM") as dpool,
        ):
            z2own = [dpool.tile([OWNP, 128], bf, tag=f"z2own{d}",
                                name=f"z2own{d}") for d in range(2)]
            z2full = [dpool.tile([NPAD, 128], bf, addr_space="Shared",
                                 tag=f"z2full{d}", name=f"z2full{d}")
                      for d in range(2)]
            ar_in = dpool.tile([128, 520], f32, tag="arin", name="arin")
            ar_out = dpool.tile([128, 520], f32, addr_space="Shared",
                                tag="arout", name="arout")

            ident = cpool.tile([128, 128], bf)
            make_identity(nc, ident[:])
            w1sb = cpool.tile([128, 4 * 512], f8)
            for k in range(4):
                nc.sync.dma_start(out=w1sb[:, k * 512:(k + 1) * 512],
                                  in_=w1all[k * 128:(k + 1) * 128, :])
            w2sb = [cpool.tile([128, 128], bf, tag=f"w2_{d}", name=f"w2_{d}")
                    for d in range(2)]
            tonsb = cpool.tile([128, 256], bf)
            b1sb = [cpool.tile([128, 256], f32, tag=f"b1_{d}", name=f"b1_{d}")
                    for d in range(2)]
            b2sb = [cpool.tile([128, 128], f32, tag=f"b2_{d}", name=f"b2_{d}")
                    for d in range(2)]
            ddsb = [cpool.tile([128, NB], f32, tag=f"dd_{d}", name=f"dd_{d}")
                    for d in range(2)]
            swsb = [cpool.tile([128, MB], f32, tag=f"sw_{d}", name=f"sw_{d}")
                    for d in range(2)]
            mcsb = cpool.tile([128, MB], f32)
            onesb = cpool.tile([128, 1], f32)
            nc.sync.dma_start(out=tonsb[:], in_=tonbc[:, :])
            nc.sync.dma_start(out=onesb[:], in_=ones_t[:, :])
            nc.sync.dma_start(out=mcsb[:], in_=mcvw_t[:, :])
            for d in range(2):
                nc.sync.dma_start(out=w2sb[d][:], in_=w2[d][:, :])
                nc.sync.dma_start(out=b1sb[d][:], in_=b1bc[d][:, :])
                nc.sync.dma_start(out=b2sb[d][:], in_=b2bc[d][:, :])
                nc.sync.dma_start(out=ddsb[d][:], in_=ddst[d][:, :])
                nc.sync.dma_start(out=swsb[d][:], in_=swv_t[d][:, :])

            # ================= P1: z = scaled([x1|x] @ W1-fused) ==========
            with (
                tc.tile_pool(name="xk", bufs=5) as xkp,
                tc.tile_pool(name="zsb", bufs=10) as zsp,
                tc.tile_pool(name="pz", bufs=6, space="PSUM") as pzp,
            ):
                DR = mybir.MatmulPerfMode.DoubleRow
                jpar = 0
                for (r0, rlen, msk) in meta["sections"]:
                    for off in range(0, rlen, NF):
                        nf = min(NF, rlen - off)
                        xk = xkp.tile([128, 4 * NF], f8, tag="xk", name="xk")
                        for k in range(4):
                            nc.sync.dma_start(
                                out=xk[:, k * NF:k * NF + nf],
                                in_=xT[k * 128:(k + 1) * 128,
                                       r0 + off:r0 + off + nf])
                        xk3 = xk[:].rearrange("p (k n) -> p k n", k=4, n=NF)
                        w13 = w1sb[:].rearrange("p (k n) -> p k n", k=4,
                                                n=512)
                        for j in range(nf // 128):
                            row = r0 + off + j * 128
                            jpar += 1
                            zs = zsp.tile([128, 512], f8, tag="zs", name="zs")
                            if not msk:
                                ps = pzp.tile([128, 512], f32, tag="pz",
                                              name="pz")
                                for k in range(0, 4, 2):
                                    nc.tensor.matmul(
                                        out=ps[:],
                                        lhsT=xk3[:, k:k + 2,
                                                 j * 128:(j + 1) * 128],
                                        rhs=w13[:, k:k + 2, :],
                                        start=(k == 0), stop=(k == 2),
                                        perf_mode=DR)
                                if jpar % 3 == 0:
                                    nc.scalar.activation(
                                        out=zs[:], in_=ps[:],
                                        func=mybir.ActivationFunctionType.Copy,
                                        scale=1.0 / W1SCALE)
                                else:
                                    nc.vector.tensor_scalar(
                                        out=zs[:], in0=ps[:],
                                        scalar1=1.0 / W1SCALE,
                                        scalar2=None, op0=MUL)
                            else:
                                ps = pzp.tile([128, 512], f32, tag="pz",
                                              name="pz")
                                for h in range(2):
                                    c0 = h * 256 + 128
                                    for k in range(0, 4, 2):
                                        nc.tensor.matmul(
                                            out=ps[:, h * 128:(h + 1) * 128],
                                            lhsT=xk3[:, k:k + 2,
                                                     j * 128:(j + 1) * 128],
                                            rhs=w13[:, k:k + 2, c0:c0 + 128],
                                            start=(k == 0), stop=(k == 2),
                                            perf_mode=DR)
                                for h in range(2):
                                    nc.vector.tensor_copy(
                                        out=zs[:, h * 256:h * 256 + 128],
                                        in_=tonsb[:, h * 128:(h + 1) * 128])
                                    if jpar % 3 == 0:
                                        nc.scalar.activation(
                                            out=zs[:, h * 256 + 128:
                                                   (h + 1) * 256],
                                            in_=ps[:, h * 128:(h + 1) * 128],
                                            func=mybir.ActivationFunctionType.Copy,
                                            scale=1.0 / W1SCALE)
                                    else:
                                        nc.vector.tensor_scalar(
                                            out=zs[:, h * 256 + 128:
                                                   (h + 1) * 256],
                                            in0=ps[:, h * 128:(h + 1) * 128],
                                            scalar1=1.0 / W1SCALE,
                                            scalar2=None, op0=MUL)
                            zw = row // WIN
                            zr = row - zw * WIN
                            weng = nc.scalar if jpar % 3 == 1 else nc.sync
                            weng.dma_start(out=z_ws[zw][zr:zr + 128, :],
                                           in_=zs[:])

            # ===== L1 agg + finalize (z2 + fused pool), per direction =====
            poolpool_cm = tc.tile_pool(name="plps", bufs=1, space="PSUM")
            poolpool = poolpool_cm.__enter__()
            poolps = poolpool.tile([128, 512], f32, tag="pl", name="pl")

            def l1_dir(d):
                sch = meta["sched1"][d]
                wlen = lambda w: min(WIN, RT - w * WIN)
                with (
                    tc.tile_pool(name=f"g1{d}", bufs=4) as gp,
                    tc.tile_pool(name=f"sI1{d}", bufs=3) as sp,
                    tc.tile_pool(name=f"ix1{d}", bufs=1) as ip,
                    tc.tile_pool(name=f"ps1{d}", bufs=2) as pwp,
                    tc.tile_pool(name=f"fin1{d}", bufs=3) as fp,
                    tc.tile_pool(name=f"h1q{d}", bufs=20) as h1p,
                    tc.tile_pool(name=f"zrb{d}", bufs=3) as zrp,
                    tc.tile_pool(name=f"agg{d}", bufs=1, space="PSUM") as ap,
                    tc.tile_pool(name=f"tr{d}", bufs=2, space="PSUM") as trp,
                    tc.tile_pool(name=f"z2p{d}", bufs=1, space="PSUM") as z2p,
                ):
                    # stage B (transpose -> @W2 -> scale -> z2own write +
                    # fused pool matmuls), decoupled from the agg pipeline
                    # via the deep h1 tile pool and one-group emission skew.
                    def stage_b(blk, bi, h1, pst):
                        trt = trp.tile([128, 256], bf, tag="t", name="t")
                        nc.tensor.transpose(
                            out=trt[:, 0:128], in_=h1[:, 0:128],
                            identity=ident[:])
                        nc.tensor.transpose(
                            out=trt[:, 128:256], in_=h1[:, 128:256],
                            identity=ident[:])
                        h1T = fp.tile([128, 256], bf, tag="h1T", name="h1T")
                        nc.scalar.copy(out=h1T[:], in_=trt[:])
                        z2ps = z2p.tile([128, 128], f32, tag="z2", name="z2")
                        nc.tensor.matmul(out=z2ps[:, 0:64],
                                         lhsT=h1T[:, 0:128],
                                         rhs=w2sb[d][:, 0:64],
                                         start=True, stop=True)
                        nc.tensor.matmul(out=z2ps[:, 64:128],
                                         lhsT=h1T[:, 128:256],
                                         rhs=w2sb[d][:, 64:128],
                                         start=True, stop=True)
                        z2sb = fp.tile([128, 128], bf, tag="z2sb",
                                       name="z2sb")
                        nc.vector.tensor_scalar(
                            out=z2sb[:], in0=z2ps[:],
                            scalar1=ddsb[d][:, blk:blk + 1],
                            scalar2=None, op0=MUL)
                        nc.sync.dma_start(
                            out=z2own[d][blk * 128:(blk + 1) * 128, :],
                            in_=z2sb[:])
                        nc.tensor.matmul(
                            out=poolps[0:64, d * 256:d * 256 + 128],
                            lhsT=z2sb[:, 0:64],
                            rhs=pst[:, bi * 128:(bi + 1) * 128],
                            start=(blk == 0), stop=(blk == NB - 1),
                            skip_group_check=True)
                        nc.tensor.matmul(
                            out=poolps[0:64, d * 256 + 128:d * 256 + 256],
                            lhsT=z2sb[:, 64:128],
                            rhs=pst[:, bi * 128:(bi + 1) * 128],
                            start=(blk == 0), stop=(blk == NB - 1),
                            skip_group_check=True)

                    itall = ip.tile([128, max(sch["TOTKT"], 1) * 8], i16,
                                    tag="ia", name="ia")
                    nc.gpsimd.dma_start(out=itall[:], in_=i1[d][:, :])
                    pending = []
                    if True:
                        for g, grp in enumerate(sch["groups"]):
                            blocks = grp["blocks"]
                            nops = len(grp["mops"])
                            gt = None
                            if grp["gops"]:
                                gt = gp.tile([128, grp["nk"] * 256], f8,
                                             tag="g", name="g")
                                for (ww, ktb, nkw) in grp["gops"]:
                                    o = ktb - grp["kt_lo"]
                                    nc.gpsimd.dma_gather(
                                        gt[:, o * 256:(o + nkw) * 256]
                                        .rearrange("p (k e) -> p k e",
                                                   k=nkw, e=256),
                                        z_ws[ww][0:wlen(ww),
                                                 256 * d:256 * d + 256],
                                        itall[:, ktb * 8:(ktb + nkw) * 8],
                                        nkw * 128, nkw * 128, 256,
                                        elem_step=512, single_packet=False)
                            if nops:
                                st = sp.tile([128, nops * 128], f8, tag="s",
                                             name="s")
                                nc.scalar.dma_start(
                                    out=st[:],
                                    in_=s1[d][:, grp["mops"][0][2] * 128:
                                              (grp["mops"][0][2] + nops)
                                              * 128])
                                aps = ap.tile([128, len(blocks) * 256], f32,
                                              tag="a", name="a")
                                o0 = grp["mops"][0][2]
                                for (kt, blk, o, st_f, sp_f) in grp["mops"]:
                                    bi = blk - blocks[0]
                                    nc.tensor.matmul(
                                        out=aps[:, bi * 256:(bi + 1) * 256],
                                        lhsT=st[:, (o - o0) * 128:
                                                (o - o0 + 1) * 128],
                                        rhs=gt[:, (kt - grp["kt_lo"]) * 256:
                                               (kt - grp["kt_lo"] + 1)
                                               * 256],
                                        start=st_f, stop=sp_f,
                                        skip_group_check=True)
                            has = {blk for (_, blk, _, _, _) in grp["mops"]}
                            # pool S slab for this group
                            pst = pwp.tile([128, len(blocks) * 128], bf,
                                           tag="ps", name="ps")
                            nc.sync.dma_start(
                                out=pst[:],
                                in_=pools_t[d][:, blocks[0] * 128:
                                               (blocks[0] + len(blocks))
                                               * 128])
                            newly = []
                            for blk in blocks:
                                bi = blk - blocks[0]
                                zrb = zrp.tile([128, 256], f8, tag="zr",
                                               name="zr")
                                nc.sync.dma_start(
                                    out=zrb[:],
                                    in_=z_ws[0][blk * 128:(blk + 1) * 128,
                                                256 * d:256 * d + 256])
                                hs = fp.tile([128, 256], f32, tag="hs",
                                             name="hs")
                                if blk in has:
                                    # hs = h_self*dinv_dst + agg
                                    nc.vector.scalar_tensor_tensor(
                                        out=hs[:], in0=zrb[:],
                                        scalar=ddsb[d][:, blk:blk + 1],
                                        in1=aps[:, bi * 256:(bi + 1) * 256],
                                        op0=MUL, op1=ADD)
                                else:
                                    nc.vector.tensor_scalar(
                                        out=hs[:], in0=zrb[:],
                                        scalar1=ddsb[d][:, blk:blk + 1],
                                        scalar2=None, op0=MUL)
                                # h1 = relu(hs*ddst + b1)
                                nc.vector.scalar_tensor_tensor(
                                    out=hs[:], in0=hs[:],
                                    scalar=ddsb[d][:, blk:blk + 1],
                                    in1=b1sb[d][:, 0:256], op0=MUL, op1=ADD)
                                h1 = h1p.tile([128, 256], bf, tag="h1",
                                              name="h1")
                                nc.scalar.activation(
                                    out=h1[:], in_=hs[:],
                                    func=mybir.ActivationFunctionType.Relu)
                                newly.append((blk, bi, h1, pst))
                            for item in pending:
                                stage_b(*item)
                            pending = newly
                    for item in pending:
                        stage_b(*item)

            def allgather(d):
                nc.gpsimd.collective_compute(
                    "AllGather", mybir.AluOpType.bypass,
                    replica_groups=[list(range(C))],
                    ins=[z2own[d].opt()], outs=[z2full[d].opt()])

            l1_dir(0)
            allgather(0)
            l1_dir(1)
            allgather(1)

            # drain pooled sums
            arsb = cpool.tile([128, 520], f32)
            nc.vector.memset(arsb[:], 0.0)
            for d in range(2):
                nc.vector.tensor_copy(out=arsb[0:64, d * 256:(d + 1) * 256],
                                      in_=poolps[0:64, d * 256:(d + 1) * 256])
            poolpool_cm.__exit__(None, None, None)

            # ========== L2 mask aggregation (node-major) ==================
            # wide per-dir product tiles
            prodw = [[cpool.tile([128, MB], f32, tag=f"pw{d}{q}",
                                 name=f"pw{d}{q}") for q in range(3)]
                     for d in range(2)]
            for d in range(2):
                for q in range(3):
                    nc.vector.memset(prodw[d][q][:], 0.0)

            def l2_dir(d):
                sch = meta["sched2"][d]
                wlen = lambda w: min(WIN, NPAD - w * WIN)
                with (
                    tc.tile_pool(name=f"g2{d}", bufs=3) as gp,
                    tc.tile_pool(name=f"sI2{d}", bufs=3) as sp,
                    tc.tile_pool(name=f"ix2{d}", bufs=3) as ip,
                    tc.tile_pool(name=f"fin2{d}", bufs=3) as fp,
                    tc.tile_pool(name=f"zsl{d}", bufs=3) as zp,
                    tc.tile_pool(name=f"mag{d}", bufs=2, space="PSUM") as ap,
                ):
                    itall2 = ip.tile([128, max(sch["TOTKT"], 1) * 8],
                                     i16, tag="ia2", name="ia2")
                    nc.gpsimd.dma_start(out=itall2[:], in_=i2[d][:, :])
                    for g, grp in enumerate(sch["groups"]):
                        blocks = grp["blocks"]
                        nops = len(grp["mops"])
                        nk = grp["nk"]
                        if nops:
                            st = sp.tile([128, nops * 128], bf, tag="s",
                                         name="s")
                            nc.scalar.dma_start(
                                out=st[:],
                                in_=s2[d][:, grp["mops"][0][2] * 128:
                                          (grp["mops"][0][2] + nops) * 128])
                            gt = gp.tile([128, nk * 128], bf, tag="g",
                                         name="g")
                            for w, ktb, nkw in grp["gops"]:
                                o = ktb - grp["kt_lo"]
                                nc.gpsimd.dma_gather(
                                    gt[:, o * 128:(o + nkw) * 128].rearrange(
                                        "p (k e) -> p k e", k=nkw, e=128),
                                    z2full[d][w * WIN:w * WIN + wlen(w), :],
                                    itall2[:, ktb * 8:(ktb + nkw) * 8],
                                    nkw * 128, nkw * 128, 128,
                                    elem_step=None, single_packet=False)
                            aps = ap.tile([128, len(blocks) * 128], f32,
                                          tag="a", name="a")
                            o0 = grp["mops"][0][2]
                            for (kt, blk, o, st_f, sp_f) in grp["mops"]:
                                bi = blk - blocks[0]
                                nc.tensor.matmul(
                                    out=aps[:, bi * 128:(bi + 1) * 128],
                                    lhsT=st[:, (o - o0) * 128:
                                            (o - o0 + 1) * 128],
                                    rhs=gt[:, (kt - grp["kt_lo"]) * 128:
                                           (kt - grp["kt_lo"] + 1) * 128],
                                    start=st_f, stop=sp_f,
                                    skip_group_check=True)
                        has = {blk for (_, blk, _, _, _) in grp["mops"]}
                        for blk in blocks:
                            bi = blk - blocks[0]
                            zsl = zp.tile([128, 128], bf, tag="zs", name="zs")
                            nc.scalar.dma_start(
                                out=zsl[:],
                                in_=z2own[d][UMPAD + blk * 128:
                                             UMPAD + (blk + 1) * 128, :])
                            hs = fp.tile([128, 128], f32, tag="hs", name="hs")
                            # hs = z_self*swv (+ agg)
                            if blk in has:
                                nc.vector.scalar_tensor_tensor(
                                    out=hs[:], in0=zsl[:],
                                    scalar=swsb[d][:, blk:blk + 1],
                                    in1=aps[:, bi * 128:(bi + 1) * 128],
                                    op0=MUL, op1=ADD)
                            else:
                                nc.vector.tensor_scalar(
                                    out=hs[:], in0=zsl[:],
                                    scalar1=swsb[d][:, blk:blk + 1],
                                    scalar2=None, op0=MUL)
                            nc.vector.tensor_tensor(
                                out=hs[:], in0=hs[:], in1=b2sb[d][:, 0:128],
                                op=ADD)
                            # products (accumulate over 64-feat free dim)
                            scr = fp.tile([128, 64], f32, tag="sc", name="sc")
                            for q, (p0, p1) in enumerate(
                                    ((0, 64), (0, 0), (64, 64))):
                                nc.vector.scalar_tensor_tensor(
                                    out=scr[:], in0=hs[:, p0:p0 + 64],
                                    scalar=1.0, in1=hs[:, p1:p1 + 64],
                                    op0=MUL, op1=MUL,
                                    accum_out=prodw[d][q][:, blk:blk + 1])

            l2_dir(0)
            l2_dir(1)

            # ========== masked SCE epilogue (wide) ========================
            with tc.tile_pool(name="ep", bufs=1) as ep:
                su = [ep.tile([128, MB], f32, tag=f"su{q}", name=f"su{q}")
                      for q in range(3)]
                for q in range(3):
                    nc.vector.tensor_tensor(out=su[q][:], in0=prodw[0][q][:],
                                            in1=prodw[1][q][:], op=ADD)

                def rsq(n, tag):
                    r = ep.tile([128, MB], f32, tag=tag, name=tag)
                    nc.scalar.sqrt(out=r[:], in_=n[:])
                    nc.vector.tensor_scalar_max(out=r[:], in0=r[:],
                                                scalar1=1e-12)
                    nc.vector.reciprocal(out=r[:], in_=r[:])
                    return r

                r1 = rsq(su[1], "r1")
                r2 = rsq(su[2], "r2")
                tt = ep.tile([128, MB], f32, tag="tt", name="tt")
                nc.vector.tensor_tensor(out=tt[:], in0=su[0][:], in1=r1[:],
                                        op=MUL)
                nc.vector.tensor_tensor(out=tt[:], in0=tt[:], in1=r2[:],
                                        op=MUL)
                nc.vector.tensor_tensor(out=tt[:], in0=tt[:], in1=mcsb[:],
                                        op=MUL)
                scr = ep.tile([128, MB], f32, tag="scr", name="scr")
                colsum = ep.tile([128, 1], f32, tag="cs", name="cs")
                nc.vector.scalar_tensor_tensor(
                    out=scr[:], in0=tt[:], scalar=-1.0, in1=mcsb[:],
                    op0=MUL, op1=ADD, accum_out=colsum[:])
                with tc.tile_pool(name="eps", bufs=1, space="PSUM") as epp:
                    macc_ps = epp.tile([1, 1], f32, tag="mp", name="mp")
                    nc.tensor.matmul(out=macc_ps[:], lhsT=colsum[:],
                                     rhs=onesb[:], start=True, stop=True)
                    nc.vector.tensor_copy(out=arsb[0:1, 512:513],
                                          in_=macc_ps[:])

            # ========== AllReduce (pools + mask partial) =================
            nc.sync.dma_start(out=ar_in[:, :], in_=arsb[:])
            nc.gpsimd.collective_compute(
                "AllReduce", mybir.AluOpType.add,
                replica_groups=[list(range(C))],
                ins=[ar_in.opt()], outs=[ar_out.opt()])

            # ========== pooled cosine + final loss =======================
            with (
                tc.tile_pool(name="fin3", bufs=2) as f2,
                tc.tile_pool(name="fps", bufs=2, space="PSUM") as fpp,
            ):
                ar2 = f2.tile([128, 520], f32, tag="ar2", name="ar2")
                nc.sync.dma_start(out=ar2[:], in_=ar_out[:, :])
                cntsb = f2.tile([128, 128], f32, tag="cnt", name="cnt")
                nc.sync.dma_start(out=cntsb[:], in_=cntbc_t[:, :])
                b2t = f2.tile([64, 4], f32, tag="b2tf", name="b2tf")
                nc.sync.dma_start(out=b2t[:], in_=b2col_t[:, :])
                pf = {}
                for d in range(2):
                    for h in range(2):
                        po = f2.tile([64, 128], f32, tag=f"po{d}{h}",
                                     name=f"po{d}{h}")
                        nc.vector.scalar_tensor_tensor(
                            out=po[:], in0=cntsb[0:64, :],
                            scalar=b2t[0:64, 2 * d + h:2 * d + h + 1],
                            in1=ar2[0:64, d * 256 + h * 128:
                                    d * 256 + (h + 1) * 128],
                            op0=MUL, op1=ADD)
                        pf[(d, h)] = po
                gsums = []
                for qi, pick in enumerate(((0, 1), (0, 0), (1, 1))):
                    qp = fpp.tile([1, 128], f32, tag="gqp", name="gqp")
                    for d in range(2):
                        pr = f2.tile([64, 128], f32, tag=f"gpr{d}",
                                     name=f"gpr{d}")
                        nc.vector.tensor_tensor(
                            out=pr[:], in0=pf[(d, pick[0])][:],
                            in1=pf[(d, pick[1])][:], op=MUL)
                        nc.tensor.matmul(
                            out=qp[:], lhsT=onesb[0:64, 0:1], rhs=pr[:],
                            start=(d == 0), stop=(d == 1),
                            skip_group_check=True)
                    sq = f2.tile([1, 128], f32, tag=f"gsq{qi}",
                                 name=f"gsq{qi}")
                    nc.vector.tensor_copy(out=sq[:], in_=qp[:])
                    gsums.append(sq)
                gdot, gn1, gn2 = gsums

                def rguard2(n, tag):
                    r = f2.tile([1, 128], f32, tag=tag, name=tag)
                    nc.scalar.sqrt(out=r[:], in_=n[:])
                    nc.vector.tensor_scalar_max(out=r[:], in0=r[:],
                                                scalar1=1e-12)
                    nc.vector.reciprocal(out=r[:], in_=r[:])
                    return r

                g1 = rguard2(gn1, "g1")
                g2 = rguard2(gn2, "g2")
                cosg = f2.tile([1, 128], f32, tag="cosg", name="cosg")
                nc.vector.tensor_tensor(out=cosg[:], in0=gdot[:], in1=g1[:],
                                        op=MUL)
                nc.vector.tensor_tensor(out=cosg[:], in0=cosg[:], in1=g2[:],
                                        op=MUL)
                onesrow = f2.tile([1, 128], f32, tag="onesr", name="onesr")
                nc.vector.memset(onesrow[:], 1.0)
                gterm = f2.tile([1, 128], f32, tag="gterm", name="gterm")
                gs = f2.tile([1, 1], f32, tag="gs", name="gs")
                nc.vector.scalar_tensor_tensor(
                    out=gterm[:], in0=cosg[:], scalar=-1.0, in1=onesrow[:],
                    op0=MUL, op1=ADD, accum_out=gs[:])
                l1t = f2.tile([1, 1], f32, tag="l1", name="l1")
                nc.scalar.activation(out=l1t[:], in_=gs[:],
                                     func=mybir.ActivationFunctionType.Copy,
                                     scale=1.0 / G)
                l2t = f2.tile([1, 1], f32, tag="l2", name="l2")
                nc.scalar.activation(out=l2t[:], in_=ar2[0:1, 512:513],
                                     func=mybir.ActivationFunctionType.Copy,
                                     scale=1.0 / M)
                nc.vector.tensor_tensor(out=l1t[:], in0=l1t[:], in1=l2t[:],
                                        op=ADD)
                nc.sync.dma_start(out=loss_t[:, :], in_=l1t[:])

    return nc


# ---------------------------------------------------------------- entry

LAST_RESULT = None


def _install_trace_hook():
    """The agent image's antenv lacks axon_hooks; synthesize it from
    trn_boot's ctypes NTFF hook so trace=True works under axon."""
    import types
    try:
        from antenv import axon_hooks  # noqa: F401
        return
    except ImportError:
        pass
    try:
        import antenv
        import trn_agent_boot.trn_boot as tb
        hook = tb._ntff_profile_via_ctypes("/opt/axon/libaxon_pjrt.so")
        mod = types.ModuleType("antenv.axon_hooks")
        mod.get_axon_ntff_profile_hook = lambda: hook
        mod.set_axon_ntff_profile_hook = lambda h: None
        sys.modules["antenv.axon_hooks"] = mod
        antenv.axon_hooks = mod
    except Exception as e:
        print(f"[kernel] trace hook install failed: {e}", file=sys.stderr)


def kernel(_trace=False, **inputs):
    global LAST_RESULT
    import time
    from concourse import bass_utils
    if _trace:
        _install_trace_hook()
    t0 = time.monotonic()
    meta, in_maps = host_prep(inputs)
    t1 = time.monotonic()
    nc = build_program(meta)
    t2 = time.monotonic()
    nc.compile()
    t3 = time.monotonic()
    res = bass_utils.run_bass_kernel_spmd(
        nc, in_maps, core_ids=list(range(C)),
        trace=_trace, trace_cores=[0] if _trace else None)
    t4 = time.monotonic()
    print(f"[kernel] prep {t1-t0:.1f}s build {t2-t1:.1f}s "
          f"compile {t3-t2:.1f}s run {t4-t3:.1f}s", file=sys.stderr)
    LAST_RESULT = res
    return np.float32(res.results[0]["loss"][0, 0])
